# revision 1
# baseline (speedup 1.0000x reference)
"""Trainium2 Bass kernel for nn_DecoderBlock (shape-guided RWKV decoder block).

Data-parallel over batch: B=8 samples -> 8 NeuronCores, one NEFF.

Per-core layout: channels on partitions (256ch -> 2 "ctiles" of 128), spatial
(h, w) flattened on the free dim (4096).

- LayerNorm over channels: square (ACT) -> ones-matmul partition reduction ->
  DRAM-bounce reshape -> tiny stat math -> K=1 matmul broadcast -> TT applies.
- q_shift / mask blend via shifted access patterns; per-channel mixes folded
  into Wk/Wv/Wr host-side (k = Wk@xn + (Wk*diag(1-mk))@md, md = mask*(xs-xn)).
- WKV: unstabilized linear recurrence A_t = lam*A_{t-1} + e^{k_t} v_t via the
  DVE TensorTensorScan instruction chained across rows (data0=0 resets at each
  row start); vertical orientation scans read E/EV through transposed APs.
- channel_fusion: grouped 3x3 conv = 9 shifted-AP matmuls accumulated in PSUM
  over zero-padded [c, 66, 66] inputs; BN folded into the next conv
  host-side; GELU+bias fused into the PSUM->SBUF evacuation on ACT.
- patch_expand: up-proj rows permuted host-side so pixel shuffle becomes a
  strided DMA.

Matmuls in float32r (full rate) or bf16; bulky intermediates bf16.
"""
import sys
import os

for _p in ('/opt/trn_rl_repo', '/root/.axon_site/_ro/trn_rl_repo'):
    if _p not in sys.path and os.path.isdir(_p):
        sys.path.append(_p)

import numpy as np

B, C, CS, COUT, H, W = 8, 256, 512, 128, 64, 64
S = H * W          # 4096
NCH = 8            # spatial chunks
CH = S // NCH      # 512
EPS = 1e-5

_CACHE = {}


def _build(weights, probe=False):
    const_inputs = {}
    import concourse.bass as bass
    from concourse import bacc
    import concourse.tile as tile
    import concourse.mybir as mybir
    import ml_dtypes

    F32 = mybir.dt.float32
    F32R = mybir.dt.float32r
    BF16 = mybir.dt.bfloat16
    Alu = mybir.AluOpType
    Act = mybir.ActivationFunctionType
    MM, AD, SU = Alu.mult, Alu.add, Alu.subtract

    w = weights
    f64 = lambda x: np.asarray(x, np.float64)
    bf = lambda a: np.asarray(a, dtype=ml_dtypes.bfloat16)

    # ---------------- host-side folding
    bnscale = 1.0 / np.sqrt(1.0 + EPS)
    g1p = f64(w['bn1_g']) * bnscale
    b1p = f64(w['bn1_b'])
    g2p = f64(w['bn2_g']) * bnscale
    b2p = f64(w['bn2_b'])
    g3p = (f64(w['bn3_g']) * bnscale).astype(np.float32)
    b3p = f64(w['bn3_b']).astype(np.float32)

    c2_eff = f64(w['c2_w']) * g1p[None, :]
    c2b_eff = (f64(w['c2_b']) + f64(w['c2_w']) @ b1p).astype(np.float32)
    c3_eff = f64(w['c3_w']) * g2p[None, :]
    c3b_eff = (f64(w['c3_b']) + f64(w['c3_w']) @ b2p).astype(np.float32)

    wk_x = f64(w['Wk']).T
    wk_d = (f64(w['Wk']) * (1.0 - f64(w['mix_k']))[None, :]).T
    wv_x = f64(w['Wv']).T
    wv_d = (f64(w['Wv']) * (1.0 - f64(w['mix_v']))[None, :]).T
    wr_x = f64(w['Wr']).T
    wr_d = (f64(w['Wr']) * (1.0 - f64(w['mix_r']))[None, :]).T
    wo_t = f64(w['Wo']).T
    sp_t = f64(w['sp_w']).T.astype(np.float32)

    lam = np.exp(-np.exp(f64(w['decay']))).astype(np.float32)
    lam64 = np.tile(lam[:, None], (1, 64))
    lam64[:, 0] = 0.0
    lam64 = lam64.astype(np.float32)
    eu = np.exp(f64(w['first'])).astype(np.float32)

    pidx = np.arange(512)
    old = (pidx % 128) * 4 + (pidx // 128)
    up_t = f64(w['up_w'])[old].T.astype(np.float32)             # [256, 512]
    upb_p = f64(w['up_b'])[old].astype(np.float32)

    c1w = f64(w['c1_w'])
    c1_l = np.zeros((9, 2, 256, 256), np.float32)
    for ti in range(9):
        dy, dx = ti // 3, ti % 3
        for g in range(2):
            c1_l[ti, g] = c1w[g * 256:(g + 1) * 256, :, dy, dx].T

    # per-channel vectors as columns of one [128, ncol] const
    cols, order = {}, []

    def addcol(name, vec):
        v = np.asarray(vec, np.float32).reshape(-1, 128)
        cols[name] = v
        order.append(name)

    addcol('eu', eu)
    addcol('ln1w', w['ln1_w'])
    addcol('ln1b', w['ln1_b'])
    addcol('knw', w['kn_w'])
    addcol('knb', w['kn_b'])
    addcol('ln2w', w['ln2_w'])
    addcol('ln2b', w['ln2_b'])
    addcol('g3p', g3p)
    addcol('b3p', b3p)
    addcol('spb', w['sp_b'])
    addcol('c3b', c3b_eff)
    addcol('c1b', w['c1_b'])
    addcol('c2b', c2b_eff)
    addcol('upb', upb_p)
    colidx, ncol = {}, 0
    for n in order:
        colidx[n] = ncol
        ncol += cols[n].shape[0]
    cvec_np = np.zeros((128, ncol), np.float32)
    for n in order:
        for i in range(cols[n].shape[0]):
            cvec_np[:, colidx[n] + i] = cols[n][i]

    ln1_triv = np.all(w['ln1_w'] == 1.0) and np.all(w['ln1_b'] == 0.0)
    kn_triv = np.all(w['kn_w'] == 1.0) and np.all(w['kn_b'] == 0.0)
    ln2_triv = np.all(w['ln2_w'] == 1.0) and np.all(w['ln2_b'] == 0.0)
    bn3_triv = np.all(g3p == g3p[0]) and np.all(b3p == 0.0)
    # uniform bn3 scale commutes with LN2 -> drop it entirely when trivial

    # ---------------- bass module
    nc = bacc.Bacc("TRN2", target_bir_lowering=False, debug=False, name="decblk")

    xin = nc.dram_tensor("xin", [C, S], F32, kind="ExternalInput")
    skin = nc.dram_tensor("skin", [CS, S], F32, kind="ExternalInput")
    mrow = nc.dram_tensor("mrow", [1, S], F32, kind="ExternalInput")
    yout = nc.dram_tensor("yout", [COUT, 2 * H, 2 * W], F32, kind="ExternalOutput")
    probes = {}

    def mkprobe(name, shape):
        if probe:
            probes[name] = nc.dram_tensor(name, shape, F32, kind="ExternalOutput")
        return probes.get(name)

    def it(arr, name):
        arr = np.ascontiguousarray(arr)
        import ml_dtypes as _md
        dt_ = {np.dtype(np.float32): F32, np.dtype(_md.bfloat16): BF16}[arr.dtype]
        const_inputs[name] = arr
        return nc.dram_tensor(name, list(arr.shape), dt_, kind="ExternalInput")
    d_lam = it(lam64, "lam64")
    d_cvec = it(cvec_np, "cvec")
    d_wk = [it(bf(wk_x), "wkx"), it(bf(wk_d), "wkd")]
    d_wv = [it(bf(wv_x), "wvx"), it(bf(wv_d), "wvd")]
    d_wr = [it(bf(wr_x), "wrx"), it(bf(wr_d), "wrd")]
    d_wo = it(bf(wo_t), "wo")
    d_sp = it(sp_t, "sp")
    d_up = it(up_t, "up")
    c1_r = c1_l.reshape(9, 2, 2, 128, 2, 128).transpose(1, 4, 3, 0, 2, 5)
    d_c1 = it(bf(c1_r), "c1")   # [g, mt, p, t, kt, m]
    d_c2 = it(bf(c2_eff.T), "c2")
    d_c3 = it(bf(c3_eff.T), "c3")
    red_np = np.zeros((128, 2, 2), np.float32)
    red_np[:, 0, 0] = 1.0
    red_np[:, 1, 1] = 1.0
    d_red = it(red_np, "red")
    d_redb = it(bf(red_np), "redb")
    bc2_np = np.zeros((2, 2, 128), np.float32)
    bc2_np[0, 0, :] = 1.0
    bc2_np[1, 1, :] = 1.0
    d_bc1 = it(bc2_np, "bc2")
    d_eps = it(np.full((128, 1), EPS, np.float32), "epsc")

    def scan_raw(out, d0, d1):
        eng = nc.vector
        if os.environ.get('BASSK_NOSCAN'):
            return eng.tensor_copy(out=out, in_=d1)
        return eng.add_instruction(mybir.InstTensorScalarPtr(
            name=nc.get_next_instruction_name(),
            is_tensor_tensor_scan=True,
            is_scalar_tensor_tensor=True,
            op0=MM, op1=AD,
            ins=[eng.lower_ap(d0), eng.lower_ap_or_imm(0.0), eng.lower_ap(d1)],
            outs=[eng.lower_ap(out)],
        ))

    def recip(out, in_):
        if os.environ.get('BASSK_SLOWRECIP'):
            return nc.vector.reciprocal(out=out, in_=in_)
        return nc.vector.reciprocal_approx_fast(out=out, in_=in_)

    def view(ap, dims, off=0):
        return bass.AP(tensor=ap.tensor, offset=ap.offset + off, ap=dims)

    with tile.TileContext(nc) as tc:
        with tc.tile_pool(name="big", bufs=1) as big, \
             tc.tile_pool(name="wres", bufs=1) as wres, \
             tc.tile_pool(name="scr", bufs=4) as scr, \
             tc.tile_pool(name="sml", bufs=3) as sml, \
             tc.tile_pool(name="y2b", bufs=9) as y2b, \
             tc.tile_pool(name="yupp", bufs=2) as yupp, \
             tc.tile_pool(name="wstr", bufs=2) as wstr, \
             tc.tile_pool(name="dsc", bufs=2, space="DRAM") as dsc, \
             tc.tile_pool(name="psmm", bufs=3, space="PSUM") as psmm, \
             tc.tile_pool(name="psst", bufs=1, space="PSUM") as psst, \
             tc.tile_pool(name="psbc", bufs=2, space="PSUM") as psbc:

            # ---- resident constants
            lt = wres.tile([128, 2, 64], F32, name="lt")
            nc.sync.dma_start(out=lt, in_=d_lam[:, :].rearrange("(t p) j -> p t j", p=128))
            cv = wres.tile([128, ncol], F32, name="cv")
            nc.sync.dma_start(out=cv, in_=d_cvec[:, :])
            red = wres.tile([128, 2, 2], F32R, name="red")
            nc.sync.dma_start(out=red, in_=d_red[:, :, :].bitcast(F32R))
            redb = wres.tile([128, 2, 2], BF16, name="redb")
            nc.sync.dma_start(out=redb, in_=d_redb[:, :, :])
            bc1 = wres.tile([2, 2, 128], F32R, name="bc1")
            nc.sync.dma_start(out=bc1, in_=d_bc1[:, :, :].bitcast(F32R))
            epsc = wres.tile([128, 1], F32, name="epsc")
            nc.sync.dma_start(out=epsc, in_=d_eps[:, :])
            c2wt = wres.tile([128, 4, 1024], BF16, name="c2wt")
            nc.sync.dma_start(out=c2wt, in_=d_c2[:, :].rearrange("(kt p) m -> p kt m", p=128))
            c3wt = wres.tile([128, 8, 256], BF16, name="c3wt")
            nc.sync.dma_start(out=c3wt, in_=d_c3[:, :].rearrange("(kt p) m -> p kt m", p=128))

            def col(name, i=0):
                return cv[:, colidx[name] + i:colidx[name] + i + 1]

            # ============ LayerNorm over channels (2 ctiles) ============
            def ln256(Xr, out_wr, name, wb=None, bf16_in=False):
                redm = redb if bf16_in else red
                dstat = dsc.tile([2, S], F32, name=f"dstat_{name}", tag="dstat")
                for ch in range(NCH):
                    sl = slice(ch * CH, (ch + 1) * CH)
                    ps = psst.tile([2, CH], F32, name=f"lnps_{name}", tag="st")
                    for ct in range(2):
                        nc.tensor.matmul(out=ps, lhsT=redm[:, 0, :], rhs=Xr(ct)[:, sl],
                                         start=(ct == 0), stop=False)
                    for ct in range(2):
                        sq = sml.tile([128, CH], BF16 if bf16_in else F32R,
                                      name=f"sq_{name}", tag="sqc", bufs=3)
                        nc.scalar.activation(
                            out=sq,
                            in_=Xr(ct)[:, sl] if bf16_in else Xr(ct)[:, sl].bitcast(F32),
                            func=Act.Square)
                        nc.tensor.matmul(out=ps, lhsT=redm[:, 1, :], rhs=sq,
                                         start=False, stop=(ct == 1))
                    stc = sml.tile([2, CH], F32, name=f"stc_{name}", tag="stc", bufs=3)
                    nc.scalar.copy(out=stc, in_=ps)
                    nc.sync.dma_start(out=dstat[:, sl], in_=stc)
                # small stat math in [128, 2, 32] layout: element (p,q,j) = stat[q, j*128+p]
                sm = sml.tile([128, 2, 32], F32, name=f"sm_{name}", tag="sm", bufs=2)
                nc.sync.dma_start(out=sm, in_=view(dstat[:, :], [[1, 128], [S, 2], [128, 32]]))
                nc.vector.tensor_scalar_mul(out=sm, in0=sm, scalar1=1.0 / C)
                t2 = sml.tile([128, 32], F32, name=f"t2_{name}", tag="t2", bufs=2)
                nc.vector.tensor_tensor(out=t2, in0=sm[:, 0, :], in1=sm[:, 0, :], op=MM)
                nc.vector.tensor_tensor(out=t2, in0=sm[:, 1, :], in1=t2, op=SU)
                nc.scalar.activation(out=t2, in_=t2, func=Act.Sqrt, bias=epsc)
                nc.vector.reciprocal(out=t2, in_=t2)                      # rstd
                nc.vector.tensor_tensor(out=sm[:, 0, :], in0=sm[:, 0, :], in1=t2, op=MM)
                drow = dsc.tile([2, S], F32, name=f"drow_{name}", tag="dstat")
                nc.sync.dma_start(out=view(drow[:, :], [[1, 128], [128, 32]]), in_=t2)
                nc.sync.dma_start(out=view(drow[:, :], [[1, 128], [128, 32]], off=S),
                                  in_=sm[:, 0, :])
                bcr = big.tile([2, S], F32R, name=f"bcr_{name}", tag="rowsbig")
                nc.sync.dma_start(out=bcr, in_=drow[:, :].bitcast(F32R))
                for ch in range(NCH):
                    sl = slice(ch * CH, (ch + 1) * CH)
                    pr = psbc.tile([128, CH], F32, name=f"pr_{name}", tag="pr")
                    pm = psbc.tile([128, CH], F32, name=f"pm_{name}", tag="pm")
                    nc.tensor.matmul(out=pr, lhsT=bc1[:, 0, :], rhs=bcr[:, sl])
                    nc.tensor.matmul(out=pm, lhsT=bc1[:, 1, :], rhs=bcr[:, sl])
                    for ct in range(2):
                        out_wr(ct, sl, pr, pm)
                if wb is not None:
                    wn, bn_, apfn = wb
                    for ct in range(2):
                        ap = apfn(ct)
                        nc.vector.tensor_scalar(
                            out=ap, in0=ap, scalar1=col(wn, ct), scalar2=col(bn_, ct),
                            op0=MM, op1=AD)

            # ============ S0/S1: load x, LN1 -> xn (bf16) ============
            x0 = big.tile([128, S], F32R, name="x0", tag="A")
            x1 = big.tile([128, S], F32R, name="x1", tag="B")
            nc.sync.dma_start(out=x0, in_=xin[0:128, :].bitcast(F32R))
            nc.sync.dma_start(out=x1, in_=xin[128:256, :].bitcast(F32R))
            mf = big.tile([128, S], F32, name="mf", tag="D")
            nc.sync.dma_start(out=mf, in_=view(mrow[:, :], [[0, 128], [1, S]]))

            xn = big.tile([128, 2, S], BF16, name="xn", tag="Cxn")
            xt = [x0, x1]

            def ln1_wr(ct, sl, pr, pm):
                nc.vector.tensor_tensor(out=xn[:, ct, sl], in0=xt[ct][:, sl].bitcast(F32),
                                        in1=pr, op=MM)
                nc.vector.tensor_tensor(out=xn[:, ct, sl], in0=xn[:, ct, sl],
                                        in1=pm, op=SU)

            ln256(lambda ct: xt[ct][:, :], ln1_wr, "ln1",
                  wb=None if ln1_triv else ("ln1w", "ln1b", lambda ct: xn[:, ct, :]))
            if probe:
                pxn = mkprobe("p_xn", [C, S])
                for ct in range(2):
                    nc.gpsimd.dma_start(out=pxn[128 * ct:128 * (ct + 1), :],
                                        in_=xn[:, ct, :])

            # ============ S2: q_shift diff * mask -> md (bf16) ============
            xn4 = xn.rearrange("p t (h w) -> p t h w", h=H)
            md = big.tile([128, 2, H, W], BF16, name="md", tag="Emd")
            nc.vector.tensor_tensor(out=md[0:64, 0, :, 1:], in0=xn4[0:64, 0, :, 0:63],
                                    in1=xn4[0:64, 0, :, 1:], op=SU)
            nc.vector.tensor_scalar_mul(out=md[0:64, 0, :, 0:1],
                                        in0=xn4[0:64, 0, :, 0:1], scalar1=-1.0)
            nc.vector.tensor_tensor(out=md[64:128, 0, :, 0:63], in0=xn4[64:128, 0, :, 1:],
                                    in1=xn4[64:128, 0, :, 0:63], op=SU)
            nc.vector.tensor_scalar_mul(out=md[64:128, 0, :, 63:64],
                                        in0=xn4[64:128, 0, :, 63:64], scalar1=-1.0)
            nc.vector.tensor_tensor(out=md[0:64, 1, 1:, :], in0=xn4[0:64, 1, 0:63, :],
                                    in1=xn4[0:64, 1, 1:, :], op=SU)
            nc.vector.tensor_scalar_mul(out=md[0:64, 1, 0:1, :],
                                        in0=xn4[0:64, 1, 0:1, :], scalar1=-1.0)
            nc.vector.tensor_tensor(out=md[64:128, 1, 0:63, :], in0=xn4[64:128, 1, 1:, :],
                                    in1=xn4[64:128, 1, 0:63, :], op=SU)
            nc.vector.tensor_scalar_mul(out=md[64:128, 1, 63:64, :],
                                        in0=xn4[64:128, 1, 63:64, :], scalar1=-1.0)
            mdf = md.rearrange("p t h w -> p t (h w)")
            for ct in range(2):
                nc.vector.tensor_tensor(out=mdf[:, ct, :], in0=mdf[:, ct, :],
                                        in1=mf, op=MM)

            # ============ S3: k/v/r matmuls -> E, V, SR; scans ============
            ev = big.tile([128, 2, S], BF16, name="ev", tag="B")
            et = big.tile([128, 2, S], BF16, name="et", tag="A")
            vv = big.tile([128, 2, S], BF16, name="vv", tag="D")
            sr = big.tile([128, 2, S], BF16, name="sr", tag="Fsr")

            def kvloop(dws, evac):
                wxt = wstr.tile([128, 2, 256], BF16, name="wxt", tag="wst", bufs=2)
                wdt = wstr.tile([128, 2, 256], BF16, name="wdt", tag="wst", bufs=2)
                nc.sync.dma_start(out=wxt, in_=dws[0][:, :].rearrange("(kt p) m -> p kt m", p=128))
                nc.sync.dma_start(out=wdt, in_=dws[1][:, :].rearrange("(kt p) m -> p kt m", p=128))
                for mt in range(2):
                    for ch in range(NCH):
                        sl = slice(ch * CH, (ch + 1) * CH)
                        ps = psmm.tile([128, CH], F32, name="kv_ps", tag="mm")
                        for kt in range(2):
                            nc.tensor.matmul(out=ps, lhsT=wxt[:, kt, 128 * mt:128 * (mt + 1)],
                                             rhs=xn[:, kt, sl], start=(kt == 0), stop=False)
                        for kt in range(2):
                            nc.tensor.matmul(out=ps, lhsT=wdt[:, kt, 128 * mt:128 * (mt + 1)],
                                             rhs=mdf[:, kt, sl], start=False, stop=(kt == 1))
                        evac(mt, sl, ps)

            kvloop(d_wk, lambda mt, sl, ps: nc.scalar.activation(
                out=et[:, mt, sl], in_=ps, func=Act.Exp))
            kvloop(d_wv, lambda mt, sl, ps: nc.scalar.copy(out=vv[:, mt, sl], in_=ps))
            kvloop(d_wr, lambda mt, sl, ps: nc.scalar.activation(
                out=sr[:, mt, sl], in_=ps, func=Act.Sigmoid))

            nc.vector.tensor_tensor(out=ev, in0=et, in1=vv, op=MM)

            ev4 = ev.rearrange("p t (h w) -> p t h w", h=H)
            et4 = et.rearrange("p t (h w) -> p t h w", h=H)
            outv = big.tile([128, 2, W, H], BF16, name="outv", tag="D")
            lt_ap = lt[:, :, :]

            def lamview(ct, nseq):
                return view(lt_ap, [lt_ap.ap[0], [0, nseq], [1, 64]], off=ct * 64)

            # vertical orientation first (reads pristine ev/et via transposed APs)
            for half in range(2):
                wr_ = slice(half * 32, (half + 1) * 32)
                av = scr.tile([128, 2, 32, 64], BF16, name="av", tag="scrt")
                bv = scr.tile([128, 2, 32, 64], BF16, name="bv", tag="scrt")
                for ct in range(2):
                    dv_ev = view(ev[:, :, :], [ev.ap[0], [1, 32], [64, 64]],
                                 off=ct * S + half * 32)
                    dv_et = view(et[:, :, :], [et.ap[0], [1, 32], [64, 64]],
                                 off=ct * S + half * 32)
                    scan_raw(av[:, ct], lamview(ct, 32), dv_ev)
                    scan_raw(bv[:, ct], lamview(ct, 32), dv_et)
                for ct in range(2):
                    base = ct * S + half * 32
                    den = scr.tile([128, 32, 64], F32, name="den", tag="scrt")
                    nc.vector.scalar_tensor_tensor(
                        out=den[:, :, 1:],
                        in0=view(et[:, :, :], [et.ap[0], [1, 32], [64, 63]], off=base + 64),
                        scalar=col('eu', ct), in1=bv[:, ct, :, 0:63], op0=MM, op1=AD)
                    nc.vector.tensor_scalar_mul(
                        out=den[:, :, 0:1],
                        in0=view(et[:, :, :], [et.ap[0], [1, 32], [64, 1]], off=base),
                        scalar1=col('eu', ct))
                    recip(out=den, in_=den)
                    nc.vector.scalar_tensor_tensor(
                        out=outv[:, ct, wr_, 1:],
                        in0=view(ev[:, :, :], [ev.ap[0], [1, 32], [64, 63]], off=base + 64),
                        scalar=col('eu', ct), in1=av[:, ct, :, 0:63], op0=MM, op1=AD)
                    nc.vector.tensor_scalar_mul(
                        out=outv[:, ct, wr_, 0:1],
                        in0=view(ev[:, :, :], [ev.ap[0], [1, 32], [64, 1]], off=base),
                        scalar1=col('eu', ct))
                    nc.vector.tensor_tensor(out=outv[:, ct, wr_, :], in0=outv[:, ct, wr_, :],
                                            in1=den, op=MM)

            # horizontal orientation; num/out in place on ev
            for half in range(2):
                hr = slice(half * 32, (half + 1) * 32)
                ah = scr.tile([128, 2, 32, 64], BF16, name="ah", tag="scrt")
                bh = scr.tile([128, 2, 32, 64], BF16, name="bh", tag="scrt")
                for ct in range(2):
                    scan_raw(ah[:, ct], lamview(ct, 32), ev4[:, ct, hr, :])
                    scan_raw(bh[:, ct], lamview(ct, 32), et4[:, ct, hr, :])
                for ct in range(2):
                    den = scr.tile([128, 32, 64], F32, name="den2", tag="scrt")
                    nc.vector.scalar_tensor_tensor(
                        out=den[:, :, 1:], in0=et4[:, ct, hr, 1:],
                        scalar=col('eu', ct), in1=bh[:, ct, :, 0:63], op0=MM, op1=AD)
                    nc.vector.tensor_scalar_mul(
                        out=den[:, :, 0:1], in0=et4[:, ct, hr, 0:1], scalar1=col('eu', ct))
                    recip(out=den, in_=den)
                    nc.vector.scalar_tensor_tensor(
                        out=ev4[:, ct, hr, 1:], in0=ev4[:, ct, hr, 1:],
                        scalar=col('eu', ct), in1=ah[:, ct, :, 0:63], op0=MM, op1=AD)
                    nc.vector.tensor_scalar_mul(
                        out=ev4[:, ct, hr, 0:1], in0=ev4[:, ct, hr, 0:1],
                        scalar1=col('eu', ct))
                    nc.vector.tensor_tensor(out=ev4[:, ct, hr, :], in0=ev4[:, ct, hr, :],
                                            in1=den, op=MM)

            # wkv = out_h + out_v^T (0.5 factor dropped: LN-invariant)
            for ct in range(2):
                ovT = view(outv[:, :, :, :], [outv.ap[0], [1, 64], [64, 64]], off=ct * S)
                nc.vector.tensor_tensor(out=ev4[:, ct, :, :], in0=ev4[:, ct, :, :],
                                        in1=ovT, op=AD)
            if probe:
                pwkv = mkprobe("p_wkv", [C, S])
                for ct in range(2):
                    nc.gpsimd.dma_start(out=pwkv[128 * ct:128 * (ct + 1), :],
                                        in_=ev[:, ct, :])

            # ============ S4: key-LN, srw, Wo+residual, skip feat ============
            def kn_wr(ct, sl, pr, pm):
                nc.vector.tensor_tensor(out=ev[:, ct, sl], in0=ev[:, ct, sl], in1=pr, op=MM)
                nc.vector.tensor_tensor(out=ev[:, ct, sl], in0=ev[:, ct, sl], in1=pm, op=SU)

            ln256(lambda ct: ev[:, ct, :], kn_wr, "kn", bf16_in=True,
                  wb=None if kn_triv else ("knw", "knb", lambda ct: ev[:, ct, :]))

            nc.vector.tensor_tensor(out=sr, in0=sr, in1=ev, op=MM)   # srw

            xcp = [scr.tile([128, 66, 66], BF16, name=f"xcp{i}", tag="scrt")
                   for i in range(4)]
            for t in xcp:
                nc.vector.memset(t[:, 0:1, :], 0.0)
                nc.vector.memset(t[:, 65:66, :], 0.0)
                nc.vector.memset(t[:, 1:65, 0:1], 0.0)
                nc.vector.memset(t[:, 1:65, 65:66], 0.0)

            wot = wstr.tile([128, 2, 256], BF16, name="wot", tag="wst", bufs=2)
            nc.sync.dma_start(out=wot, in_=d_wo[:, :].rearrange("(kt p) m -> p kt m", p=128))
            for mt in range(2):
                for ch in range(NCH):
                    sl = slice(ch * CH, (ch + 1) * CH)
                    h0 = ch * 8
                    ps = psmm.tile([128, CH], F32, name="wo_ps", tag="mm")
                    for kt in range(2):
                        nc.tensor.matmul(out=ps, lhsT=wot[:, kt, 128 * mt:128 * (mt + 1)],
                                         rhs=sr[:, kt, sl], start=(kt == 0), stop=(kt == 1))
                    nc.vector.tensor_tensor(
                        out=xcp[mt][:, 1 + h0:9 + h0, 1:65],
                        in0=xn4[:, mt, h0:h0 + 8, :],
                        in1=ps.rearrange("p (a b) -> p a b", a=8), op=AD)

            spt = wstr.tile([128, 4, 256], F32R, name="spt", tag="wst4", bufs=1)
            nc.sync.dma_start(out=spt,
                              in_=d_sp[:, :].rearrange("(kt p) m -> p kt m", p=128).bitcast(F32R))
            for ch in range(NCH):
                sl = slice(ch * CH, (ch + 1) * CH)
                h0 = ch * 8
                skc = big.tile([128, 4, CH], F32R, name="skc",
                               tag="A" if ch % 2 == 0 else "B")
                nc.sync.dma_start(
                    out=skc,
                    in_=skin[:, sl].rearrange("(kt p) n -> p kt n", p=128).bitcast(F32R))
                for mt in range(2):
                    ps = psmm.tile([128, CH], F32, name="sp_ps", tag="mm")
                    for kt in range(4):
                        nc.tensor.matmul(out=ps, lhsT=spt[:, kt, 128 * mt:128 * (mt + 1)],
                                         rhs=skc[:, kt, :], start=(kt == 0), stop=(kt == 3))
                    nc.scalar.activation(
                        out=xcp[2 + mt][:, 1 + h0:9 + h0, 1:65],
                        in_=ps.rearrange("p (a b) -> p a b", a=8),
                        func=Act.Identity, bias=col('spb', mt))

            if probe:
                pxc = mkprobe("p_xcat", [CS, S])
                for i in range(4):
                    nc.gpsimd.dma_start(
                        out=pxc[128 * i:128 * (i + 1), :].rearrange("p (a b) -> p a b", a=64),
                        in_=xcp[i][:, 1:65, 1:65])

            # ============ S5: grouped 3x3 conv -> gelu -> y1 (bf16) ============
            y1a = big.tile([128, 2, S], BF16, name="y1a", tag="A")
            y1b = big.tile([128, 2, S], BF16, name="y1b", tag="Cxn")
            y1t = [y1a, y1b]
            # prime the wst9 slots so the c1 weight DMAs land after the
            # scan/Wo stages (works around early-SBUF corruption of the
            # first-loaded tiles)
            if not os.environ.get('BASSK_NOPRIME'):
                for i in range(2):
                    pr_ = wstr.tile([128, 1], BF16, name=f"prime{i}", tag="wst9")
                    nc.vector.tensor_copy(out=pr_, in_=xcp[i][:, 0, 0:1])
            if probe and os.environ.get('BASSK_CANARY'):
                cnry = wstr.tile([128, 9, 2, 128], BF16, name="cnry", tag="wst9")
                nc.sync.dma_start(out=cnry, in_=d_c1[1, 0, :, :, :, :])
                marks = [("m0", cnry[:, 0, 0, 0:64]),
                         ("m1", xn[:, 0, 0:64]),
                         ("m2", ev[:, 0, 0:64]),
                         ("m3", sr[:, 0, 0:64])]
                for mi, (mn, mark) in enumerate(marks):
                    stg_c = sml.tile([128, 64], BF16, name=f"cst{mi}",
                                     tag="cst", bufs=4)
                    nc.vector.tensor_tensor(
                        out=stg_c, in0=cnry[:, 0, 0, 0:64],
                        in1=mark, op=Alu.bypass)
                    pc = mkprobe(f"p_cn{mi}", [128, 64])
                    nc.gpsimd.dma_start(out=pc[:, :], in_=stg_c)
            for g in (1, 0):
                for mt in range(2):
                    c1gm = wstr.tile([128, 9, 2, 128], BF16, name="c1gm", tag="wst9")
                    nc.sync.dma_start(out=c1gm, in_=d_c1[g, mt, :, :, :, :])
                    if probe and mt == 0:
                        pw = mkprobe(f"p_c1w_{g}", [128, 9 * 2 * 128])
                        nc.gpsimd.dma_start(out=pw[:, :],
                                            in_=c1gm.rearrange("p a b c -> p (a b c)"))
                    for ch in range(NCH):
                        h0 = ch * 8
                        ps = psmm.tile([128, CH], F32, name="c1_ps", tag="mm")
                        i = 0
                        for ti in range(9):
                            dy, dx = ti // 3 - 1, ti % 3 - 1
                            for kt in range(2):
                                nc.tensor.matmul(
                                    out=ps.rearrange("p (a b) -> p a b", a=8),
                                    lhsT=c1gm[:, ti, kt, :],
                                    rhs=xcp[2 * g + kt][:, 1 + h0 + dy:9 + h0 + dy,
                                                        1 + dx:65 + dx],
                                    start=(i == 0), stop=(i == 17))
                                i += 1
                        if probe and mt == 0 and ch == 0:
                            pps = mkprobe(f"p_c1ps_{g}", [128, CH])
                            stg = sml.tile([128, CH], F32, name="stg", tag="sqc")
                            nc.scalar.copy(out=stg, in_=ps)
                            nc.gpsimd.dma_start(out=pps[:, :], in_=stg)
                        nc.scalar.activation(
                            out=y1t[g][:, mt, ch * CH:(ch + 1) * CH], in_=ps,
                            func=Act.Gelu, bias=col('c1b', 2 * g + mt))

            if probe:
                py1 = mkprobe("p_y1", [CS, S])
                for i in range(4):
                    nc.gpsimd.dma_start(out=py1[128 * i:128 * (i + 1), :],
                                        in_=y1t[i // 2][:, i % 2, :])

            # ============ S6: c2 -> gelu -> c3 -> gelu(+bn3) -> y3 ============
            y3 = [big.tile([128, S], F32R, name="y3_0", tag="Emd"),
                  big.tile([128, S], F32R, name="y3_1", tag="D")]
            for ch in range(NCH):
                sl = slice(ch * CH, (ch + 1) * CH)
                ytiles = []
                for mt in range(8):
                    ps = psmm.tile([128, CH], F32, name="c2_ps", tag="mm")
                    for kt in range(4):
                        nc.tensor.matmul(out=ps, lhsT=c2wt[:, kt, 128 * mt:128 * (mt + 1)],
                                         rhs=y1t[kt // 2][:, kt % 2, sl],
                                         start=(kt == 0), stop=(kt == 3))
                    yt = y2b.tile([128, CH], BF16, name="y2t", tag="y2t")
                    nc.scalar.activation(out=yt, in_=ps, func=Act.Gelu, bias=col('c2b', mt))
                    ytiles.append(yt)
                for mt in range(2):
                    ps = psmm.tile([128, CH], F32, name="c3_ps", tag="mm")
                    for kt in range(8):
                        nc.tensor.matmul(out=ps, lhsT=c3wt[:, kt, 128 * mt:128 * (mt + 1)],
                                         rhs=ytiles[kt], start=(kt == 0), stop=(kt == 7))
                    nc.scalar.activation(out=y3[mt][:, sl], in_=ps, func=Act.Gelu,
                                         bias=col('c3b', mt))
                    if not bn3_triv:
                        nc.vector.tensor_scalar(out=y3[mt][:, sl],
                                                in0=y3[mt][:, sl].bitcast(F32),
                                                scalar1=col('g3p', mt),
                                                scalar2=col('b3p', mt), op0=MM, op1=AD)

            if probe:
                py3 = mkprobe("p_y3", [C, S])
                for i in range(2):
                    nc.gpsimd.dma_start(out=py3[128 * i:128 * (i + 1), :],
                                        in_=y3[i][:, :].bitcast(F32))

            # ============ S7: LN2, up-proj, pixel-shuffle DMA out ============
            def ln2_wr(ct, sl, pr, pm):
                nc.vector.tensor_tensor(out=y3[ct][:, sl], in0=y3[ct][:, sl].bitcast(F32),
                                        in1=pr, op=MM)
                nc.vector.tensor_tensor(out=y3[ct][:, sl], in0=y3[ct][:, sl].bitcast(F32),
                                        in1=pm, op=SU)

            ln256(lambda ct: y3[ct][:, :], ln2_wr, "ln2",
                  wb=None if ln2_triv else ("ln2w", "ln2b", lambda ct: y3[ct][:, :]))

            upt = wstr.tile([128, 2, 512], F32R, name="upt", tag="wst4", bufs=1)
            nc.sync.dma_start(out=upt,
                              in_=d_up[:, :].rearrange("(kt p) m -> p kt m", p=128).bitcast(F32R))
            for r in range(2):
                for ch in range(NCH):
                    sl = slice(ch * CH, (ch + 1) * CH)
                    h0 = ch * 8
                    ub = yupp.tile([128, 8, 64, 2], F32, name="ub", tag="ub")
                    for q in range(2):
                        rq = 2 * r + q
                        ps = psmm.tile([128, CH], F32, name="up_ps", tag="mm")
                        for kt in range(2):
                            nc.tensor.matmul(out=ps,
                                             lhsT=upt[:, kt, 128 * rq:128 * (rq + 1)],
                                             rhs=y3[kt][:, sl],
                                             start=(kt == 0), stop=(kt == 1))
                        nc.scalar.activation(out=ub[:, :, :, q],
                                             in_=ps.rearrange("p (a b) -> p a b", a=8),
                                             func=Act.Identity, bias=col('upb', rq))
                    dst = view(yout[:, :, :], [[128 * 128, 128], [256, 8], [1, 128]],
                               off=(2 * h0 + r) * 128)
                    nc.sync.dma_start(out=dst, in_=ub.rearrange("p a b q -> p a (b q)"))

    nc.compile()
    return nc, const_inputs


def _get_nc(weights, probe=False):
    import hashlib
    hsh = hashlib.sha1()
    for k in sorted(weights):
        hsh.update(k.encode())
        hsh.update(np.ascontiguousarray(weights[k]).tobytes())
    key = (hsh.hexdigest(), probe)
    if key not in _CACHE:
        _CACHE[key] = _build(weights, probe=probe)
    return _CACHE[key]


def kernel(**inputs):
    from concourse.bass_utils import run_bass_kernel_spmd

    x = np.asarray(inputs['x'], np.float32)
    skip = np.asarray(inputs['skip'], np.float32)
    mask = np.asarray(inputs['saliency_mask'], np.float32)
    weights = {k: np.asarray(v, np.float32) for k, v in inputs.items()
               if k not in ('x', 'skip', 'saliency_mask')}

    probe = bool(os.environ.get('BASSK_PROBE'))
    nc, const_inputs = _get_nc(weights, probe=probe)

    in_maps = []
    for b in range(B):
        m = dict(
            xin=np.ascontiguousarray(x[b].reshape(C, S)),
            skin=np.ascontiguousarray(skip[b].reshape(CS, S)),
            mrow=np.ascontiguousarray(mask[b].reshape(1, S)),
        )
        m.update(const_inputs)
        in_maps.append(m)
    res = run_bass_kernel_spmd(nc, in_maps, core_ids=list(range(B)),
                               trace=bool(os.environ.get('BASSK_TRACE')))
    kernel.last_results = res
    out = np.stack([res.results[b]['yout'] for b in range(B)], axis=0)
    return out



# revision 11
# speedup vs baseline: 1.1802x; 1.1802x over previous
"""Trainium2 Bass kernel for nn_DecoderBlock (shape-guided RWKV decoder block).

Data-parallel over batch: B=8 samples -> 8 NeuronCores, one NEFF.

v2: restructured for PE occupancy (HAM clock-gate awareness) and engine
balance:
- all matmuls bf16 (FWL stays enabled, no fp32-HIGH power throttle);
- skip-conv + the skip-group half of the 3x3 conv run on PE during the
  DVE-only WKV scan phase (program-order interleaving);
- LayerNorms pipelined per-chunk (stat DRAM bounce in bf16, broadcast
  via K=2 matmul, applies read PSUM directly);
- scan den/num tiles zero-padded (no edge fixup ops); vertical outputs
  written h-major so the final combine is contiguous;
- Pool engine (gpsimd) takes dtype converts and the scan-output
  multiplies; ACT takes all PSUM evacuations fused with bias+act.
"""
import sys
import os

for _p in ('/opt/trn_rl_repo', '/root/.axon_site/_ro/trn_rl_repo'):
    if _p not in sys.path and os.path.isdir(_p):
        sys.path.append(_p)

import numpy as np

B, C, CS, COUT, H, W = 8, 256, 512, 128, 64, 64
S = H * W          # 4096
NCH = 8            # spatial chunks
CH = S // NCH      # 512
EPS = 1e-5

_CACHE = {}


def _build(weights, probe=False):
    const_inputs = {}
    import concourse.bass as bass
    from concourse import bacc
    import concourse.tile as tile
    import concourse.mybir as mybir
    import ml_dtypes

    F32 = mybir.dt.float32
    BF16 = mybir.dt.bfloat16
    Alu = mybir.AluOpType
    Act = mybir.ActivationFunctionType
    MM, AD, SU = Alu.mult, Alu.add, Alu.subtract

    w = weights
    f64 = lambda x: np.asarray(x, np.float64)
    bf = lambda a: np.asarray(a, dtype=ml_dtypes.bfloat16)

    # ---------------- host-side folding
    bnscale = 1.0 / np.sqrt(1.0 + EPS)
    g1p = f64(w['bn1_g']) * bnscale
    b1p = f64(w['bn1_b'])
    g2p = f64(w['bn2_g']) * bnscale
    b2p = f64(w['bn2_b'])
    g3p = (f64(w['bn3_g']) * bnscale).astype(np.float32)
    b3p = f64(w['bn3_b']).astype(np.float32)

    c2_eff = f64(w['c2_w']) * g1p[None, :]
    c2b_eff = (f64(w['c2_b']) + f64(w['c2_w']) @ b1p).astype(np.float32)
    c3_eff = f64(w['c3_w']) * g2p[None, :]
    c3b_eff = (f64(w['c3_b']) + f64(w['c3_w']) @ b2p).astype(np.float32)

    wk_x = f64(w['Wk']).T
    wk_d = (f64(w['Wk']) * (1.0 - f64(w['mix_k']))[None, :]).T
    wv_x = f64(w['Wv']).T
    wv_d = (f64(w['Wv']) * (1.0 - f64(w['mix_v']))[None, :]).T
    wr_x = f64(w['Wr']).T
    wr_d = (f64(w['Wr']) * (1.0 - f64(w['mix_r']))[None, :]).T
    wo_t = f64(w['Wo']).T
    sp_t = f64(w['sp_w']).T

    lam = np.exp(-np.exp(f64(w['decay']))).astype(np.float32)
    lam64 = np.tile(lam[:, None], (1, 64))
    lam64[:, 0] = 0.0
    lam64 = lam64.astype(np.float32)
    eu = np.exp(f64(w['first'])).astype(np.float32)

    pidx = np.arange(512)
    old = (pidx % 128) * 4 + (pidx // 128)
    up_t = f64(w['up_w'])[old].T                                # [256, 512]
    upb_p = f64(w['up_b'])[old].astype(np.float32)

    c1w = f64(w['c1_w'])
    c1_l = np.zeros((9, 2, 256, 256), np.float32)
    for ti in range(9):
        dy, dx = ti // 3, ti % 3
        for g in range(2):
            c1_l[ti, g] = c1w[g * 256:(g + 1) * 256, :, dy, dx].T

    # per-channel vectors as columns of one [128, ncol] const
    cols, order = {}, []

    def addcol(name, vec):
        v = np.asarray(vec, np.float32).reshape(-1, 128)
        cols[name] = v
        order.append(name)

    addcol('eu', eu)
    addcol('ln1w', w['ln1_w'])
    addcol('ln1b', w['ln1_b'])
    addcol('knw', w['kn_w'])
    addcol('knb', w['kn_b'])
    addcol('ln2w', w['ln2_w'])
    addcol('ln2b', w['ln2_b'])
    addcol('g3p', g3p)
    addcol('b3p', b3p)
    addcol('spb', w['sp_b'])
    addcol('c3b', c3b_eff)
    addcol('c1b', w['c1_b'])
    addcol('c2b', c2b_eff)
    addcol('upb', upb_p)
    colidx, ncol = {}, 0
    for n in order:
        colidx[n] = ncol
        ncol += cols[n].shape[0]
    cvec_np = np.zeros((128, ncol), np.float32)
    for n in order:
        for i in range(cols[n].shape[0]):
            cvec_np[:, colidx[n] + i] = cols[n][i]

    ln1_triv = np.all(w['ln1_w'] == 1.0) and np.all(w['ln1_b'] == 0.0)
    kn_triv = np.all(w['kn_w'] == 1.0) and np.all(w['kn_b'] == 0.0)
    ln2_triv = np.all(w['ln2_w'] == 1.0) and np.all(w['ln2_b'] == 0.0)
    bn3_triv = np.all(g3p == g3p[0]) and np.all(b3p == 0.0)
    # uniform bn3 scale commutes with LN2 -> drop it entirely when trivial

    # ---------------- bass module
    nc = bacc.Bacc("TRN2", target_bir_lowering=False, debug=False, name="decblk")

    xin = nc.dram_tensor("xin", [C, S], F32, kind="ExternalInput")
    skin = nc.dram_tensor("skin", [CS, S], F32, kind="ExternalInput")
    mrow = nc.dram_tensor("mrow", [1, S], F32, kind="ExternalInput")
    yout = nc.dram_tensor("yout", [COUT, 2 * H, 2 * W], F32, kind="ExternalOutput")
    probes = {}

    def mkprobe(name, shape):
        if probe:
            probes[name] = nc.dram_tensor(name, shape, F32, kind="ExternalOutput")
        return probes.get(name)

    def it(arr, name):
        arr = np.ascontiguousarray(arr)
        import ml_dtypes as _md
        dt_ = {np.dtype(np.float32): F32, np.dtype(_md.bfloat16): BF16}[arr.dtype]
        const_inputs[name] = arr
        return nc.dram_tensor(name, list(arr.shape), dt_, kind="ExternalInput")

    d_lam = it(lam64, "lam64")
    d_cvec = it(cvec_np, "cvec")
    d_wk = [it(bf(wk_x), "wkx"), it(bf(wk_d), "wkd")]
    d_wv = [it(bf(wv_x), "wvx"), it(bf(wv_d), "wvd")]
    d_wr = [it(bf(wr_x), "wrx"), it(bf(wr_d), "wrd")]
    d_wo = it(bf(wo_t), "wo")
    d_sp = it(bf(sp_t), "sp")
    d_up = it(bf(up_t), "up")
    c1_r = c1_l.reshape(9, 2, 2, 128, 2, 128).transpose(1, 4, 3, 0, 2, 5)
    d_c1 = it(bf(c1_r), "c1")   # [g, mt, p, t, kt, m]
    d_c2 = it(bf(c2_eff.T), "c2")
    d_c3 = it(bf(c3_eff.T), "c3")
    red_np = np.zeros((128, 2, 2), np.float32)
    red_np[:, 0, 0] = 1.0
    red_np[:, 1, 1] = 1.0
    d_redb = it(bf(red_np), "redb")
    bc2_np = np.zeros((2, 2, 128), np.float32)
    bc2_np[0, 0, :] = 1.0
    bc2_np[1, 1, :] = 1.0
    d_bc1 = it(bf(bc2_np), "bc2")
    d_eps = it(np.full((128, 1), EPS, np.float32), "epsc")

    def scan_raw(out, d0, d1):
        eng = nc.vector
        return eng.add_instruction(mybir.InstTensorScalarPtr(
            name=nc.get_next_instruction_name(),
            is_tensor_tensor_scan=True,
            is_scalar_tensor_tensor=True,
            op0=MM, op1=AD,
            ins=[eng.lower_ap(d0), eng.lower_ap_or_imm(0.0), eng.lower_ap(d1)],
            outs=[eng.lower_ap(out)],
        ))

    def view(ap, dims, off=0):
        return bass.AP(tensor=ap.tensor, offset=ap.offset + off, ap=dims)

    with tile.TileContext(nc) as tc:
        with tc.tile_pool(name="big", bufs=1) as big, \
             tc.tile_pool(name="dnp", bufs=2) as dnp, \
             tc.tile_pool(name="scn", bufs=3) as scn, \
             tc.tile_pool(name="wres", bufs=1) as wres, \
             tc.tile_pool(name="sml", bufs=3) as sml, \
             tc.tile_pool(name="y2b", bufs=8) as y2b, \
             tc.tile_pool(name="wstr", bufs=2) as wstr, \
             tc.tile_pool(name="wsk", bufs=2) as wsk, \
             tc.tile_pool(name="xcpp", bufs=2) as xcpp, \
             tc.tile_pool(name="dsc", bufs=2, space="DRAM") as dsc, \
             tc.tile_pool(name="psmm", bufs=3, space="PSUM") as psmm, \
             tc.tile_pool(name="psst", bufs=1, space="PSUM") as psst, \
             tc.tile_pool(name="psbc", bufs=2, space="PSUM") as psbc:

            # ---- resident constants
            lt = wres.tile([128, 2, 64], F32, name="lt")
            nc.sync.dma_start(out=lt, in_=d_lam[:, :].rearrange("(t p) j -> p t j", p=128))
            cv = wres.tile([128, ncol], F32, name="cv")
            nc.sync.dma_start(out=cv, in_=d_cvec[:, :])
            redb = wres.tile([128, 2, 2], BF16, name="redb")
            nc.sync.dma_start(out=redb, in_=d_redb[:, :, :])
            bc1 = wres.tile([2, 2, 128], BF16, name="bc1")
            nc.sync.dma_start(out=bc1, in_=d_bc1[:, :, :])
            epsc = wres.tile([128, 1], F32, name="epsc")
            nc.sync.dma_start(out=epsc, in_=d_eps[:, :])
            wot = wres.tile([128, 2, 256], BF16, name="wot")
            nc.sync.dma_start(out=wot, in_=d_wo[:, :].rearrange("(kt p) m -> p kt m", p=128))
            kvrw = []
            for nm, dws in (("wk", d_wk), ("wv", d_wv), ("wr", d_wr)):
                wxt = wres.tile([128, 2, 256], BF16, name=f"{nm}x")
                wdt = wres.tile([128, 2, 256], BF16, name=f"{nm}d")
                nc.sync.dma_start(out=wxt, in_=dws[0][:, :].rearrange("(kt p) m -> p kt m", p=128))
                nc.sync.dma_start(out=wdt, in_=dws[1][:, :].rearrange("(kt p) m -> p kt m", p=128))
                kvrw.append((wxt, wdt))
            c2wt = wres.tile([128, 4, 1024], BF16, name="c2wt")
            nc.sync.dma_start(out=c2wt, in_=d_c2[:, :].rearrange("(kt p) m -> p kt m", p=128))

            def col(name, i=0):
                return cv[:, colidx[name] + i:colidx[name] + i + 1]

            # =========== chunked LayerNorm over channels ===========
            # stats per chunk -> DRAM bounce -> one global stat-math ->
            # per chunk bf16 bounce -> K=2 broadcast matmul -> applies.
            def ln_stats_chunk(Xr, ps, sl):
                # Xr(ct) -> [128, CH] bf16 slices; accumulate sum/sumsq rows
                nc.tensor.matmul(out=ps, lhsT=redb[:, 0, :], rhs=Xr(0)[:, sl],
                                 start=True, stop=False)
                nc.tensor.matmul(out=ps, lhsT=redb[:, 0, :], rhs=Xr(1)[:, sl],
                                 start=False, stop=False)
                for ct in range(2):
                    sq = sml.tile([128, CH], BF16, name="sq", tag="sqc", bufs=3)
                    nc.scalar.activation(out=sq, in_=Xr(ct)[:, sl], func=Act.Square)
                    nc.tensor.matmul(out=ps, lhsT=redb[:, 1, :], rhs=sq,
                                     start=False, stop=(ct == 1))

            def ln256(Xr, out_wr, name, wb=None):
                dstat = dsc.tile([2, S], F32, name=f"dstat_{name}", tag="dstat")
                drow = dsc.tile([2, S], BF16, name=f"drow_{name}", tag="drow")
                for ch in range(NCH):
                    sl = slice(ch * CH, (ch + 1) * CH)
                    ps = psst.tile([2, CH], F32, name=f"lnps_{name}", tag="st")
                    ln_stats_chunk(Xr, ps, sl)
                    stc = sml.tile([2, CH], F32, name=f"stc_{name}", tag="stc", bufs=2)
                    nc.scalar.copy(out=stc, in_=ps)
                    nc.sync.dma_start(out=dstat[:, sl], in_=stc)
                # global stat math in [128, 2, 32] layout: (p,q,j) = stat[q, j*128+p]
                sm = sml.tile([128, 2, 32], F32, name=f"sm_{name}", tag="sm", bufs=2)
                nc.sync.dma_start(out=sm, in_=view(dstat[:, :], [[1, 128], [S, 2], [128, 32]]))
                nc.vector.tensor_scalar_mul(out=sm, in0=sm, scalar1=1.0 / C)
                t2 = sml.tile([128, 32], F32, name=f"t2_{name}", tag="t2", bufs=2)
                nc.vector.tensor_tensor(out=t2, in0=sm[:, 0, :], in1=sm[:, 0, :], op=MM)
                nc.vector.tensor_tensor(out=t2, in0=sm[:, 1, :], in1=t2, op=SU)
                nc.scalar.activation(out=t2, in_=t2, func=Act.Sqrt, bias=epsc)
                nc.vector.reciprocal(out=t2, in_=t2)                      # rstd
                smb = sml.tile([128, 2, 32], BF16, name=f"smb_{name}", tag="smb", bufs=2)
                nc.vector.tensor_copy(out=smb[:, 0, :], in_=t2)
                nc.vector.tensor_tensor(out=smb[:, 1, :], in0=sm[:, 0, :], in1=t2, op=MM)
                nc.sync.dma_start(out=view(drow[:, :], [[1, 128], [S, 2], [128, 32]]),
                                  in_=smb)
                for ch in range(NCH):
                    sl = slice(ch * CH, (ch + 1) * CH)
                    bcrc = sml.tile([2, CH], BF16, name=f"bcr_{name}", tag="bcrc", bufs=3)
                    nc.sync.dma_start(out=bcrc, in_=drow[:, sl])
                    pr = psbc.tile([128, CH], F32, name=f"pr_{name}", tag="pr")
                    pm = psbc.tile([128, CH], F32, name=f"pm_{name}", tag="pm")
                    nc.tensor.matmul(out=pr, lhsT=bc1[:, 0, :], rhs=bcrc)
                    nc.tensor.matmul(out=pm, lhsT=bc1[:, 1, :], rhs=bcrc)
                    for ct in range(2):
                        out_wr(ct, sl, pr, pm)
                if wb is not None:
                    wn, bn_, apfn = wb
                    for ct in range(2):
                        ap = apfn(ct)
                        nc.vector.tensor_scalar(
                            out=ap, in0=ap, scalar1=col(wn, ct), scalar2=col(bn_, ct),
                            op0=MM, op1=AD)

            # ============ phase A: load x, convert, LN1; skip conv ============
            x0 = big.tile([128, S], F32, name="x0", tag="A")
            nc.sync.dma_start(out=x0, in_=xin[0:128, :])
            xb = big.tile([128, 2, S], BF16, name="xb", tag="Xb")
            nc.gpsimd.tensor_copy(out=xb[:, 0, :], in_=x0)
            x1 = big.tile([128, S], F32, name="x1", tag="B")
            nc.sync.dma_start(out=x1, in_=xin[128:256, :])
            nc.gpsimd.tensor_copy(out=xb[:, 1, :], in_=x1)

            # mask: fp32 row -> bf16 (in [32,128] layout) -> DRAM -> broadcast
            m1 = sml.tile([32, 128], F32, name="m1", tag="m1", bufs=1)
            nc.sync.dma_start(out=m1, in_=view(mrow[:, :], [[128, 32], [1, 128]]))
            m1b = sml.tile([32, 128], BF16, name="m1b", tag="m1b", bufs=1)
            nc.vector.tensor_copy(out=m1b, in_=m1)
            dmask = dsc.tile([32, 128], BF16, name="dmask", tag="dmask", bufs=1)
            nc.sync.dma_start(out=dmask, in_=m1b)
            mfb = big.tile([128, S], BF16, name="mfb", tag="Mf")
            nc.sync.dma_start(out=mfb, in_=view(dmask[:, :], [[0, 128], [1, S]]))

            # xcp tiles for skip group written early; borders zeroed
            xcp_g1 = []
            for i in range(2):
                t = xcpp.tile([128, 66, 66], BF16, name=f"xcp{2 + i}", tag="xcp")
                nc.vector.memset(t[:, 0:1, :], 0.0)
                nc.vector.memset(t[:, 65:66, :], 0.0)
                nc.vector.memset(t[:, 1:65, 0:1], 0.0)
                nc.vector.memset(t[:, 1:65, 65:66], 0.0)
                xcp_g1.append(t)

            sptb = wstr.tile([128, 4, 256], BF16, name="sptb", tag="wst9")
            nc.sync.dma_start(out=sptb, in_=d_sp[:, :].rearrange("(kt p) m -> p kt m", p=128))

            xn = big.tile([128, 2, S], BF16, name="xn", tag="Cxn")

            def ln1_wr(ct, sl, pr, pm):
                nc.vector.tensor_tensor(out=xn[:, ct, sl], in0=xb[:, ct, sl],
                                        in1=pr, op=MM)
                nc.vector.tensor_tensor(out=xn[:, ct, sl], in0=xn[:, ct, sl],
                                        in1=pm, op=SU)

            # LN1 stats per chunk with skip-conv chunks interleaved (PE filler)
            dstat1 = dsc.tile([2, S], F32, name="dstat_ln1", tag="dstat")
            drow1 = dsc.tile([2, S], BF16, name="drow_ln1", tag="drow")
            HCH = CH // 2   # 256: skip conv in half-chunks for smaller staging
            for ch in range(NCH):
                sl = slice(ch * CH, (ch + 1) * CH)
                ps = psst.tile([2, CH], F32, name="lnps_ln1", tag="st")
                ln_stats_chunk(lambda ct: xb[:, ct, :], ps, sl)
                stc = sml.tile([2, CH], F32, name="stc_ln1", tag="stc", bufs=2)
                nc.scalar.copy(out=stc, in_=ps)
                nc.sync.dma_start(out=dstat1[:, sl], in_=stc)
                # skip conv for this chunk (2 half-chunks)
                for hh in range(2):
                    hsl = slice(ch * CH + hh * HCH, ch * CH + (hh + 1) * HCH)
                    h0 = ch * 8 + hh * 4
                    skc = wsk.tile([128, 4, HCH], F32, name="skc", tag="skc")
                    nc.sync.dma_start(
                        out=skc,
                        in_=skin[:, hsl].rearrange("(kt p) n -> p kt n", p=128))
                    skb = wsk.tile([128, 4, HCH], BF16, name="skb", tag="skb", bufs=1)
                    nc.gpsimd.tensor_copy(out=skb, in_=skc)
                    for mt in range(2):
                        psk = psmm.tile([128, HCH], F32, name="sp_ps", tag="mm")
                        for kt in range(4):
                            nc.tensor.matmul(out=psk,
                                             lhsT=sptb[:, kt, 128 * mt:128 * (mt + 1)],
                                             rhs=skb[:, kt, :],
                                             start=(kt == 0), stop=(kt == 3))
                        nc.scalar.activation(
                            out=xcp_g1[mt][:, 1 + h0:5 + h0, 1:65],
                            in_=psk.rearrange("p (a b) -> p a b", a=4),
                            func=Act.Identity, bias=col('spb', mt))
            # LN1 global math + per-chunk broadcast/apply
            sm = sml.tile([128, 2, 32], F32, name="sm_ln1", tag="sm", bufs=2)
            nc.sync.dma_start(out=sm, in_=view(dstat1[:, :], [[1, 128], [S, 2], [128, 32]]))
            nc.vector.tensor_scalar_mul(out=sm, in0=sm, scalar1=1.0 / C)
            t2 = sml.tile([128, 32], F32, name="t2_ln1", tag="t2", bufs=2)
            nc.vector.tensor_tensor(out=t2, in0=sm[:, 0, :], in1=sm[:, 0, :], op=MM)
            nc.vector.tensor_tensor(out=t2, in0=sm[:, 1, :], in1=t2, op=SU)
            nc.scalar.activation(out=t2, in_=t2, func=Act.Sqrt, bias=epsc)
            nc.vector.reciprocal(out=t2, in_=t2)
            smb = sml.tile([128, 2, 32], BF16, name="smb_ln1", tag="smb", bufs=2)
            nc.vector.tensor_copy(out=smb[:, 0, :], in_=t2)
            nc.vector.tensor_tensor(out=smb[:, 1, :], in0=sm[:, 0, :], in1=t2, op=MM)
            nc.sync.dma_start(out=view(drow1[:, :], [[1, 128], [S, 2], [128, 32]]),
                              in_=smb)
            for ch in range(NCH):
                sl = slice(ch * CH, (ch + 1) * CH)
                bcrc = sml.tile([2, CH], BF16, name="bcr_ln1", tag="bcrc", bufs=3)
                nc.sync.dma_start(out=bcrc, in_=drow1[:, sl])
                pr = psbc.tile([128, CH], F32, name="pr_ln1", tag="pr")
                pm = psbc.tile([128, CH], F32, name="pm_ln1", tag="pm")
                nc.tensor.matmul(out=pr, lhsT=bc1[:, 0, :], rhs=bcrc)
                nc.tensor.matmul(out=pm, lhsT=bc1[:, 1, :], rhs=bcrc)
                for ct in range(2):
                    ln1_wr(ct, sl, pr, pm)
            if not ln1_triv:
                for ct in range(2):
                    nc.vector.tensor_scalar(
                        out=xn[:, ct, :], in0=xn[:, ct, :],
                        scalar1=col('ln1w', ct), scalar2=col('ln1b', ct),
                        op0=MM, op1=AD)
            if probe:
                pxn = mkprobe("p_xn", [C, S])
                for ct in range(2):
                    nc.gpsimd.dma_start(out=pxn[128 * ct:128 * (ct + 1), :],
                                        in_=xn[:, ct, :])

            # ============ q_shift diff * mask -> mdf (bf16) ============
            xn4 = xn.rearrange("p t (h w) -> p t h w", h=H)
            md = big.tile([128, 2, H, W], BF16, name="md", tag="Xb")
            nc.vector.tensor_tensor(out=md[0:64, 0, :, 1:], in0=xn4[0:64, 0, :, 0:63],
                                    in1=xn4[0:64, 0, :, 1:], op=SU)
            nc.vector.tensor_scalar_mul(out=md[0:64, 0, :, 0:1],
                                        in0=xn4[0:64, 0, :, 0:1], scalar1=-1.0)
            nc.vector.tensor_tensor(out=md[64:128, 0, :, 0:63], in0=xn4[64:128, 0, :, 1:],
                                    in1=xn4[64:128, 0, :, 0:63], op=SU)
            nc.vector.tensor_scalar_mul(out=md[64:128, 0, :, 63:64],
                                        in0=xn4[64:128, 0, :, 63:64], scalar1=-1.0)
            nc.gpsimd.tensor_tensor(out=md[0:64, 1, 1:, :], in0=xn4[0:64, 1, 0:63, :],
                                    in1=xn4[0:64, 1, 1:, :], op=SU)
            nc.gpsimd.tensor_scalar_mul(out=md[0:64, 1, 0:1, :],
                                        in0=xn4[0:64, 1, 0:1, :], scalar1=-1.0)
            nc.gpsimd.tensor_tensor(out=md[64:128, 1, 0:63, :], in0=xn4[64:128, 1, 1:, :],
                                    in1=xn4[64:128, 1, 0:63, :], op=SU)
            nc.gpsimd.tensor_scalar_mul(out=md[64:128, 1, 63:64, :],
                                        in0=xn4[64:128, 1, 63:64, :], scalar1=-1.0)
            mdf = md.rearrange("p t h w -> p t (h w)")
            nc.vector.tensor_tensor(out=mdf[:, 0, :], in0=mdf[:, 0, :], in1=mfb, op=MM)
            nc.gpsimd.tensor_tensor(out=mdf[:, 1, :], in0=mdf[:, 1, :], in1=mfb, op=MM)

            # ============ phase B: k/v matmuls, ev; then scans ============
            et = big.tile([128, 2, S], BF16, name="et", tag="A")
            vv = big.tile([128, 2, S], BF16, name="vv", tag="D")
            ev = big.tile([128, 2, S], BF16, name="ev", tag="B")
            sr = big.tile([128, 2, S], BF16, name="sr", tag="Fsr")

            def kv_chunk(widx, ch, evac):
                wxt, wdt = kvrw[widx]
                sl = slice(ch * CH, (ch + 1) * CH)
                for mt in range(2):
                    ps = psmm.tile([128, CH], F32, name="kv_ps", tag="mm")
                    for kt in range(2):
                        nc.tensor.matmul(out=ps, lhsT=wxt[:, kt, 128 * mt:128 * (mt + 1)],
                                         rhs=xn[:, kt, sl], start=(kt == 0), stop=False)
                    for kt in range(2):
                        nc.tensor.matmul(out=ps, lhsT=wdt[:, kt, 128 * mt:128 * (mt + 1)],
                                         rhs=mdf[:, kt, sl], start=False, stop=(kt == 1))
                    evac(mt, sl, ps)

            for ch in range(NCH):
                kv_chunk(0, ch, lambda mt, sl, ps: nc.scalar.activation(
                    out=et[:, mt, sl], in_=ps, func=Act.Exp))
            for ch in range(NCH):
                kv_chunk(1, ch, lambda mt, sl, ps: nc.scalar.copy(
                    out=vv[:, mt, sl], in_=ps))
                sl = slice(ch * CH, (ch + 1) * CH)
                nc.gpsimd.tensor_tensor(out=ev[:, :, sl], in0=et[:, :, sl],
                                        in1=vv[:, :, sl], op=MM)

            ev4 = ev.rearrange("p t (h w) -> p t h w", h=H)
            et4 = et.rearrange("p t (h w) -> p t h w", h=H)
            outv = big.tile([128, 2, H, W], BF16, name="outv", tag="D")  # h-major
            lt_ap = lt[:, :, :]

            def lamview(ct, nseq):
                return view(lt_ap, [lt_ap.ap[0], [0, nseq], [1, 64]], off=ct * 64)

            # ---- vertical scans (read pristine ev/et via transposed APs),
            # outputs written h-major with a leading zero row (h-1 shift).
            def vscan_group(half):
                wr_ = slice(half * 32, (half + 1) * 32)
                for ct in range(2):
                    avh = scn.tile([128, 65, 32], BF16, name="avh", tag="scnt")
                    bvh = scn.tile([128, 65, 32], BF16, name="bvh", tag="scnt")
                    nc.vector.memset(avh[:, 0:1, :], 0.0)
                    nc.vector.memset(bvh[:, 0:1, :], 0.0)
                    dv_ev = view(ev[:, :, :], [ev.ap[0], [1, 32], [64, 64]],
                                 off=ct * S + half * 32)
                    dv_et = view(et[:, :, :], [et.ap[0], [1, 32], [64, 64]],
                                 off=ct * S + half * 32)
                    scan_raw(view(avh[:, :, :], [avh.ap[0], [1, 32], [32, 64]], off=32),
                             lamview(ct, 32), dv_ev)
                    scan_raw(view(bvh[:, :, :], [bvh.ap[0], [1, 32], [32, 64]], off=32),
                             lamview(ct, 32), dv_et)
                    den = dnp.tile([128, 64, 32], F32, name="denv", tag="den")
                    nc.vector.scalar_tensor_tensor(
                        out=den, in0=et4[:, ct, :, wr_],
                        scalar=col('eu', ct), in1=bvh[:, 0:64, :], op0=MM, op1=AD)
                    nc.vector.reciprocal_approx_fast(out=den, in_=den)
                    nc.vector.scalar_tensor_tensor(
                        out=outv[:, ct, :, wr_], in0=ev4[:, ct, :, wr_],
                        scalar=col('eu', ct), in1=avh[:, 0:64, :], op0=MM, op1=AD)
                    nc.gpsimd.tensor_tensor(out=outv[:, ct, :, wr_],
                                            in0=outv[:, ct, :, wr_], in1=den, op=MM)

            # ---- horizontal scans; num/out in place on ev; zero-padded cols
            def hscan_group(half):
                hr = slice(half * 32, (half + 1) * 32)
                for ct in range(2):
                    ahz = scn.tile([128, 32, 66], BF16, name="ahz", tag="scnt")
                    bhz = scn.tile([128, 32, 66], BF16, name="bhz", tag="scnt")
                    nc.vector.memset(ahz[:, :, 0:1], 0.0)
                    nc.vector.memset(bhz[:, :, 0:1], 0.0)
                    scan_raw(view(ahz[:, :, :], [ahz.ap[0], [66, 32], [1, 64]], off=1),
                             lamview(ct, 32), ev4[:, ct, hr, :])
                    scan_raw(view(bhz[:, :, :], [bhz.ap[0], [66, 32], [1, 64]], off=1),
                             lamview(ct, 32), et4[:, ct, hr, :])
                    den = dnp.tile([128, 32, 64], F32, name="denh", tag="den")
                    nc.vector.scalar_tensor_tensor(
                        out=den, in0=et4[:, ct, hr, :],
                        scalar=col('eu', ct), in1=bhz[:, :, 0:64], op0=MM, op1=AD)
                    nc.vector.reciprocal_approx_fast(out=den, in_=den)
                    nc.vector.scalar_tensor_tensor(
                        out=ev4[:, ct, hr, :], in0=ev4[:, ct, hr, :],
                        scalar=col('eu', ct), in1=ahz[:, :, 0:64], op0=MM, op1=AD)
                    nc.gpsimd.tensor_tensor(out=ev4[:, ct, hr, :],
                                            in0=ev4[:, ct, hr, :], in1=den, op=MM)

            # ============ phase C: scans on DVE; r-proj + c1(skip grp) on PE ===
            vscan_group(0)
            vscan_group(1)

            for ch in range(NCH):
                kv_chunk(2, ch, lambda mt, sl, ps: nc.scalar.activation(
                    out=sr[:, mt, sl], in_=ps, func=Act.Sigmoid))

            y1b = big.tile([128, 2, S], BF16, name="y1b", tag="Xb")

            def c1_group(g, ytile, xtiles):
                for mt in range(2):
                    c1gm = wstr.tile([128, 9, 2, 128], BF16, name="c1gm", tag="wst9")
                    nc.sync.dma_start(out=c1gm, in_=d_c1[g, mt, :, :, :, :])
                    for ch in range(NCH):
                        h0 = ch * 8
                        ps = psmm.tile([128, CH], F32, name="c1_ps", tag="mm")
                        i = 0
                        for ti in range(9):
                            dy, dx = ti // 3 - 1, ti % 3 - 1
                            for kt in range(2):
                                nc.tensor.matmul(
                                    out=ps.rearrange("p (a b) -> p a b", a=8),
                                    lhsT=c1gm[:, ti, kt, :],
                                    rhs=xtiles[kt][:, 1 + h0 + dy:9 + h0 + dy,
                                                   1 + dx:65 + dx],
                                    start=(i == 0), stop=(i == 17))
                                i += 1
                        nc.scalar.activation(
                            out=ytile[:, mt, ch * CH:(ch + 1) * CH], in_=ps,
                            func=Act.Gelu, bias=col('c1b', 2 * g + mt))

            c1_group(1, y1b, xcp_g1)

            hscan_group(0)
            hscan_group(1)

            # wkv = out_h + out_v (0.5 factor dropped: LN-invariant)
            for ct in range(2):
                nc.vector.tensor_tensor(out=ev4[:, ct, :, :], in0=ev4[:, ct, :, :],
                                        in1=outv[:, ct, :, :], op=AD)
            if probe:
                pwkv = mkprobe("p_wkv", [C, S])
                for ct in range(2):
                    nc.gpsimd.dma_start(out=pwkv[128 * ct:128 * (ct + 1), :],
                                        in_=ev[:, ct, :])

            # ============ phase D: key-LN, srw, Wo+residual ============
            def kn_wr(ct, sl, pr, pm):
                nc.vector.tensor_tensor(out=ev[:, ct, sl], in0=ev[:, ct, sl], in1=pr, op=MM)
                nc.vector.tensor_tensor(out=ev[:, ct, sl], in0=ev[:, ct, sl], in1=pm, op=SU)

            ln256(lambda ct: ev[:, ct, :], kn_wr, "kn",
                  wb=None if kn_triv else ("knw", "knb", lambda ct: ev[:, ct, :]))

            xcp_g0 = []
            for i in range(2):
                t = xcpp.tile([128, 66, 66], BF16, name=f"xcp{i}", tag="xcp")
                nc.vector.memset(t[:, 0:1, :], 0.0)
                nc.vector.memset(t[:, 65:66, :], 0.0)
                nc.vector.memset(t[:, 1:65, 0:1], 0.0)
                nc.vector.memset(t[:, 1:65, 65:66], 0.0)
                xcp_g0.append(t)

            for ch in range(NCH):
                sl = slice(ch * CH, (ch + 1) * CH)
                h0 = ch * 8
                nc.gpsimd.tensor_tensor(out=sr[:, :, sl], in0=sr[:, :, sl],
                                        in1=ev[:, :, sl], op=MM)     # srw
                for mt in range(2):
                    ps = psmm.tile([128, CH], F32, name="wo_ps", tag="mm")
                    for kt in range(2):
                        nc.tensor.matmul(out=ps, lhsT=wot[:, kt, 128 * mt:128 * (mt + 1)],
                                         rhs=sr[:, kt, sl], start=(kt == 0), stop=(kt == 1))
                    nc.vector.tensor_tensor(
                        out=xcp_g0[mt][:, 1 + h0:9 + h0, 1:65],
                        in0=xn4[:, mt, h0:h0 + 8, :],
                        in1=ps.rearrange("p (a b) -> p a b", a=8), op=AD)
            if probe:
                pxc = mkprobe("p_xcat", [CS, S])
                for i, t in enumerate(xcp_g0 + xcp_g1):
                    nc.gpsimd.dma_start(
                        out=pxc[128 * i:128 * (i + 1), :].rearrange("p (a b) -> p a b", a=64),
                        in_=t[:, 1:65, 1:65])

            # ============ phase E: grouped conv, main group ============
            y1a = big.tile([128, 2, S], BF16, name="y1a", tag="A")
            c1_group(0, y1a, xcp_g0)
            y1t = [y1a, y1b]

            if probe:
                py1 = mkprobe("p_y1", [CS, S])
                for i in range(4):
                    nc.gpsimd.dma_start(out=py1[128 * i:128 * (i + 1), :],
                                        in_=y1t[i // 2][:, i % 2, :])

            # ============ phase F: c2 -> gelu -> c3 -> gelu(+bn3) -> y3 ======
            y3 = [big.tile([128, S], BF16, name="y3_0", tag="Cxn"),
                  big.tile([128, S], BF16, name="y3_1", tag="Mf")]
            c3wt = wstr.tile([128, 8, 256], BF16, name="c3wt", tag="wst9")
            nc.sync.dma_start(out=c3wt, in_=d_c3[:, :].rearrange("(kt p) m -> p kt m", p=128))
            for ch in range(NCH):
                sl = slice(ch * CH, (ch + 1) * CH)
                ytiles = []
                for mt in range(8):
                    ps = psmm.tile([128, CH], F32, name="c2_ps", tag="mm")
                    for kt in range(4):
                        nc.tensor.matmul(out=ps, lhsT=c2wt[:, kt, 128 * mt:128 * (mt + 1)],
                                         rhs=y1t[kt // 2][:, kt % 2, sl],
                                         start=(kt == 0), stop=(kt == 3))
                    yt = y2b.tile([128, CH], BF16, name="y2t", tag="y2t")
                    nc.scalar.activation(out=yt, in_=ps, func=Act.Gelu, bias=col('c2b', mt))
                    ytiles.append(yt)
                for mt in range(2):
                    ps = psmm.tile([128, CH], F32, name="c3_ps", tag="mm")
                    for kt in range(8):
                        nc.tensor.matmul(out=ps, lhsT=c3wt[:, kt, 128 * mt:128 * (mt + 1)],
                                         rhs=ytiles[kt], start=(kt == 0), stop=(kt == 7))
                    nc.scalar.activation(out=y3[mt][:, sl], in_=ps, func=Act.Gelu,
                                         bias=col('c3b', mt))
                    if not bn3_triv:
                        nc.vector.tensor_scalar(out=y3[mt][:, sl], in0=y3[mt][:, sl],
                                                scalar1=col('g3p', mt),
                                                scalar2=col('b3p', mt), op0=MM, op1=AD)

            if probe:
                py3 = mkprobe("p_y3", [C, S])
                for i in range(2):
                    nc.gpsimd.dma_start(out=py3[128 * i:128 * (i + 1), :],
                                        in_=y3[i][:, :])

            # ============ phase G: LN2, up-proj, pixel-shuffle DMA out =======
            def ln2_wr(ct, sl, pr, pm):
                nc.vector.tensor_tensor(out=y3[ct][:, sl], in0=y3[ct][:, sl],
                                        in1=pr, op=MM)
                nc.vector.tensor_tensor(out=y3[ct][:, sl], in0=y3[ct][:, sl],
                                        in1=pm, op=SU)

            ln256(lambda ct: y3[ct][:, :], ln2_wr, "ln2",
                  wb=None if ln2_triv else ("ln2w", "ln2b", lambda ct: y3[ct][:, :]))

            uptb = wstr.tile([128, 2, 512], BF16, name="uptb", tag="wst9")
            nc.sync.dma_start(out=uptb, in_=d_up[:, :].rearrange("(kt p) m -> p kt m", p=128))
            for r in range(2):
                for ch in range(NCH):
                    sl = slice(ch * CH, (ch + 1) * CH)
                    h0 = ch * 8
                    ub = wsk.tile([128, 8, 64, 2], F32, name="ub", tag="skc")
                    for q in range(2):
                        rq = 2 * r + q
                        ps = psmm.tile([128, CH], F32, name="up_ps", tag="mm")
                        for kt in range(2):
                            nc.tensor.matmul(out=ps,
                                             lhsT=uptb[:, kt, 128 * rq:128 * (rq + 1)],
                                             rhs=y3[kt][:, sl],
                                             start=(kt == 0), stop=(kt == 1))
                        nc.scalar.activation(out=ub[:, :, :, q],
                                             in_=ps.rearrange("p (a b) -> p a b", a=8),
                                             func=Act.Identity, bias=col('upb', rq))
                    dst = view(yout[:, :, :], [[128 * 128, 128], [256, 8], [1, 128]],
                               off=(2 * h0 + r) * 128)
                    nc.sync.dma_start(out=dst, in_=ub.rearrange("p a b q -> p a (b q)"))

    nc.compile()
    return nc, const_inputs


def _get_nc(weights, probe=False):
    import hashlib
    hsh = hashlib.sha1()
    for k in sorted(weights):
        hsh.update(k.encode())
        hsh.update(np.ascontiguousarray(weights[k]).tobytes())
    key = (hsh.hexdigest(), probe)
    if key not in _CACHE:
        _CACHE[key] = _build(weights, probe=probe)
    return _CACHE[key]


def kernel(**inputs):
    from concourse.bass_utils import run_bass_kernel_spmd

    x = np.asarray(inputs['x'], np.float32)
    skip = np.asarray(inputs['skip'], np.float32)
    mask = np.asarray(inputs['saliency_mask'], np.float32)
    weights = {k: np.asarray(v, np.float32) for k, v in inputs.items()
               if k not in ('x', 'skip', 'saliency_mask')}

    probe = bool(os.environ.get('BASSK_PROBE'))
    nc, const_inputs = _get_nc(weights, probe=probe)

    in_maps = []
    for b in range(B):
        m = dict(
            xin=np.ascontiguousarray(x[b].reshape(C, S)),
            skin=np.ascontiguousarray(skip[b].reshape(CS, S)),
            mrow=np.ascontiguousarray(mask[b].reshape(1, S)),
        )
        m.update(const_inputs)
        in_maps.append(m)
    res = run_bass_kernel_spmd(nc, in_maps, core_ids=list(range(B)),
                               trace=bool(os.environ.get('BASSK_TRACE')))
    kernel.last_results = res
    out = np.stack([res.results[b]['yout'] for b in range(B)], axis=0)
    return out


# revision 18
# speedup vs baseline: 1.2594x; 1.0671x over previous
"""Trainium2 Bass kernel for nn_DecoderBlock (shape-guided RWKV decoder block).

Data-parallel over batch: B=8 samples -> 8 NeuronCores, one NEFF.

v3: fully pipelined per-chunk structure for PE occupancy (HAM clock-gate)
and engine balance:
- all matmuls bf16 (FWL enabled, no fp32-HIGH power throttle);
- LN1 tail loop fuses broadcast/apply with per-chunk q_shift-diff (md) and
  the k/v projections + ev product, so the PE never waits on a serial md;
- WKV scans write near-contiguous zero-padded tiles; r-projection and the
  skip-group half of the 3x3 conv run on PE during the scan phase;
- key-LN tail loop fuses broadcast/apply/srw/Wo with the main-group 3x3
  conv at 1-chunk lag; c2/c3 loop fuses LN2 stats; LN2 tail fuses up-proj
  and output DMA;
- Pool (gpsimd) takes strided edge ops, scan-output multiplies, skip
  converts, and LN bounce DMA issue; DVE keeps scans/den/num/applies.
"""
import sys
import os

for _p in ('/opt/trn_rl_repo', '/root/.axon_site/_ro/trn_rl_repo'):
    if _p not in sys.path and os.path.isdir(_p):
        sys.path.append(_p)

import numpy as np

B, C, CS, COUT, H, W = 8, 256, 512, 128, 64, 64
S = H * W          # 4096
NCH = 8            # spatial chunks
CH = S // NCH      # 512
EPS = 1e-5

_CACHE = {}


def _build(weights, probe=False):
    const_inputs = {}
    import concourse.bass as bass
    from concourse import bacc
    import concourse.tile as tile
    import concourse.mybir as mybir
    import ml_dtypes

    F32 = mybir.dt.float32
    BF16 = mybir.dt.bfloat16
    Alu = mybir.AluOpType
    Act = mybir.ActivationFunctionType
    MM, AD, SU = Alu.mult, Alu.add, Alu.subtract

    w = weights
    f64 = lambda x: np.asarray(x, np.float64)
    bf = lambda a: np.asarray(a, dtype=ml_dtypes.bfloat16)

    # ---------------- host-side folding
    bnscale = 1.0 / np.sqrt(1.0 + EPS)
    g1p = f64(w['bn1_g']) * bnscale
    b1p = f64(w['bn1_b'])
    g2p = f64(w['bn2_g']) * bnscale
    b2p = f64(w['bn2_b'])
    g3p = (f64(w['bn3_g']) * bnscale).astype(np.float32)
    b3p = f64(w['bn3_b']).astype(np.float32)

    c2_eff = f64(w['c2_w']) * g1p[None, :]
    c2b_eff = (f64(w['c2_b']) + f64(w['c2_w']) @ b1p).astype(np.float32)
    c3_eff = f64(w['c3_w']) * g2p[None, :]
    c3b_eff = (f64(w['c3_b']) + f64(w['c3_w']) @ b2p).astype(np.float32)

    wk_x = f64(w['Wk']).T
    wk_d = (f64(w['Wk']) * (1.0 - f64(w['mix_k']))[None, :]).T
    wv_x = f64(w['Wv']).T
    wv_d = (f64(w['Wv']) * (1.0 - f64(w['mix_v']))[None, :]).T
    wr_x = f64(w['Wr']).T
    wr_d = (f64(w['Wr']) * (1.0 - f64(w['mix_r']))[None, :]).T
    wo_t = f64(w['Wo']).T
    sp_t = f64(w['sp_w']).T

    lam = np.exp(-np.exp(f64(w['decay']))).astype(np.float32)
    lam64 = np.tile(lam[:, None], (1, 64))
    lam64[:, 0] = 0.0
    lam64 = lam64.astype(np.float32)
    eu = np.exp(f64(w['first'])).astype(np.float32)

    pidx = np.arange(512)
    old = (pidx % 128) * 4 + (pidx // 128)
    up_t = f64(w['up_w'])[old].T                                # [256, 512]
    upb_p = f64(w['up_b'])[old].astype(np.float32)

    c1w = f64(w['c1_w'])
    c1_l = np.zeros((9, 2, 256, 256), np.float32)
    for ti in range(9):
        dy, dx = ti // 3, ti % 3
        for g in range(2):
            c1_l[ti, g] = c1w[g * 256:(g + 1) * 256, :, dy, dx].T

    # per-channel vectors as columns of one [128, ncol] const
    cols, order = {}, []

    def addcol(name, vec):
        v = np.asarray(vec, np.float32).reshape(-1, 128)
        cols[name] = v
        order.append(name)

    addcol('eu', eu)
    addcol('ln1w', w['ln1_w'])
    addcol('ln1b', w['ln1_b'])
    addcol('knw', w['kn_w'])
    addcol('knb', w['kn_b'])
    addcol('ln2w', w['ln2_w'])
    addcol('ln2b', w['ln2_b'])
    addcol('g3p', g3p)
    addcol('b3p', b3p)
    addcol('spb', w['sp_b'])
    addcol('c3b', c3b_eff)
    addcol('c1b', w['c1_b'])
    addcol('c2b', c2b_eff)
    addcol('upb', upb_p)
    colidx, ncol = {}, 0
    for n in order:
        colidx[n] = ncol
        ncol += cols[n].shape[0]
    cvec_np = np.zeros((128, ncol), np.float32)
    for n in order:
        for i in range(cols[n].shape[0]):
            cvec_np[:, colidx[n] + i] = cols[n][i]

    ln1_triv = np.all(w['ln1_w'] == 1.0) and np.all(w['ln1_b'] == 0.0)
    kn_triv = np.all(w['kn_w'] == 1.0) and np.all(w['kn_b'] == 0.0)
    ln2_triv = np.all(w['ln2_w'] == 1.0) and np.all(w['ln2_b'] == 0.0)
    bn3_triv = np.all(g3p == g3p[0]) and np.all(b3p == 0.0)
    # uniform bn3 scale commutes with LN2 -> drop it entirely when trivial

    # ---------------- bass module
    nc = bacc.Bacc("TRN2", target_bir_lowering=False, debug=False, name="decblk")

    xin = nc.dram_tensor("xin", [C, S], F32, kind="ExternalInput")
    skin = nc.dram_tensor("skin", [CS, S], F32, kind="ExternalInput")
    mrow = nc.dram_tensor("mrow", [1, S], F32, kind="ExternalInput")
    yout = nc.dram_tensor("yout", [COUT, 2 * H, 2 * W], F32, kind="ExternalOutput")
    probes = {}

    def mkprobe(name, shape):
        if probe:
            probes[name] = nc.dram_tensor(name, shape, F32, kind="ExternalOutput")
        return probes.get(name)

    def it(arr, name):
        arr = np.ascontiguousarray(arr)
        import ml_dtypes as _md
        dt_ = {np.dtype(np.float32): F32, np.dtype(_md.bfloat16): BF16}[arr.dtype]
        const_inputs[name] = arr
        return nc.dram_tensor(name, list(arr.shape), dt_, kind="ExternalInput")

    d_lam = it(lam64, "lam64")
    d_cvec = it(cvec_np, "cvec")
    d_wk = [it(bf(wk_x), "wkx"), it(bf(wk_d), "wkd")]
    d_wv = [it(bf(wv_x), "wvx"), it(bf(wv_d), "wvd")]
    d_wr = [it(bf(wr_x), "wrx"), it(bf(wr_d), "wrd")]
    d_wo = it(bf(wo_t), "wo")
    d_sp = it(bf(sp_t), "sp")
    d_up = it(bf(up_t), "up")
    c1_r = c1_l.reshape(9, 2, 2, 128, 2, 128).transpose(1, 4, 3, 0, 2, 5)
    d_c1 = it(bf(c1_r), "c1")   # [g, mt, p, t, kt, m]
    d_c2 = it(bf(c2_eff.T), "c2")
    d_c3 = it(bf(c3_eff.T), "c3")
    red_np = np.zeros((128, 2, 2), np.float32)
    red_np[:, 0, 0] = 1.0
    red_np[:, 1, 1] = 1.0
    d_redb = it(bf(red_np), "redb")
    bc2_np = np.zeros((2, 2, 128), np.float32)
    bc2_np[0, 0, :] = 1.0
    bc2_np[1, 1, :] = 1.0
    d_bc1 = it(bf(bc2_np), "bc2")
    d_eps = it(np.full((128, 1), EPS, np.float32), "epsc")

    def scan_raw(out, d0, d1):
        eng = nc.vector
        return eng.add_instruction(mybir.InstTensorScalarPtr(
            name=nc.get_next_instruction_name(),
            is_tensor_tensor_scan=True,
            is_scalar_tensor_tensor=True,
            op0=MM, op1=AD,
            ins=[eng.lower_ap(d0), eng.lower_ap_or_imm(0.0), eng.lower_ap(d1)],
            outs=[eng.lower_ap(out)],
        ))

    def view(ap, dims, off=0):
        return bass.AP(tensor=ap.tensor, offset=ap.offset + off, ap=dims)

    with tile.TileContext(nc) as tc:
        with tc.tile_pool(name="big", bufs=1) as big, \
             tc.tile_pool(name="dnp", bufs=2) as dnp, \
             tc.tile_pool(name="scn", bufs=3) as scn, \
             tc.tile_pool(name="wres", bufs=1) as wres, \
             tc.tile_pool(name="sml", bufs=3) as sml, \
             tc.tile_pool(name="y2b", bufs=8) as y2b, \
             tc.tile_pool(name="wstr", bufs=2) as wstr, \
             tc.tile_pool(name="wsk", bufs=2) as wsk, \
             tc.tile_pool(name="xcpp", bufs=2) as xcpp, \
             tc.tile_pool(name="dsc", bufs=2, space="DRAM") as dsc, \
             tc.tile_pool(name="psmm", bufs=3, space="PSUM") as psmm, \
             tc.tile_pool(name="psst", bufs=1, space="PSUM") as psst, \
             tc.tile_pool(name="psbc", bufs=2, space="PSUM") as psbc:

            # ---- resident constants
            lt = wres.tile([128, 2, 64], F32, name="lt")
            nc.sync.dma_start(out=lt, in_=d_lam[:, :].rearrange("(t p) j -> p t j", p=128))
            cv = wres.tile([128, ncol], F32, name="cv")
            nc.sync.dma_start(out=cv, in_=d_cvec[:, :])
            redb = wres.tile([128, 2, 2], BF16, name="redb")
            nc.sync.dma_start(out=redb, in_=d_redb[:, :, :])
            bc1 = wres.tile([2, 2, 128], BF16, name="bc1")
            nc.sync.dma_start(out=bc1, in_=d_bc1[:, :, :])
            epsc = wres.tile([128, 1], F32, name="epsc")
            nc.sync.dma_start(out=epsc, in_=d_eps[:, :])
            wot = wres.tile([128, 2, 256], BF16, name="wot")
            nc.sync.dma_start(out=wot, in_=d_wo[:, :].rearrange("(kt p) m -> p kt m", p=128))
            kvrw = []
            for nm, dws in (("wk", d_wk), ("wv", d_wv), ("wr", d_wr)):
                wxt = wres.tile([128, 2, 256], BF16, name=f"{nm}x")
                wdt = wres.tile([128, 2, 256], BF16, name=f"{nm}d")
                nc.sync.dma_start(out=wxt, in_=dws[0][:, :].rearrange("(kt p) m -> p kt m", p=128))
                nc.sync.dma_start(out=wdt, in_=dws[1][:, :].rearrange("(kt p) m -> p kt m", p=128))
                kvrw.append((wxt, wdt))
            c2wt = wres.tile([128, 4, 1024], BF16, name="c2wt")
            nc.sync.dma_start(out=c2wt, in_=d_c2[:, :].rearrange("(kt p) m -> p kt m", p=128))

            def col(name, i=0):
                return cv[:, colidx[name] + i:colidx[name] + i + 1]

            def ln_stats_chunk(Xr, ps, sl, name):
                nc.tensor.matmul(out=ps, lhsT=redb[:, 0, :], rhs=Xr(0)[:, sl],
                                 start=True, stop=False)
                nc.tensor.matmul(out=ps, lhsT=redb[:, 0, :], rhs=Xr(1)[:, sl],
                                 start=False, stop=False)
                for ct in range(2):
                    sq = sml.tile([128, CH], BF16, name=f"sq_{name}", tag="sqc", bufs=2)
                    nc.scalar.activation(out=sq, in_=Xr(ct)[:, sl], func=Act.Square)
                    nc.tensor.matmul(out=ps, lhsT=redb[:, 1, :], rhs=sq,
                                     start=False, stop=(ct == 1))

            def ln_stats_bounce(Xr, dstat, ch, name):
                sl = slice(ch * CH, (ch + 1) * CH)
                ps = psst.tile([2, CH], F32, name=f"lnps_{name}", tag="st")
                ln_stats_chunk(Xr, ps, sl, name)
                stc = sml.tile([2, CH], F32, name=f"stc_{name}", tag="stc", bufs=2)
                nc.scalar.copy(out=stc, in_=ps)
                nc.gpsimd.dma_start(out=dstat[:, sl], in_=stc)

            def ln_math(dstat, drow, name):
                sm = sml.tile([128, 2, 32], F32, name=f"sm_{name}", tag="sm", bufs=2)
                nc.gpsimd.dma_start(out=sm, in_=view(dstat[:, :], [[1, 128], [S, 2], [128, 32]]))
                nc.vector.tensor_scalar_mul(out=sm, in0=sm, scalar1=1.0 / C)
                t2 = sml.tile([128, 32], F32, name=f"t2_{name}", tag="t2", bufs=2)
                nc.vector.tensor_tensor(out=t2, in0=sm[:, 0, :], in1=sm[:, 0, :], op=MM)
                nc.vector.tensor_tensor(out=t2, in0=sm[:, 1, :], in1=t2, op=SU)
                nc.scalar.activation(out=t2, in_=t2, func=Act.Sqrt, bias=epsc)
                nc.vector.reciprocal(out=t2, in_=t2)                      # rstd
                smb = sml.tile([128, 2, 32], BF16, name=f"smb_{name}", tag="smb", bufs=2)
                nc.vector.tensor_copy(out=smb[:, 0, :], in_=t2)
                nc.vector.tensor_tensor(out=smb[:, 1, :], in0=sm[:, 0, :], in1=t2, op=MM)
                nc.gpsimd.dma_start(out=view(drow[:, :], [[1, 128], [S, 2], [128, 32]]),
                                    in_=smb)

            def ln_bcast(drow, ch, name):
                sl = slice(ch * CH, (ch + 1) * CH)
                bcrc = sml.tile([2, CH], BF16, name=f"bcr_{name}", tag="bcrc", bufs=2)
                nc.gpsimd.dma_start(out=bcrc, in_=drow[:, sl])
                pr = psbc.tile([128, CH], F32, name=f"pr_{name}", tag="pr")
                pm = psbc.tile([128, CH], F32, name=f"pm_{name}", tag="pm")
                nc.tensor.matmul(out=pr, lhsT=bc1[:, 0, :], rhs=bcrc)
                nc.tensor.matmul(out=pm, lhsT=bc1[:, 1, :], rhs=bcrc)
                return pr, pm

            # ============ phase A: load x (pipelined halves), LN1, skip ======
            x0 = big.tile([128, S], F32, name="x0", tag="A")
            x1 = big.tile([128, S], F32, name="x1", tag="B")
            xb = big.tile([128, 2, S], BF16, name="xb", tag="Xb")
            HS = S // 2
            for hf in range(2):
                hsl = slice(hf * HS, (hf + 1) * HS)
                nc.sync.dma_start(out=x0[:, hsl], in_=xin[0:128, hsl])
                nc.vector.tensor_copy(out=xb[:, 0, hsl], in_=x0[:, hsl])
            for hf in range(2):
                hsl = slice(hf * HS, (hf + 1) * HS)
                nc.sync.dma_start(out=x1[:, hsl], in_=xin[128:256, hsl])
                nc.vector.tensor_copy(out=xb[:, 1, hsl], in_=x1[:, hsl])

            # mask: fp32 row -> bf16 (in [32,128] layout) -> DRAM -> broadcast
            m1 = sml.tile([32, 128], F32, name="m1", tag="m1", bufs=1)
            nc.sync.dma_start(out=m1, in_=view(mrow[:, :], [[128, 32], [1, 128]]))
            m1b = sml.tile([32, 128], BF16, name="m1b", tag="m1b", bufs=1)
            nc.vector.tensor_copy(out=m1b, in_=m1)
            dmask = dsc.tile([32, 128], BF16, name="dmask", tag="dmask", bufs=1)
            nc.sync.dma_start(out=dmask, in_=m1b)
            mfb = big.tile([128, S], BF16, name="mfb", tag="Mf")
            nc.sync.dma_start(out=mfb, in_=view(dmask[:, :], [[0, 128], [1, S]]))

            # xcp tiles for skip group written early; borders zeroed
            xcp_g1 = []
            for i in range(2):
                t = xcpp.tile([128, 66, 66], BF16, name=f"xcp{2 + i}", tag="xcp")
                nc.gpsimd.memset(t[:, 0:1, :], 0.0)
                nc.gpsimd.memset(t[:, 65:66, :], 0.0)
                nc.gpsimd.memset(t[:, 1:65, 0:1], 0.0)
                nc.gpsimd.memset(t[:, 1:65, 65:66], 0.0)
                xcp_g1.append(t)

            sptb = wstr.tile([128, 4, 256], BF16, name="sptb", tag="wst9")
            nc.sync.dma_start(out=sptb, in_=d_sp[:, :].rearrange("(kt p) m -> p kt m", p=128))

            xn = big.tile([128, 2, S], BF16, name="xn", tag="Cxn")
            xn4 = xn.rearrange("p t (h w) -> p t h w", h=H)

            # LN1 stats per chunk with skip-conv half-chunks interleaved
            dstat1 = dsc.tile([2, S], F32, name="dstat_ln1", tag="dstat")
            drow1 = dsc.tile([2, S], BF16, name="drow_ln1", tag="drow")
            HCH = CH // 2
            for ch in range(NCH):
                ln_stats_bounce(lambda ct: xb[:, ct, :], dstat1, ch, "ln1")
                for hh in range(2):
                    hsl = slice(ch * CH + hh * HCH, ch * CH + (hh + 1) * HCH)
                    h0 = ch * 8 + hh * 4
                    skc = wsk.tile([128, 4, HCH], F32, name="skc", tag="skc")
                    nc.sync.dma_start(
                        out=skc,
                        in_=skin[:, hsl].rearrange("(kt p) n -> p kt n", p=128))
                    skb = wsk.tile([128, 4, HCH], BF16, name="skb", tag="skb", bufs=1)
                    nc.gpsimd.tensor_copy(out=skb, in_=skc)
                    for mt in range(2):
                        psk = psmm.tile([128, HCH], F32, name="sp_ps", tag="mm")
                        for kt in range(4):
                            nc.tensor.matmul(out=psk,
                                             lhsT=sptb[:, kt, 128 * mt:128 * (mt + 1)],
                                             rhs=skb[:, kt, :],
                                             start=(kt == 0), stop=(kt == 3))
                        nc.scalar.activation(
                            out=xcp_g1[mt][:, 1 + h0:5 + h0, 1:65],
                            in_=psk.rearrange("p (a b) -> p a b", a=4),
                            func=Act.Identity, bias=col('spb', mt))
            ln_math(dstat1, drow1, "ln1")

            # ---- per-chunk q_shift diff (+mask) for rows [8ch, 8ch+8);
            # lives in a small ring tile, consumed by k/v/r of the same chunk
            def md_chunk(ch):
                r0 = ch * 8
                rs = slice(r0, r0 + 8)
                mdc = sml.tile([128, 2, 8, W], BF16, name="mdc", tag="mdc", bufs=2)
                # ct=0: w-shifts (chunk-local rows)
                nc.vector.tensor_tensor(out=mdc[0:64, 0, :, 1:],
                                        in0=xn4[0:64, 0, rs, 0:63],
                                        in1=xn4[0:64, 0, rs, 1:], op=SU)
                nc.gpsimd.tensor_scalar_mul(out=mdc[0:64, 0, :, 0:1],
                                            in0=xn4[0:64, 0, rs, 0:1], scalar1=-1.0)
                nc.vector.tensor_tensor(out=mdc[64:128, 0, :, 0:63],
                                        in0=xn4[64:128, 0, rs, 1:],
                                        in1=xn4[64:128, 0, rs, 0:63], op=SU)
                nc.gpsimd.tensor_scalar_mul(out=mdc[64:128, 0, :, 63:64],
                                            in0=xn4[64:128, 0, rs, 63:64], scalar1=-1.0)
                # ct=1: h-shifts (reads rows r0-1 .. r0+8)
                if ch == 0:
                    nc.gpsimd.tensor_scalar_mul(out=mdc[0:64, 1, 0:1, :],
                                                in0=xn4[0:64, 1, 0:1, :], scalar1=-1.0)
                    nc.vector.tensor_tensor(out=mdc[0:64, 1, 1:8, :],
                                            in0=xn4[0:64, 1, 0:7, :],
                                            in1=xn4[0:64, 1, 1:8, :], op=SU)
                else:
                    nc.vector.tensor_tensor(out=mdc[0:64, 1, :, :],
                                            in0=xn4[0:64, 1, r0 - 1:r0 + 7, :],
                                            in1=xn4[0:64, 1, rs, :], op=SU)
                if ch == NCH - 1:
                    nc.vector.tensor_tensor(out=mdc[64:128, 1, 0:7, :],
                                            in0=xn4[64:128, 1, 57:64, :],
                                            in1=xn4[64:128, 1, 56:63, :], op=SU)
                    nc.gpsimd.tensor_scalar_mul(out=mdc[64:128, 1, 7:8, :],
                                                in0=xn4[64:128, 1, 63:64, :], scalar1=-1.0)
                else:
                    nc.vector.tensor_tensor(out=mdc[64:128, 1, :, :],
                                            in0=xn4[64:128, 1, r0 + 1:r0 + 9, :],
                                            in1=xn4[64:128, 1, rs, :], op=SU)
                sl = slice(ch * CH, (ch + 1) * CH)
                mdr = mdc.rearrange("p t h w -> p t (h w)")
                for ct in range(2):
                    nc.gpsimd.tensor_tensor(out=mdr[:, ct, :], in0=mdr[:, ct, :],
                                            in1=mfb[:, sl], op=MM)
                return mdr

            et = big.tile([128, 2, S], BF16, name="et", tag="A")
            vv = big.tile([128, 2, S], BF16, name="vv", tag="D")
            ev = big.tile([128, 2, S], BF16, name="ev", tag="B")
            sr = big.tile([128, 2, S], BF16, name="sr", tag="Fsr")

            def kv_chunk(widx, ch, mdr, evac):
                wxt, wdt = kvrw[widx]
                sl = slice(ch * CH, (ch + 1) * CH)
                for mt in range(2):
                    ps = psmm.tile([128, CH], F32, name="kv_ps", tag="mm")
                    for kt in range(2):
                        nc.tensor.matmul(out=ps, lhsT=wxt[:, kt, 128 * mt:128 * (mt + 1)],
                                         rhs=xn[:, kt, sl], start=(kt == 0), stop=False)
                    for kt in range(2):
                        nc.tensor.matmul(out=ps, lhsT=wdt[:, kt, 128 * mt:128 * (mt + 1)],
                                         rhs=mdr[:, kt, :], start=False, stop=(kt == 1))
                    evac(mt, sl, ps)

            def kve_chunk(ch):
                mdr = md_chunk(ch)
                kv_chunk(0, ch, mdr, lambda mt, sl, ps: nc.scalar.activation(
                    out=et[:, mt, sl], in_=ps, func=Act.Exp))
                kv_chunk(1, ch, mdr, lambda mt, sl, ps: nc.scalar.copy(
                    out=vv[:, mt, sl], in_=ps))
                # r projection evacuated raw (Copy shares the exp ACT table);
                # sigmoid applied in phase C
                kv_chunk(2, ch, mdr, lambda mt, sl, ps: nc.scalar.copy(
                    out=sr[:, mt, sl], in_=ps))
                sl = slice(ch * CH, (ch + 1) * CH)
                nc.vector.tensor_tensor(out=ev[:, :, sl], in0=et[:, :, sl],
                                        in1=vv[:, :, sl], op=MM)

            # LN1 tail: broadcast/apply per chunk, fused with md/k/v/ev at lag 1
            for ch in range(NCH):
                pr, pm = ln_bcast(drow1, ch, "ln1")
                sl = slice(ch * CH, (ch + 1) * CH)
                for ct in range(2):
                    nc.vector.tensor_tensor(out=xn[:, ct, sl], in0=xb[:, ct, sl],
                                            in1=pr, op=MM)
                    nc.vector.tensor_tensor(out=xn[:, ct, sl], in0=xn[:, ct, sl],
                                            in1=pm, op=SU)
                if not ln1_triv:
                    for ct in range(2):
                        nc.vector.tensor_scalar(
                            out=xn[:, ct, sl], in0=xn[:, ct, sl],
                            scalar1=col('ln1w', ct), scalar2=col('ln1b', ct),
                            op0=MM, op1=AD)
                if ch >= 1:
                    kve_chunk(ch - 1)
            kve_chunk(NCH - 1)
            if probe:
                pxn = mkprobe("p_xn", [C, S])
                for ct in range(2):
                    nc.gpsimd.dma_start(out=pxn[128 * ct:128 * (ct + 1), :],
                                        in_=xn[:, ct, :])

            # ============ phase C: WKV scans (DVE) vs r/c1-skip (PE) =========
            ev4 = ev.rearrange("p t (h w) -> p t h w", h=H)
            et4 = et.rearrange("p t (h w) -> p t h w", h=H)
            outv = big.tile([128, 2, W, H], BF16, name="outv", tag="D")  # w-major
            lt_ap = lt[:, :, :]

            def lamview(ct, nseq):
                return view(lt_ap, [lt_ap.ap[0], [0, nseq], [1, 64]], off=ct * 64)

            # vertical: scan along h per (w, ct); outputs w-major, zero-padded
            # leading h column so den/num read the h-1 shift without edge ops.
            def vscan_group(half):
                wr_ = slice(half * 32, (half + 1) * 32)
                for ct in range(2):
                    avh = scn.tile([128, 32, 65], BF16, name="avh", tag="scnt")
                    bvh = scn.tile([128, 32, 65], BF16, name="bvh", tag="scnt")
                    nc.gpsimd.memset(avh[:, :, 0:1], 0.0)
                    nc.gpsimd.memset(bvh[:, :, 0:1], 0.0)
                    dv_ev = view(ev[:, :, :], [ev.ap[0], [1, 32], [64, 64]],
                                 off=ct * S + half * 32)
                    dv_et = view(et[:, :, :], [et.ap[0], [1, 32], [64, 64]],
                                 off=ct * S + half * 32)
                    scan_raw(view(avh[:, :, :], [avh.ap[0], [65, 32], [1, 64]], off=1),
                             lamview(ct, 32), dv_ev)
                    scan_raw(view(bvh[:, :, :], [bvh.ap[0], [65, 32], [1, 64]], off=1),
                             lamview(ct, 32), dv_et)
                    den = dnp.tile([128, 32, 64], F32, name="denv", tag="den")
                    nc.vector.scalar_tensor_tensor(
                        out=den, in0=dv_et,
                        scalar=col('eu', ct), in1=bvh[:, :, 0:64], op0=MM, op1=AD)
                    nc.vector.reciprocal_approx_fast(out=den, in_=den)
                    nc.vector.scalar_tensor_tensor(
                        out=outv[:, ct, wr_, :], in0=dv_ev,
                        scalar=col('eu', ct), in1=avh[:, :, 0:64], op0=MM, op1=AD)
                    nc.gpsimd.tensor_tensor(out=outv[:, ct, wr_, :],
                                            in0=outv[:, ct, wr_, :], in1=den, op=MM)

            # horizontal: scan along w per (h, ct); num/out in place on ev
            def hscan_group(half):
                hr = slice(half * 32, (half + 1) * 32)
                for ct in range(2):
                    ahz = scn.tile([128, 32, 66], BF16, name="ahz", tag="scnt")
                    bhz = scn.tile([128, 32, 66], BF16, name="bhz", tag="scnt")
                    nc.gpsimd.memset(ahz[:, :, 0:1], 0.0)
                    nc.gpsimd.memset(bhz[:, :, 0:1], 0.0)
                    scan_raw(view(ahz[:, :, :], [ahz.ap[0], [66, 32], [1, 64]], off=1),
                             lamview(ct, 32), ev4[:, ct, hr, :])
                    scan_raw(view(bhz[:, :, :], [bhz.ap[0], [66, 32], [1, 64]], off=1),
                             lamview(ct, 32), et4[:, ct, hr, :])
                    den = dnp.tile([128, 32, 64], F32, name="denh", tag="den")
                    nc.vector.scalar_tensor_tensor(
                        out=den, in0=et4[:, ct, hr, :],
                        scalar=col('eu', ct), in1=bhz[:, :, 0:64], op0=MM, op1=AD)
                    nc.vector.reciprocal_approx_fast(out=den, in_=den)
                    nc.vector.scalar_tensor_tensor(
                        out=ev4[:, ct, hr, :], in0=ev4[:, ct, hr, :],
                        scalar=col('eu', ct), in1=ahz[:, :, 0:64], op0=MM, op1=AD)
                    nc.gpsimd.tensor_tensor(out=ev4[:, ct, hr, :],
                                            in0=ev4[:, ct, hr, :], in1=den, op=MM)

            def c1_chunk(ch, wts, ytile, xtiles, g):
                h0 = ch * 8
                for mt in range(2):
                    ps = psmm.tile([128, CH], F32, name="c1_ps", tag="mm")
                    i = 0
                    for ti in range(9):
                        dy, dx = ti // 3 - 1, ti % 3 - 1
                        for kt in range(2):
                            nc.tensor.matmul(
                                out=ps.rearrange("p (a b) -> p a b", a=8),
                                lhsT=wts[mt][:, ti, kt, :],
                                rhs=xtiles[kt][:, 1 + h0 + dy:9 + h0 + dy,
                                               1 + dx:65 + dx],
                                start=(i == 0), stop=(i == 17))
                            i += 1
                    nc.scalar.activation(
                        out=ytile[:, mt, ch * CH:(ch + 1) * CH], in_=ps,
                        func=Act.Gelu, bias=col('c1b', 2 * g + mt))

            vscan_group(0)
            vscan_group(1)

            for ch in range(NCH):
                sl = slice(ch * CH, (ch + 1) * CH)
                nc.scalar.activation(out=sr[:, :, sl], in_=sr[:, :, sl],
                                     func=Act.Sigmoid)

            y1b = big.tile([128, 2, S], BF16, name="y1b", tag="Xb")
            c1w_g1 = []
            for mt in range(2):
                t = wstr.tile([128, 9, 2, 128], BF16, name=f"c1g1m{mt}", tag="wst9")
                nc.sync.dma_start(out=t, in_=d_c1[1, mt, :, :, :, :])
                c1w_g1.append(t)
            for ch in range(NCH // 2):
                c1_chunk(ch, c1w_g1, y1b, xcp_g1, 1)

            hscan_group(0)
            hscan_group(1)

            for ch in range(NCH // 2, NCH):
                c1_chunk(ch, c1w_g1, y1b, xcp_g1, 1)

            # wkv = out_h + out_v^T (0.5 factor dropped: LN-invariant)
            for ct in range(2):
                ovT = view(outv[:, :, :, :], [outv.ap[0], [1, 64], [64, 64]], off=ct * S)
                nc.vector.tensor_tensor(out=ev4[:, ct, :, :], in0=ev4[:, ct, :, :],
                                        in1=ovT, op=AD)
            if probe:
                pwkv = mkprobe("p_wkv", [C, S])
                for ct in range(2):
                    nc.gpsimd.dma_start(out=pwkv[128 * ct:128 * (ct + 1), :],
                                        in_=ev[:, ct, :])

            # key-LN stats while the rest of c1-skip runs on PE
            dstat_kn = dsc.tile([2, S], F32, name="dstat_kn", tag="dstat")
            drow_kn = dsc.tile([2, S], BF16, name="drow_kn", tag="drow")
            for ch in range(NCH):
                ln_stats_bounce(lambda ct: ev[:, ct, :], dstat_kn, ch, "kn")
            ln_math(dstat_kn, drow_kn, "kn")

            # ============ phase D: kn apply + srw + Wo + c1 main (lag 1) =====
            xcp_g0 = []
            for i in range(2):
                t = xcpp.tile([128, 66, 66], BF16, name=f"xcp{i}", tag="xcp")
                nc.gpsimd.memset(t[:, 0:1, :], 0.0)
                nc.gpsimd.memset(t[:, 65:66, :], 0.0)
                nc.gpsimd.memset(t[:, 1:65, 0:1], 0.0)
                nc.gpsimd.memset(t[:, 1:65, 65:66], 0.0)
                xcp_g0.append(t)
            c1w_g0 = []
            for mt in range(2):
                t = wstr.tile([128, 9, 2, 128], BF16, name=f"c1g0m{mt}", tag="wst9")
                nc.sync.dma_start(out=t, in_=d_c1[0, mt, :, :, :, :])
                c1w_g0.append(t)

            def wo_chunk(ch):
                sl = slice(ch * CH, (ch + 1) * CH)
                h0 = ch * 8
                nc.vector.tensor_tensor(out=sr[:, :, sl], in0=sr[:, :, sl],
                                        in1=ev[:, :, sl], op=MM)     # srw
                for mt in range(2):
                    ps = psmm.tile([128, CH], F32, name="wo_ps", tag="mm")
                    for kt in range(2):
                        nc.tensor.matmul(out=ps, lhsT=wot[:, kt, 128 * mt:128 * (mt + 1)],
                                         rhs=sr[:, kt, sl], start=(kt == 0), stop=(kt == 1))
                    nc.vector.tensor_tensor(
                        out=xcp_g0[mt][:, 1 + h0:9 + h0, 1:65],
                        in0=xn4[:, mt, h0:h0 + 8, :],
                        in1=ps.rearrange("p (a b) -> p a b", a=8), op=AD)

            y1a = big.tile([128, 2, S], BF16, name="y1a", tag="A")
            for ch in range(NCH):
                pr, pm = ln_bcast(drow_kn, ch, "kn")
                sl = slice(ch * CH, (ch + 1) * CH)
                for ct in range(2):
                    nc.vector.tensor_tensor(out=ev[:, ct, sl], in0=ev[:, ct, sl],
                                            in1=pr, op=MM)
                    nc.vector.tensor_tensor(out=ev[:, ct, sl], in0=ev[:, ct, sl],
                                            in1=pm, op=SU)
                if not kn_triv:
                    for ct in range(2):
                        nc.vector.tensor_scalar(
                            out=ev[:, ct, sl], in0=ev[:, ct, sl],
                            scalar1=col('knw', ct), scalar2=col('knb', ct),
                            op0=MM, op1=AD)
                wo_chunk(ch)
                if ch >= 2:
                    c1_chunk(ch - 2, c1w_g0, y1a, xcp_g0, 0)
            if probe:
                pxc = mkprobe("p_xcat", [CS, S])
                for i, t in enumerate(xcp_g0 + xcp_g1):
                    nc.gpsimd.dma_start(
                        out=pxc[128 * i:128 * (i + 1), :].rearrange("p (a b) -> p a b", a=64),
                        in_=t[:, 1:65, 1:65])
            c1_chunk(NCH - 2, c1w_g0, y1a, xcp_g0, 0)
            c1_chunk(NCH - 1, c1w_g0, y1a, xcp_g0, 0)
            y1t = [y1a, y1b]

            if probe:
                py1 = mkprobe("p_y1", [CS, S])
                for i in range(4):
                    nc.gpsimd.dma_start(out=py1[128 * i:128 * (i + 1), :],
                                        in_=y1t[i // 2][:, i % 2, :])

            # ============ phase F: c2/c3 with LN2 stats fused ============
            y3 = [big.tile([128, S], BF16, name="y3_0", tag="Cxn"),
                  big.tile([128, S], BF16, name="y3_1", tag="Mf")]
            c3wt = wstr.tile([128, 8, 256], BF16, name="c3wt", tag="wst9")
            nc.sync.dma_start(out=c3wt, in_=d_c3[:, :].rearrange("(kt p) m -> p kt m", p=128))
            dstat2 = dsc.tile([2, S], F32, name="dstat_ln2", tag="dstat")
            drow2 = dsc.tile([2, S], BF16, name="drow_ln2", tag="drow")
            for ch in range(NCH):
                sl = slice(ch * CH, (ch + 1) * CH)
                ytiles = []
                for mt in range(8):
                    ps = psmm.tile([128, CH], F32, name="c2_ps", tag="mm")
                    for kt in range(4):
                        nc.tensor.matmul(out=ps, lhsT=c2wt[:, kt, 128 * mt:128 * (mt + 1)],
                                         rhs=y1t[kt // 2][:, kt % 2, sl],
                                         start=(kt == 0), stop=(kt == 3))
                    yt = y2b.tile([128, CH], BF16, name="y2t", tag="y2t")
                    nc.scalar.activation(out=yt, in_=ps, func=Act.Gelu, bias=col('c2b', mt))
                    ytiles.append(yt)
                for mt in range(2):
                    ps = psmm.tile([128, CH], F32, name="c3_ps", tag="mm")
                    for kt in range(8):
                        nc.tensor.matmul(out=ps, lhsT=c3wt[:, kt, 128 * mt:128 * (mt + 1)],
                                         rhs=ytiles[kt], start=(kt == 0), stop=(kt == 7))
                    nc.scalar.activation(out=y3[mt][:, sl], in_=ps, func=Act.Gelu,
                                         bias=col('c3b', mt))
                    if not bn3_triv:
                        nc.vector.tensor_scalar(out=y3[mt][:, sl], in0=y3[mt][:, sl],
                                                scalar1=col('g3p', mt),
                                                scalar2=col('b3p', mt), op0=MM, op1=AD)
                ln_stats_bounce(lambda ct: y3[ct][:, :], dstat2, ch, "ln2")
            ln_math(dstat2, drow2, "ln2")

            if probe:
                py3 = mkprobe("p_y3", [C, S])
                for i in range(2):
                    nc.gpsimd.dma_start(out=py3[128 * i:128 * (i + 1), :],
                                        in_=y3[i][:, :])

            # ============ phase G: LN2 apply + up-proj + shuffle-out =========
            uptb = wstr.tile([128, 2, 512], BF16, name="uptb", tag="wst9")
            nc.sync.dma_start(out=uptb, in_=d_up[:, :].rearrange("(kt p) m -> p kt m", p=128))
            for ch in range(NCH):
                pr, pm = ln_bcast(drow2, ch, "ln2")
                sl = slice(ch * CH, (ch + 1) * CH)
                for ct in range(2):
                    nc.vector.tensor_tensor(out=y3[ct][:, sl], in0=y3[ct][:, sl],
                                            in1=pr, op=MM)
                    nc.vector.tensor_tensor(out=y3[ct][:, sl], in0=y3[ct][:, sl],
                                            in1=pm, op=SU)
                if not ln2_triv:
                    for ct in range(2):
                        nc.vector.tensor_scalar(
                            out=y3[ct][:, sl], in0=y3[ct][:, sl],
                            scalar1=col('ln2w', ct), scalar2=col('ln2b', ct),
                            op0=MM, op1=AD)
                h0 = ch * 8
                for r in range(2):
                    ub = wsk.tile([128, 8, 64, 2], F32, name="ub", tag="skc")
                    for q in range(2):
                        rq = 2 * r + q
                        ps = psmm.tile([128, CH], F32, name="up_ps", tag="mm")
                        for kt in range(2):
                            nc.tensor.matmul(out=ps,
                                             lhsT=uptb[:, kt, 128 * rq:128 * (rq + 1)],
                                             rhs=y3[kt][:, sl],
                                             start=(kt == 0), stop=(kt == 1))
                        nc.scalar.activation(out=ub[:, :, :, q],
                                             in_=ps.rearrange("p (a b) -> p a b", a=8),
                                             func=Act.Identity, bias=col('upb', rq))
                    dst = view(yout[:, :, :], [[128 * 128, 128], [256, 8], [1, 128]],
                               off=(2 * h0 + r) * 128)
                    nc.sync.dma_start(out=dst, in_=ub.rearrange("p a b q -> p a (b q)"))

    nc.compile()
    return nc, const_inputs


def _get_nc(weights, probe=False):
    import hashlib
    hsh = hashlib.sha1()
    for k in sorted(weights):
        hsh.update(k.encode())
        hsh.update(np.ascontiguousarray(weights[k]).tobytes())
    key = (hsh.hexdigest(), probe)
    if key not in _CACHE:
        _CACHE[key] = _build(weights, probe=probe)
    return _CACHE[key]


def kernel(**inputs):
    from concourse.bass_utils import run_bass_kernel_spmd

    x = np.asarray(inputs['x'], np.float32)
    skip = np.asarray(inputs['skip'], np.float32)
    mask = np.asarray(inputs['saliency_mask'], np.float32)
    weights = {k: np.asarray(v, np.float32) for k, v in inputs.items()
               if k not in ('x', 'skip', 'saliency_mask')}

    probe = bool(os.environ.get('BASSK_PROBE'))
    nc, const_inputs = _get_nc(weights, probe=probe)

    in_maps = []
    for b in range(B):
        m = dict(
            xin=np.ascontiguousarray(x[b].reshape(C, S)),
            skin=np.ascontiguousarray(skip[b].reshape(CS, S)),
            mrow=np.ascontiguousarray(mask[b].reshape(1, S)),
        )
        m.update(const_inputs)
        in_maps.append(m)
    res = run_bass_kernel_spmd(nc, in_maps, core_ids=list(range(B)),
                               trace=bool(os.environ.get('BASSK_TRACE')))
    kernel.last_results = res
    out = np.stack([res.results[b]['yout'] for b in range(B)], axis=0)
    return out


# revision 29
# speedup vs baseline: 1.3144x; 1.0437x over previous
"""Trainium2 Bass kernel for nn_DecoderBlock (shape-guided RWKV decoder block).

Data-parallel over batch: B=8 samples -> 8 NeuronCores, one NEFF.

v3: fully pipelined per-chunk structure for PE occupancy (HAM clock-gate)
and engine balance:
- all matmuls bf16 (FWL enabled, no fp32-HIGH power throttle);
- LN1 tail loop fuses broadcast/apply with per-chunk q_shift-diff (md) and
  the k/v projections + ev product, so the PE never waits on a serial md;
- WKV scans write near-contiguous zero-padded tiles; r-projection and the
  skip-group half of the 3x3 conv run on PE during the scan phase;
- key-LN tail loop fuses broadcast/apply/srw/Wo with the main-group 3x3
  conv at 1-chunk lag; c2/c3 loop fuses LN2 stats; LN2 tail fuses up-proj
  and output DMA;
- Pool (gpsimd) takes strided edge ops, scan-output multiplies, skip
  converts, and LN bounce DMA issue; DVE keeps scans/den/num/applies.
"""
import sys
import os

for _p in ('/opt/trn_rl_repo', '/root/.axon_site/_ro/trn_rl_repo'):
    if _p not in sys.path and os.path.isdir(_p):
        sys.path.append(_p)

import numpy as np

B, C, CS, COUT, H, W = 8, 256, 512, 128, 64, 64
S = H * W          # 4096
NCH = 8            # spatial chunks
CH = S // NCH      # 512
EPS = 1e-5

_CACHE = {}


def _build(weights, probe=False):
    const_inputs = {}
    import concourse.bass as bass
    from concourse import bacc
    import concourse.tile as tile
    import concourse.mybir as mybir
    import ml_dtypes

    F32 = mybir.dt.float32
    BF16 = mybir.dt.bfloat16
    Alu = mybir.AluOpType
    Act = mybir.ActivationFunctionType
    MM, AD, SU = Alu.mult, Alu.add, Alu.subtract

    w = weights
    f64 = lambda x: np.asarray(x, np.float64)
    bf = lambda a: np.asarray(a, dtype=ml_dtypes.bfloat16)

    # ---------------- host-side folding
    bnscale = 1.0 / np.sqrt(1.0 + EPS)
    g1p = f64(w['bn1_g']) * bnscale
    b1p = f64(w['bn1_b'])
    g2p = f64(w['bn2_g']) * bnscale
    b2p = f64(w['bn2_b'])
    g3p = (f64(w['bn3_g']) * bnscale).astype(np.float32)
    b3p = f64(w['bn3_b']).astype(np.float32)

    c2_eff = f64(w['c2_w']) * g1p[None, :]
    c2b_eff = (f64(w['c2_b']) + f64(w['c2_w']) @ b1p).astype(np.float32)
    c3_eff = f64(w['c3_w']) * g2p[None, :]
    c3b_eff = (f64(w['c3_b']) + f64(w['c3_w']) @ b2p).astype(np.float32)

    wk_x = f64(w['Wk']).T
    wk_d = (f64(w['Wk']) * (1.0 - f64(w['mix_k']))[None, :]).T
    wv_x = f64(w['Wv']).T
    wv_d = (f64(w['Wv']) * (1.0 - f64(w['mix_v']))[None, :]).T
    wr_x = f64(w['Wr']).T
    wr_d = (f64(w['Wr']) * (1.0 - f64(w['mix_r']))[None, :]).T
    # r gate evacuated as tanh(x/2); sigmoid(x) = 0.5*(tanh(x/2)+1), the
    # (t+1) is folded into srw and the 0.5 into Wo here.
    wo_t = f64(w['Wo']).T * 0.5
    sp_t = f64(w['sp_w']).T

    lam = np.exp(-np.exp(f64(w['decay']))).astype(np.float32)
    lam64 = np.tile(lam[:, None], (1, 64))
    lam64[:, 0] = 0.0
    lam64 = lam64.astype(np.float32)
    eu = np.exp(f64(w['first'])).astype(np.float32)

    pidx = np.arange(512)
    old = (pidx % 128) * 4 + (pidx // 128)
    up_t = f64(w['up_w'])[old].T                                # [256, 512]
    upb_p = f64(w['up_b'])[old].astype(np.float32)

    c1w = f64(w['c1_w'])
    c1_l = np.zeros((9, 2, 256, 256), np.float32)
    for ti in range(9):
        dy, dx = ti // 3, ti % 3
        for g in range(2):
            c1_l[ti, g] = c1w[g * 256:(g + 1) * 256, :, dy, dx].T

    # per-channel vectors as columns of one [128, ncol] const
    cols, order = {}, []

    def addcol(name, vec):
        v = np.asarray(vec, np.float32).reshape(-1, 128)
        cols[name] = v
        order.append(name)

    addcol('eu', eu)
    addcol('ln1w', w['ln1_w'])
    addcol('ln1b', w['ln1_b'])
    addcol('knw', w['kn_w'])
    addcol('knb', w['kn_b'])
    addcol('ln2w', w['ln2_w'])
    addcol('ln2b', w['ln2_b'])
    addcol('g3p', g3p)
    addcol('b3p', b3p)
    addcol('spb', w['sp_b'])
    addcol('c3b', c3b_eff)
    addcol('c1b', w['c1_b'])
    addcol('c2b', c2b_eff)
    addcol('upb', upb_p)
    colidx, ncol = {}, 0
    for n in order:
        colidx[n] = ncol
        ncol += cols[n].shape[0]
    cvec_np = np.zeros((128, ncol), np.float32)
    for n in order:
        for i in range(cols[n].shape[0]):
            cvec_np[:, colidx[n] + i] = cols[n][i]

    ln1_triv = np.all(w['ln1_w'] == 1.0) and np.all(w['ln1_b'] == 0.0)
    kn_triv = np.all(w['kn_w'] == 1.0) and np.all(w['kn_b'] == 0.0)
    ln2_triv = np.all(w['ln2_w'] == 1.0) and np.all(w['ln2_b'] == 0.0)
    bn3_triv = np.all(g3p == g3p[0]) and np.all(b3p == 0.0)
    # uniform bn3 scale commutes with LN2 -> drop it entirely when trivial

    # ---------------- bass module
    nc = bacc.Bacc("TRN2", target_bir_lowering=False, debug=False, name="decblk")

    xin = nc.dram_tensor("xin", [C, S], F32, kind="ExternalInput")
    skin = nc.dram_tensor("skin", [CS, S], F32, kind="ExternalInput")
    mrow = nc.dram_tensor("mrow", [1, S], F32, kind="ExternalInput")
    yout = nc.dram_tensor("yout", [COUT, 2 * H, 2 * W], F32, kind="ExternalOutput")
    probes = {}

    def mkprobe(name, shape):
        if probe:
            probes[name] = nc.dram_tensor(name, shape, F32, kind="ExternalOutput")
        return probes.get(name)

    def it(arr, name):
        arr = np.ascontiguousarray(arr)
        import ml_dtypes as _md
        dt_ = {np.dtype(np.float32): F32, np.dtype(_md.bfloat16): BF16}[arr.dtype]
        const_inputs[name] = arr
        return nc.dram_tensor(name, list(arr.shape), dt_, kind="ExternalInput")

    d_lam = it(lam64, "lam64")
    d_cvec = it(cvec_np, "cvec")
    d_wk = [it(bf(wk_x), "wkx"), it(bf(wk_d), "wkd")]
    d_wv = [it(bf(wv_x), "wvx"), it(bf(wv_d), "wvd")]
    d_wr = [it(bf(wr_x), "wrx"), it(bf(wr_d), "wrd")]
    d_wo = it(bf(wo_t), "wo")
    d_sp = it(bf(sp_t), "sp")
    d_up = it(bf(up_t), "up")
    c1_r = c1_l.reshape(9, 2, 2, 128, 2, 128).transpose(1, 4, 3, 0, 2, 5)
    d_c1 = it(bf(c1_r), "c1")   # [g, mt, p, t, kt, m]
    d_c2 = it(bf(c2_eff.T), "c2")
    d_c3 = it(bf(c3_eff.T), "c3")
    red_np = np.zeros((128, 2, 2), np.float32)
    red_np[:, 0, 0] = 1.0
    red_np[:, 1, 1] = 1.0
    d_redb = it(bf(red_np), "redb")
    bc2_np = np.zeros((2, 2, 128), np.float32)
    bc2_np[0, 0, :] = 1.0
    bc2_np[1, 1, :] = 1.0
    d_bc1 = it(bf(bc2_np), "bc2")
    d_eps = it(np.full((128, 1), EPS, np.float32), "epsc")

    def scan_raw(out, d0, d1):
        eng = nc.vector
        return eng.add_instruction(mybir.InstTensorScalarPtr(
            name=nc.get_next_instruction_name(),
            is_tensor_tensor_scan=True,
            is_scalar_tensor_tensor=True,
            op0=MM, op1=AD,
            ins=[eng.lower_ap(d0), eng.lower_ap_or_imm(0.0), eng.lower_ap(d1)],
            outs=[eng.lower_ap(out)],
        ))

    def view(ap, dims, off=0):
        return bass.AP(tensor=ap.tensor, offset=ap.offset + off, ap=dims)

    with tile.TileContext(nc) as tc:
        with tc.tile_pool(name="big", bufs=1) as big, \
             tc.tile_pool(name="dnp", bufs=2) as dnp, \
             tc.tile_pool(name="scn", bufs=3) as scn, \
             tc.tile_pool(name="wres", bufs=1) as wres, \
             tc.tile_pool(name="sml", bufs=3) as sml, \
             tc.tile_pool(name="y2b", bufs=8) as y2b, \
             tc.tile_pool(name="wstr", bufs=2) as wstr, \
             tc.tile_pool(name="wsk", bufs=2) as wsk, \
             tc.tile_pool(name="xcpp", bufs=2) as xcpp, \
             tc.tile_pool(name="dsc", bufs=2, space="DRAM") as dsc, \
             tc.tile_pool(name="psmm", bufs=3, space="PSUM") as psmm, \
             tc.tile_pool(name="psst", bufs=1, space="PSUM") as psst, \
             tc.tile_pool(name="psbc", bufs=2, space="PSUM") as psbc:

            # ---- resident constants
            lt = wres.tile([128, 2, 64], F32, name="lt")
            nc.sync.dma_start(out=lt, in_=d_lam[:, :].rearrange("(t p) j -> p t j", p=128))
            cv = wres.tile([128, ncol], F32, name="cv")
            nc.sync.dma_start(out=cv, in_=d_cvec[:, :])
            redb = wres.tile([128, 2, 2], BF16, name="redb")
            nc.sync.dma_start(out=redb, in_=d_redb[:, :, :])
            bc1 = wres.tile([2, 2, 128], BF16, name="bc1")
            nc.sync.dma_start(out=bc1, in_=d_bc1[:, :, :])
            epsc = wres.tile([128, 1], F32, name="epsc")
            nc.sync.dma_start(out=epsc, in_=d_eps[:, :])
            wot = wres.tile([128, 2, 256], BF16, name="wot")
            nc.sync.dma_start(out=wot, in_=d_wo[:, :].rearrange("(kt p) m -> p kt m", p=128))
            kvrw = []
            for nm, dws in (("wk", d_wk), ("wv", d_wv), ("wr", d_wr)):
                wxt = wres.tile([128, 2, 256], BF16, name=f"{nm}x")
                wdt = wres.tile([128, 2, 256], BF16, name=f"{nm}d")
                nc.sync.dma_start(out=wxt, in_=dws[0][:, :].rearrange("(kt p) m -> p kt m", p=128))
                nc.sync.dma_start(out=wdt, in_=dws[1][:, :].rearrange("(kt p) m -> p kt m", p=128))
                kvrw.append((wxt, wdt))
            c2wt = wres.tile([128, 4, 1024], BF16, name="c2wt")
            nc.sync.dma_start(out=c2wt, in_=d_c2[:, :].rearrange("(kt p) m -> p kt m", p=128))

            def col(name, i=0):
                return cv[:, colidx[name] + i:colidx[name] + i + 1]

            def ln_stats_chunk(Xr, ps, sl, name):
                nc.tensor.matmul(out=ps, lhsT=redb[:, 0, :], rhs=Xr(0)[:, sl],
                                 start=True, stop=False)
                nc.tensor.matmul(out=ps, lhsT=redb[:, 0, :], rhs=Xr(1)[:, sl],
                                 start=False, stop=False)
                for ct in range(2):
                    sq = sml.tile([128, CH], BF16, name=f"sq_{name}", tag="sqc", bufs=2)
                    nc.scalar.activation(out=sq, in_=Xr(ct)[:, sl], func=Act.Square)
                    nc.tensor.matmul(out=ps, lhsT=redb[:, 1, :], rhs=sq,
                                     start=False, stop=(ct == 1))

            def ln_stats_bounce(Xr, dstat, ch, name):
                sl = slice(ch * CH, (ch + 1) * CH)
                ps = psst.tile([2, CH], F32, name=f"lnps_{name}", tag="st")
                ln_stats_chunk(Xr, ps, sl, name)
                stc = sml.tile([2, CH], F32, name=f"stc_{name}", tag="stc", bufs=2)
                nc.scalar.copy(out=stc, in_=ps)
                nc.sync.dma_start(out=dstat[:, sl], in_=stc)

            def ln_math(dstat, drow, name):
                sm = sml.tile([128, 2, 32], F32, name=f"sm_{name}", tag="sm", bufs=2)
                nc.sync.dma_start(out=sm, in_=view(dstat[:, :], [[1, 128], [S, 2], [128, 32]]))
                nc.vector.tensor_scalar_mul(out=sm, in0=sm, scalar1=1.0 / C)
                t2 = sml.tile([128, 32], F32, name=f"t2_{name}", tag="t2", bufs=2)
                nc.vector.tensor_tensor(out=t2, in0=sm[:, 0, :], in1=sm[:, 0, :], op=MM)
                nc.vector.tensor_tensor(out=t2, in0=sm[:, 1, :], in1=t2, op=SU)
                nc.scalar.activation(out=t2, in_=t2, func=Act.Sqrt, bias=epsc)
                nc.vector.reciprocal(out=t2, in_=t2)                      # rstd
                smb = sml.tile([128, 2, 32], BF16, name=f"smb_{name}", tag="smb", bufs=2)
                nc.vector.tensor_copy(out=smb[:, 0, :], in_=t2)
                nc.vector.tensor_tensor(out=smb[:, 1, :], in0=sm[:, 0, :], in1=t2, op=MM)
                nc.sync.dma_start(out=view(drow[:, :], [[1, 128], [S, 2], [128, 32]]),
                                  in_=smb)

            def ln_bcast(drow, ch, name):
                sl = slice(ch * CH, (ch + 1) * CH)
                bcrc = sml.tile([2, CH], BF16, name=f"bcr_{name}", tag="bcrc", bufs=2)
                nc.scalar.dma_start(out=bcrc, in_=drow[:, sl])
                pr = psbc.tile([128, CH], F32, name=f"pr_{name}", tag="pr")
                pm = psbc.tile([128, CH], F32, name=f"pm_{name}", tag="pm")
                nc.tensor.matmul(out=pr, lhsT=bc1[:, 0, :], rhs=bcrc)
                nc.tensor.matmul(out=pm, lhsT=bc1[:, 1, :], rhs=bcrc)
                return pr, pm

            # ============ phase A: load x (pipelined halves), LN1, skip ======
            x0 = big.tile([128, S], F32, name="x0", tag="A")
            x1 = big.tile([128, S], F32, name="x1", tag="B")
            xb = big.tile([128, 2, S], BF16, name="xb", tag="Xb")
            HS = S // 2
            for hf in range(2):
                hsl = slice(hf * HS, (hf + 1) * HS)
                nc.sync.dma_start(out=x0[:, hsl], in_=xin[0:128, hsl])
                nc.sync.dma_start(out=x1[:, hsl], in_=xin[128:256, hsl])
                nc.vector.tensor_copy(out=xb[:, 0, hsl], in_=x0[:, hsl])
                nc.vector.tensor_copy(out=xb[:, 1, hsl], in_=x1[:, hsl])

            # mask: fp32 row -> bf16 (in [32,128] layout) -> DRAM -> broadcast
            m1 = sml.tile([32, 128], F32, name="m1", tag="m1", bufs=1)
            nc.sync.dma_start(out=m1, in_=view(mrow[:, :], [[128, 32], [1, 128]]))
            m1b = sml.tile([32, 128], BF16, name="m1b", tag="m1b", bufs=1)
            nc.vector.tensor_copy(out=m1b, in_=m1)
            dmask = dsc.tile([32, 128], BF16, name="dmask", tag="dmask", bufs=1)
            nc.sync.dma_start(out=dmask, in_=m1b)
            mfb = big.tile([128, S], BF16, name="mfb", tag="Mf")
            nc.sync.dma_start(out=mfb, in_=view(dmask[:, :], [[0, 128], [1, S]]))

            # xcp tiles for skip group written early; borders zeroed
            xcp_g1 = []
            for i in range(2):
                t = xcpp.tile([128, 66, 66], BF16, name=f"xcp{2 + i}", tag="xcp")
                nc.gpsimd.memset(t[:, 0:1, :], 0.0)
                nc.gpsimd.memset(t[:, 65:66, :], 0.0)
                nc.gpsimd.memset(t[:, 1:65, 0:1], 0.0)
                nc.gpsimd.memset(t[:, 1:65, 65:66], 0.0)
                xcp_g1.append(t)

            sptb = wstr.tile([128, 4, 256], BF16, name="sptb", tag="wst9")
            nc.sync.dma_start(out=sptb, in_=d_sp[:, :].rearrange("(kt p) m -> p kt m", p=128))

            xn = big.tile([128, 2, S], BF16, name="xn", tag="Cxn")
            xn4 = xn.rearrange("p t (h w) -> p t h w", h=H)

            # LN1 stats per chunk with skip-conv half-chunks interleaved
            dstat1 = dsc.tile([2, S], F32, name="dstat_ln1", tag="dstat")
            drow1 = dsc.tile([2, S], BF16, name="drow_ln1", tag="drow")
            HCH = CH // 2
            for ch in range(NCH):
                ln_stats_bounce(lambda ct: xb[:, ct, :], dstat1, ch, "ln1")
                for hh in range(2):
                    hsl = slice(ch * CH + hh * HCH, ch * CH + (hh + 1) * HCH)
                    h0 = ch * 8 + hh * 4
                    skc = wsk.tile([128, 4, HCH], F32, name="skc", tag="skc")
                    nc.sync.dma_start(
                        out=skc,
                        in_=skin[:, hsl].rearrange("(kt p) n -> p kt n", p=128))
                    skb = wsk.tile([128, 4, HCH], BF16, name="skb", tag="skb", bufs=1)
                    nc.gpsimd.tensor_copy(out=skb, in_=skc)
                    for mt in range(2):
                        psk = psmm.tile([128, HCH], F32, name="sp_ps", tag="mm")
                        for kt in range(4):
                            nc.tensor.matmul(out=psk,
                                             lhsT=sptb[:, kt, 128 * mt:128 * (mt + 1)],
                                             rhs=skb[:, kt, :],
                                             start=(kt == 0), stop=(kt == 3))
                        nc.scalar.activation(
                            out=xcp_g1[mt][:, 1 + h0:5 + h0, 1:65],
                            in_=psk.rearrange("p (a b) -> p a b", a=4),
                            func=Act.Identity, bias=col('spb', mt))
            ln_math(dstat1, drow1, "ln1")

            # ---- per-chunk q_shift diff (+mask) for rows [8ch, 8ch+8);
            # lives in a small ring tile, consumed by k/v/r of the same chunk
            def md_chunk(ch):
                r0 = ch * 8
                rs = slice(r0, r0 + 8)
                mdc = sml.tile([128, 2, 8, W], BF16, name="mdc", tag="mdc", bufs=2)
                # ct=0: w-shifts (chunk-local rows)
                nc.vector.tensor_tensor(out=mdc[0:64, 0, :, 1:],
                                        in0=xn4[0:64, 0, rs, 0:63],
                                        in1=xn4[0:64, 0, rs, 1:], op=SU)
                nc.gpsimd.tensor_scalar_mul(out=mdc[0:64, 0, :, 0:1],
                                            in0=xn4[0:64, 0, rs, 0:1], scalar1=-1.0)
                nc.vector.tensor_tensor(out=mdc[64:128, 0, :, 0:63],
                                        in0=xn4[64:128, 0, rs, 1:],
                                        in1=xn4[64:128, 0, rs, 0:63], op=SU)
                nc.gpsimd.tensor_scalar_mul(out=mdc[64:128, 0, :, 63:64],
                                            in0=xn4[64:128, 0, rs, 63:64], scalar1=-1.0)
                # ct=1: h-shifts (reads rows r0-1 .. r0+8)
                if ch == 0:
                    nc.gpsimd.tensor_scalar_mul(out=mdc[0:64, 1, 0:1, :],
                                                in0=xn4[0:64, 1, 0:1, :], scalar1=-1.0)
                    nc.vector.tensor_tensor(out=mdc[0:64, 1, 1:8, :],
                                            in0=xn4[0:64, 1, 0:7, :],
                                            in1=xn4[0:64, 1, 1:8, :], op=SU)
                else:
                    nc.vector.tensor_tensor(out=mdc[0:64, 1, :, :],
                                            in0=xn4[0:64, 1, r0 - 1:r0 + 7, :],
                                            in1=xn4[0:64, 1, rs, :], op=SU)
                if ch == NCH - 1:
                    nc.vector.tensor_tensor(out=mdc[64:128, 1, 0:7, :],
                                            in0=xn4[64:128, 1, 57:64, :],
                                            in1=xn4[64:128, 1, 56:63, :], op=SU)
                    nc.gpsimd.tensor_scalar_mul(out=mdc[64:128, 1, 7:8, :],
                                                in0=xn4[64:128, 1, 63:64, :], scalar1=-1.0)
                else:
                    nc.vector.tensor_tensor(out=mdc[64:128, 1, :, :],
                                            in0=xn4[64:128, 1, r0 + 1:r0 + 9, :],
                                            in1=xn4[64:128, 1, rs, :], op=SU)
                sl = slice(ch * CH, (ch + 1) * CH)
                mdr = mdc.rearrange("p t h w -> p t (h w)")
                for ct in range(2):
                    nc.vector.tensor_tensor(out=mdr[:, ct, :], in0=mdr[:, ct, :],
                                            in1=mfb[:, sl], op=MM)
                return mdr

            et = big.tile([128, 2, S], BF16, name="et", tag="A")
            vv = big.tile([128, 2, S], BF16, name="vv", tag="D")
            ev = big.tile([128, 2, S], BF16, name="ev", tag="B")
            sr = big.tile([128, 2, S], BF16, name="sr", tag="Fsr")

            def kv_chunk(widx, ch, mdr, evac):
                wxt, wdt = kvrw[widx]
                sl = slice(ch * CH, (ch + 1) * CH)
                for mt in range(2):
                    ps = psmm.tile([128, CH], F32, name="kv_ps", tag="mm")
                    for kt in range(2):
                        nc.tensor.matmul(out=ps, lhsT=wxt[:, kt, 128 * mt:128 * (mt + 1)],
                                         rhs=xn[:, kt, sl], start=(kt == 0), stop=False)
                    for kt in range(2):
                        nc.tensor.matmul(out=ps, lhsT=wdt[:, kt, 128 * mt:128 * (mt + 1)],
                                         rhs=mdr[:, kt, :], start=False, stop=(kt == 1))
                    evac(mt, sl, ps)

            def kve_chunk(ch):
                mdr = md_chunk(ch)
                kv_chunk(0, ch, mdr, lambda mt, sl, ps: nc.scalar.activation(
                    out=et[:, mt, sl], in_=ps, func=Act.Exp))
                kv_chunk(1, ch, mdr, lambda mt, sl, ps: nc.scalar.copy(
                    out=vv[:, mt, sl], in_=ps))
                # r gate: tanh(x/2) (tanh shares the exp ACT table; no reload)
                kv_chunk(2, ch, mdr, lambda mt, sl, ps: nc.scalar.activation(
                    out=sr[:, mt, sl], in_=ps, func=Act.Tanh, scale=0.5))
                sl = slice(ch * CH, (ch + 1) * CH)
                nc.vector.tensor_tensor(out=ev[:, :, sl], in0=et[:, :, sl],
                                        in1=vv[:, :, sl], op=MM)

            # LN1 tail: broadcast/apply per chunk, fused with md/k/v/ev at lag 1
            for ch in range(NCH):
                pr, pm = ln_bcast(drow1, ch, "ln1")
                sl = slice(ch * CH, (ch + 1) * CH)
                for ct in range(2):
                    nc.vector.tensor_tensor(out=xn[:, ct, sl], in0=xb[:, ct, sl],
                                            in1=pr, op=MM)
                    nc.vector.tensor_tensor(out=xn[:, ct, sl], in0=xn[:, ct, sl],
                                            in1=pm, op=SU)
                if not ln1_triv:
                    for ct in range(2):
                        nc.vector.tensor_scalar(
                            out=xn[:, ct, sl], in0=xn[:, ct, sl],
                            scalar1=col('ln1w', ct), scalar2=col('ln1b', ct),
                            op0=MM, op1=AD)
                if ch >= 1:
                    kve_chunk(ch - 1)
            kve_chunk(NCH - 1)
            if probe:
                pxn = mkprobe("p_xn", [C, S])
                for ct in range(2):
                    nc.gpsimd.dma_start(out=pxn[128 * ct:128 * (ct + 1), :],
                                        in_=xn[:, ct, :])

            # ============ phase C: WKV scans (DVE) vs r/c1-skip (PE) =========
            ev4 = ev.rearrange("p t (h w) -> p t h w", h=H)
            et4 = et.rearrange("p t (h w) -> p t h w", h=H)
            outv = big.tile([128, 2, W, H], BF16, name="outv", tag="D")  # w-major
            lt_ap = lt[:, :, :]

            def lamview(ct, nseq):
                return view(lt_ap, [lt_ap.ap[0], [0, nseq], [1, 64]], off=ct * 64)

            # vertical: scan along h per (w, ct); outputs w-major, zero-padded
            # leading h column so den/num read the h-1 shift without edge ops.
            def vscan_group(half):
                wr_ = slice(half * 32, (half + 1) * 32)
                for ct in range(2):
                    avh = scn.tile([128, 32, 65], BF16, name="avh", tag="scnt")
                    bvh = scn.tile([128, 32, 65], BF16, name="bvh", tag="scnt")
                    nc.gpsimd.memset(avh[:, :, 0:1], 0.0)
                    nc.gpsimd.memset(bvh[:, :, 0:1], 0.0)
                    dv_ev = view(ev[:, :, :], [ev.ap[0], [1, 32], [64, 64]],
                                 off=ct * S + half * 32)
                    dv_et = view(et[:, :, :], [et.ap[0], [1, 32], [64, 64]],
                                 off=ct * S + half * 32)
                    scan_raw(view(avh[:, :, :], [avh.ap[0], [65, 32], [1, 64]], off=1),
                             lamview(ct, 32), dv_ev)
                    scan_raw(view(bvh[:, :, :], [bvh.ap[0], [65, 32], [1, 64]], off=1),
                             lamview(ct, 32), dv_et)
                    den = dnp.tile([128, 32, 64], F32, name="denv", tag="den")
                    nc.vector.scalar_tensor_tensor(
                        out=den, in0=dv_et,
                        scalar=col('eu', ct), in1=bvh[:, :, 0:64], op0=MM, op1=AD)
                    nc.vector.reciprocal_approx_fast(out=den, in_=den)
                    nc.vector.scalar_tensor_tensor(
                        out=outv[:, ct, wr_, :], in0=dv_ev,
                        scalar=col('eu', ct), in1=avh[:, :, 0:64], op0=MM, op1=AD)
                    nc.gpsimd.tensor_tensor(out=outv[:, ct, wr_, :],
                                            in0=outv[:, ct, wr_, :], in1=den, op=MM)

            # horizontal: scan along w per (h, ct); num/out in place on ev
            def hscan_group(half):
                hr = slice(half * 32, (half + 1) * 32)
                for ct in range(2):
                    ahz = scn.tile([128, 32, 66], BF16, name="ahz", tag="scnt")
                    bhz = scn.tile([128, 32, 66], BF16, name="bhz", tag="scnt")
                    nc.gpsimd.memset(ahz[:, :, 0:1], 0.0)
                    nc.gpsimd.memset(bhz[:, :, 0:1], 0.0)
                    scan_raw(view(ahz[:, :, :], [ahz.ap[0], [66, 32], [1, 64]], off=1),
                             lamview(ct, 32), ev4[:, ct, hr, :])
                    scan_raw(view(bhz[:, :, :], [bhz.ap[0], [66, 32], [1, 64]], off=1),
                             lamview(ct, 32), et4[:, ct, hr, :])
                    den = dnp.tile([128, 32, 64], F32, name="denh", tag="den")
                    nc.vector.scalar_tensor_tensor(
                        out=den, in0=et4[:, ct, hr, :],
                        scalar=col('eu', ct), in1=bhz[:, :, 0:64], op0=MM, op1=AD)
                    nc.vector.reciprocal_approx_fast(out=den, in_=den)
                    nc.vector.scalar_tensor_tensor(
                        out=ev4[:, ct, hr, :], in0=ev4[:, ct, hr, :],
                        scalar=col('eu', ct), in1=ahz[:, :, 0:64], op0=MM, op1=AD)
                    nc.gpsimd.tensor_tensor(out=ev4[:, ct, hr, :],
                                            in0=ev4[:, ct, hr, :], in1=den, op=MM)

            def c1_chunk(ch, wts, ytile, xtiles, g):
                h0 = ch * 8
                for mt in range(2):
                    ps = psmm.tile([128, CH], F32, name="c1_ps", tag="mm")
                    i = 0
                    for ti in range(9):
                        dy, dx = ti // 3 - 1, ti % 3 - 1
                        for kt in range(2):
                            nc.tensor.matmul(
                                out=ps.rearrange("p (a b) -> p a b", a=8),
                                lhsT=wts[mt][:, ti, kt, :],
                                rhs=xtiles[kt][:, 1 + h0 + dy:9 + h0 + dy,
                                               1 + dx:65 + dx],
                                start=(i == 0), stop=(i == 17))
                            i += 1
                    nc.scalar.activation(
                        out=ytile[:, mt, ch * CH:(ch + 1) * CH], in_=ps,
                        func=Act.Gelu, bias=col('c1b', 2 * g + mt))

            vscan_group(0)
            vscan_group(1)

            y1b = big.tile([128, 2, S], BF16, name="y1b", tag="Xb")
            c1w_g1 = []
            for mt in range(2):
                t = wstr.tile([128, 9, 2, 128], BF16, name=f"c1g1m{mt}", tag="wst9")
                nc.sync.dma_start(out=t, in_=d_c1[1, mt, :, :, :, :])
                c1w_g1.append(t)
            for ch in range(NCH // 2):
                c1_chunk(ch, c1w_g1, y1b, xcp_g1, 1)

            hscan_group(0)
            hscan_group(1)

            for ch in range(NCH // 2, NCH):
                c1_chunk(ch, c1w_g1, y1b, xcp_g1, 1)

            # wkv = out_h + out_v^T (0.5 factor dropped: LN-invariant)
            for ct in range(2):
                ovT = view(outv[:, :, :, :], [outv.ap[0], [1, 64], [64, 64]], off=ct * S)
                nc.vector.tensor_tensor(out=ev4[:, ct, :, :], in0=ev4[:, ct, :, :],
                                        in1=ovT, op=AD)
            if probe:
                pwkv = mkprobe("p_wkv", [C, S])
                for ct in range(2):
                    nc.gpsimd.dma_start(out=pwkv[128 * ct:128 * (ct + 1), :],
                                        in_=ev[:, ct, :])

            # key-LN stats while the rest of c1-skip runs on PE
            dstat_kn = dsc.tile([2, S], F32, name="dstat_kn", tag="dstat")
            drow_kn = dsc.tile([2, S], BF16, name="drow_kn", tag="drow")
            for ch in range(NCH):
                ln_stats_bounce(lambda ct: ev[:, ct, :], dstat_kn, ch, "kn")
            ln_math(dstat_kn, drow_kn, "kn")

            # ============ phase D: kn apply + srw + Wo + c1 main (lag 1) =====
            xcp_g0 = []
            for i in range(2):
                t = xcpp.tile([128, 66, 66], BF16, name=f"xcp{i}", tag="xcp")
                nc.gpsimd.memset(t[:, 0:1, :], 0.0)
                nc.gpsimd.memset(t[:, 65:66, :], 0.0)
                nc.gpsimd.memset(t[:, 1:65, 0:1], 0.0)
                nc.gpsimd.memset(t[:, 1:65, 65:66], 0.0)
                xcp_g0.append(t)
            c1w_g0 = []
            for mt in range(2):
                t = wstr.tile([128, 9, 2, 128], BF16, name=f"c1g0m{mt}", tag="wst9")
                nc.sync.dma_start(out=t, in_=d_c1[0, mt, :, :, :, :])
                c1w_g0.append(t)

            def wo_chunk(ch):
                sl = slice(ch * CH, (ch + 1) * CH)
                h0 = ch * 8
                # srw = (tanh(r/2)+1) * wkv  (sigmoid affine, 0.5 in Wo)
                nc.vector.scalar_tensor_tensor(out=sr[:, :, sl], in0=sr[:, :, sl],
                                               scalar=1.0, in1=ev[:, :, sl],
                                               op0=AD, op1=MM)
                for mt in range(2):
                    ps = psmm.tile([128, CH], F32, name="wo_ps", tag="mm")
                    for kt in range(2):
                        nc.tensor.matmul(out=ps, lhsT=wot[:, kt, 128 * mt:128 * (mt + 1)],
                                         rhs=sr[:, kt, sl], start=(kt == 0), stop=(kt == 1))
                    nc.vector.tensor_tensor(
                        out=xcp_g0[mt][:, 1 + h0:9 + h0, 1:65],
                        in0=xn4[:, mt, h0:h0 + 8, :],
                        in1=ps.rearrange("p (a b) -> p a b", a=8), op=AD)

            y1a = big.tile([128, 2, S], BF16, name="y1a", tag="A")
            for ch in range(NCH):
                pr, pm = ln_bcast(drow_kn, ch, "kn")
                sl = slice(ch * CH, (ch + 1) * CH)
                for ct in range(2):
                    nc.vector.tensor_tensor(out=ev[:, ct, sl], in0=ev[:, ct, sl],
                                            in1=pr, op=MM)
                    nc.vector.tensor_tensor(out=ev[:, ct, sl], in0=ev[:, ct, sl],
                                            in1=pm, op=SU)
                if not kn_triv:
                    for ct in range(2):
                        nc.vector.tensor_scalar(
                            out=ev[:, ct, sl], in0=ev[:, ct, sl],
                            scalar1=col('knw', ct), scalar2=col('knb', ct),
                            op0=MM, op1=AD)
                wo_chunk(ch)
                if ch >= 1:
                    c1_chunk(ch - 1, c1w_g0, y1a, xcp_g0, 0)
            if probe:
                pxc = mkprobe("p_xcat", [CS, S])
                for i, t in enumerate(xcp_g0 + xcp_g1):
                    nc.gpsimd.dma_start(
                        out=pxc[128 * i:128 * (i + 1), :].rearrange("p (a b) -> p a b", a=64),
                        in_=t[:, 1:65, 1:65])
            c1_chunk(NCH - 1, c1w_g0, y1a, xcp_g0, 0)
            y1t = [y1a, y1b]

            if probe:
                py1 = mkprobe("p_y1", [CS, S])
                for i in range(4):
                    nc.gpsimd.dma_start(out=py1[128 * i:128 * (i + 1), :],
                                        in_=y1t[i // 2][:, i % 2, :])

            # ============ phase F: c2/c3 with LN2 stats fused ============
            y3 = [big.tile([128, S], BF16, name="y3_0", tag="Cxn"),
                  big.tile([128, S], BF16, name="y3_1", tag="Mf")]
            c3wt = wstr.tile([128, 8, 256], BF16, name="c3wt", tag="wst9")
            nc.sync.dma_start(out=c3wt, in_=d_c3[:, :].rearrange("(kt p) m -> p kt m", p=128))
            dstat2 = dsc.tile([2, S], F32, name="dstat_ln2", tag="dstat")
            drow2 = dsc.tile([2, S], BF16, name="drow_ln2", tag="drow")
            for ch in range(NCH):
                sl = slice(ch * CH, (ch + 1) * CH)
                ytiles = []
                for mt in range(8):
                    ps = psmm.tile([128, CH], F32, name="c2_ps", tag="mm")
                    for kt in range(4):
                        nc.tensor.matmul(out=ps, lhsT=c2wt[:, kt, 128 * mt:128 * (mt + 1)],
                                         rhs=y1t[kt // 2][:, kt % 2, sl],
                                         start=(kt == 0), stop=(kt == 3))
                    yt = y2b.tile([128, CH], BF16, name="y2t", tag="y2t")
                    nc.scalar.activation(out=yt, in_=ps, func=Act.Gelu, bias=col('c2b', mt))
                    ytiles.append(yt)
                for mt in range(2):
                    ps = psmm.tile([128, CH], F32, name="c3_ps", tag="mm")
                    for kt in range(8):
                        nc.tensor.matmul(out=ps, lhsT=c3wt[:, kt, 128 * mt:128 * (mt + 1)],
                                         rhs=ytiles[kt], start=(kt == 0), stop=(kt == 7))
                    nc.scalar.activation(out=y3[mt][:, sl], in_=ps, func=Act.Gelu,
                                         bias=col('c3b', mt))
                    if not bn3_triv:
                        nc.vector.tensor_scalar(out=y3[mt][:, sl], in0=y3[mt][:, sl],
                                                scalar1=col('g3p', mt),
                                                scalar2=col('b3p', mt), op0=MM, op1=AD)
                ln_stats_bounce(lambda ct: y3[ct][:, :], dstat2, ch, "ln2")
            ln_math(dstat2, drow2, "ln2")

            if probe:
                py3 = mkprobe("p_y3", [C, S])
                for i in range(2):
                    nc.gpsimd.dma_start(out=py3[128 * i:128 * (i + 1), :],
                                        in_=y3[i][:, :])

            # ============ phase G: LN2 apply + up-proj + shuffle-out =========
            uptb = wstr.tile([128, 2, 512], BF16, name="uptb", tag="wst9")
            nc.sync.dma_start(out=uptb, in_=d_up[:, :].rearrange("(kt p) m -> p kt m", p=128))
            for ch in range(NCH):
                pr, pm = ln_bcast(drow2, ch, "ln2")
                sl = slice(ch * CH, (ch + 1) * CH)
                for ct in range(2):
                    nc.vector.tensor_tensor(out=y3[ct][:, sl], in0=y3[ct][:, sl],
                                            in1=pr, op=MM)
                    nc.vector.tensor_tensor(out=y3[ct][:, sl], in0=y3[ct][:, sl],
                                            in1=pm, op=SU)
                if not ln2_triv:
                    for ct in range(2):
                        nc.vector.tensor_scalar(
                            out=y3[ct][:, sl], in0=y3[ct][:, sl],
                            scalar1=col('ln2w', ct), scalar2=col('ln2b', ct),
                            op0=MM, op1=AD)
                h0 = ch * 8
                for r in range(2):
                    ub = wsk.tile([128, 8, 64, 2], F32, name="ub", tag="skc")
                    for q in range(2):
                        rq = 2 * r + q
                        ps = psmm.tile([128, CH], F32, name="up_ps", tag="mm")
                        for kt in range(2):
                            nc.tensor.matmul(out=ps,
                                             lhsT=uptb[:, kt, 128 * rq:128 * (rq + 1)],
                                             rhs=y3[kt][:, sl],
                                             start=(kt == 0), stop=(kt == 1))
                        nc.scalar.activation(out=ub[:, :, :, q],
                                             in_=ps.rearrange("p (a b) -> p a b", a=8),
                                             func=Act.Identity, bias=col('upb', rq))
                    dst = view(yout[:, :, :], [[128 * 128, 128], [256, 8], [1, 128]],
                               off=(2 * h0 + r) * 128)
                    nc.sync.dma_start(out=dst, in_=ub.rearrange("p a b q -> p a (b q)"))

    nc.compile()
    return nc, const_inputs


def _get_nc(weights, probe=False):
    import hashlib
    hsh = hashlib.sha1()
    for k in sorted(weights):
        hsh.update(k.encode())
        hsh.update(np.ascontiguousarray(weights[k]).tobytes())
    key = (hsh.hexdigest(), probe)
    if key not in _CACHE:
        _CACHE[key] = _build(weights, probe=probe)
    return _CACHE[key]


def kernel(**inputs):
    from concourse.bass_utils import run_bass_kernel_spmd

    x = np.asarray(inputs['x'], np.float32)
    skip = np.asarray(inputs['skip'], np.float32)
    mask = np.asarray(inputs['saliency_mask'], np.float32)
    weights = {k: np.asarray(v, np.float32) for k, v in inputs.items()
               if k not in ('x', 'skip', 'saliency_mask')}

    probe = bool(os.environ.get('BASSK_PROBE'))
    nc, const_inputs = _get_nc(weights, probe=probe)

    in_maps = []
    for b in range(B):
        m = dict(
            xin=np.ascontiguousarray(x[b].reshape(C, S)),
            skin=np.ascontiguousarray(skip[b].reshape(CS, S)),
            mrow=np.ascontiguousarray(mask[b].reshape(1, S)),
        )
        m.update(const_inputs)
        in_maps.append(m)
    res = run_bass_kernel_spmd(nc, in_maps, core_ids=list(range(B)),
                               trace=bool(os.environ.get('BASSK_TRACE')))
    kernel.last_results = res
    out = np.stack([res.results[b]['yout'] for b in range(B)], axis=0)
    return out


# revision 42
# speedup vs baseline: 1.5429x; 1.1739x over previous
"""Trainium2 Bass kernel for nn_DecoderBlock (shape-guided RWKV decoder block).

Data-parallel over batch: B=8 samples -> 8 NeuronCores, one NEFF.

v3: fully pipelined per-chunk structure for PE occupancy (HAM clock-gate)
and engine balance:
- all matmuls bf16 (FWL enabled, no fp32-HIGH power throttle);
- LN1 tail loop fuses broadcast/apply with per-chunk q_shift-diff (md) and
  the k/v projections + ev product, so the PE never waits on a serial md;
- WKV scans write near-contiguous zero-padded tiles; r-projection and the
  skip-group half of the 3x3 conv run on PE during the scan phase;
- key-LN tail loop fuses broadcast/apply/srw/Wo with the main-group 3x3
  conv at 1-chunk lag; c2/c3 loop fuses LN2 stats; LN2 tail fuses up-proj
  and output DMA;
- Pool (gpsimd) takes strided edge ops, scan-output multiplies, skip
  converts, and LN bounce DMA issue; DVE keeps scans/den/num/applies.
"""
import sys
import os

for _p in ('/opt/trn_rl_repo', '/root/.axon_site/_ro/trn_rl_repo'):
    if _p not in sys.path and os.path.isdir(_p):
        sys.path.append(_p)

import numpy as np

B, C, CS, COUT, H, W = 8, 256, 512, 128, 64, 64
S = H * W          # 4096
NCH = 8            # spatial chunks
CH = S // NCH      # 512
EPS = 1e-5

_CACHE = {}


def _build(weights, probe=False):
    const_inputs = {}
    import concourse.bass as bass
    from concourse import bacc
    import concourse.tile as tile
    import concourse.mybir as mybir
    import ml_dtypes

    F32 = mybir.dt.float32
    BF16 = mybir.dt.bfloat16
    Alu = mybir.AluOpType
    Act = mybir.ActivationFunctionType
    MM, AD, SU = Alu.mult, Alu.add, Alu.subtract

    w = weights
    f64 = lambda x: np.asarray(x, np.float64)
    bf = lambda a: np.asarray(a, dtype=ml_dtypes.bfloat16)

    # ---------------- host-side folding
    bnscale = 1.0 / np.sqrt(1.0 + EPS)
    g1p = f64(w['bn1_g']) * bnscale
    b1p = f64(w['bn1_b'])
    g2p = f64(w['bn2_g']) * bnscale
    b2p = f64(w['bn2_b'])
    g3p = (f64(w['bn3_g']) * bnscale).astype(np.float32)
    b3p = f64(w['bn3_b']).astype(np.float32)

    c2_eff = f64(w['c2_w']) * g1p[None, :]
    c2b_eff = (f64(w['c2_b']) + f64(w['c2_w']) @ b1p).astype(np.float32)
    c3_eff = f64(w['c3_w']) * g2p[None, :]
    c3b_eff = (f64(w['c3_b']) + f64(w['c3_w']) @ b2p).astype(np.float32)

    wk_x = f64(w['Wk']).T
    wk_d = (f64(w['Wk']) * (1.0 - f64(w['mix_k']))[None, :]).T
    wv_x = f64(w['Wv']).T
    wv_d = (f64(w['Wv']) * (1.0 - f64(w['mix_v']))[None, :]).T
    wr_x = f64(w['Wr']).T
    wr_d = (f64(w['Wr']) * (1.0 - f64(w['mix_r']))[None, :]).T
    # r gate evacuated as tanh(x/2); sigmoid(x) = 0.5*(tanh(x/2)+1), the
    # (t+1) is folded into srw and the 0.5 into Wo here.
    wo_t = f64(w['Wo']).T * 0.5
    sp_t = f64(w['sp_w']).T

    lam = np.exp(-np.exp(f64(w['decay']))).astype(np.float32)
    lam64 = np.tile(lam[:, None], (1, 64))
    lam64[:, 0] = 0.0
    lam64 = lam64.astype(np.float32)
    eu = np.exp(f64(w['first'])).astype(np.float32)

    pidx = np.arange(512)
    old = (pidx % 128) * 4 + (pidx // 128)
    up_t = f64(w['up_w'])[old].T                                # [256, 512]
    upb_p = f64(w['up_b'])[old].astype(np.float32)

    c1w = f64(w['c1_w'])
    c1_l = np.zeros((9, 2, 256, 256), np.float32)
    for ti in range(9):
        dy, dx = ti // 3, ti % 3
        for g in range(2):
            c1_l[ti, g] = c1w[g * 256:(g + 1) * 256, :, dy, dx].T

    # per-channel vectors as columns of one [128, ncol] const
    cols, order = {}, []

    def addcol(name, vec):
        v = np.asarray(vec, np.float32).reshape(-1, 128)
        cols[name] = v
        order.append(name)

    addcol('eu', eu)
    addcol('ln1w', w['ln1_w'])
    addcol('ln1b', w['ln1_b'])
    addcol('knw', w['kn_w'])
    addcol('knb', w['kn_b'])
    addcol('ln2w', w['ln2_w'])
    addcol('ln2b', w['ln2_b'])
    addcol('g3p', g3p)
    addcol('b3p', b3p)
    addcol('spb', w['sp_b'])
    addcol('c3b', c3b_eff)
    addcol('c1b', w['c1_b'])
    addcol('c2b', c2b_eff)
    addcol('upb', upb_p)
    colidx, ncol = {}, 0
    for n in order:
        colidx[n] = ncol
        ncol += cols[n].shape[0]
    cvec_np = np.zeros((128, ncol), np.float32)
    for n in order:
        for i in range(cols[n].shape[0]):
            cvec_np[:, colidx[n] + i] = cols[n][i]

    ln1_triv = np.all(w['ln1_w'] == 1.0) and np.all(w['ln1_b'] == 0.0)
    kn_triv = np.all(w['kn_w'] == 1.0) and np.all(w['kn_b'] == 0.0)
    ln2_triv = np.all(w['ln2_w'] == 1.0) and np.all(w['ln2_b'] == 0.0)
    bn3_triv = np.all(g3p == g3p[0]) and np.all(b3p == 0.0)
    # uniform bn3 scale commutes with LN2 -> drop it entirely when trivial

    # ---------------- bass module
    nc = bacc.Bacc("TRN2", target_bir_lowering=False, debug=False, name="decblk")

    xin = nc.dram_tensor("xin", [C, S], F32, kind="ExternalInput")
    skin = nc.dram_tensor("skin", [CS, S], F32, kind="ExternalInput")
    mrow = nc.dram_tensor("mrow", [1, S], F32, kind="ExternalInput")
    yout = nc.dram_tensor("yout", [COUT, 2 * H, 2 * W], F32, kind="ExternalOutput")
    probes = {}

    def mkprobe(name, shape):
        if probe:
            probes[name] = nc.dram_tensor(name, shape, F32, kind="ExternalOutput")
        return probes.get(name)

    def it(arr, name):
        arr = np.ascontiguousarray(arr)
        import ml_dtypes as _md
        dt_ = {np.dtype(np.float32): F32, np.dtype(_md.bfloat16): BF16}[arr.dtype]
        const_inputs[name] = arr
        return nc.dram_tensor(name, list(arr.shape), dt_, kind="ExternalInput")

    d_lam = it(lam64, "lam64")
    d_cvec = it(cvec_np, "cvec")
    d_wk = [it(bf(wk_x), "wkx"), it(bf(wk_d), "wkd")]
    d_wv = [it(bf(wv_x), "wvx"), it(bf(wv_d), "wvd")]
    d_wr = [it(bf(wr_x), "wrx"), it(bf(wr_d), "wrd")]
    d_wo = it(bf(wo_t), "wo")
    d_sp = it(bf(sp_t), "sp")
    d_up = it(bf(up_t), "up")
    c1_r = c1_l.reshape(9, 2, 2, 128, 2, 128).transpose(1, 4, 3, 0, 2, 5)
    d_c1 = it(bf(c1_r), "c1")   # [g, mt, p, t, kt, m]
    d_c2 = it(bf(c2_eff.T), "c2")
    d_c3 = it(bf(c3_eff.T), "c3")
    red_np = np.zeros((128, 2, 2), np.float32)
    red_np[:, 0, 0] = 1.0
    red_np[:, 1, 1] = 1.0
    d_redb = it(bf(red_np), "redb")
    bc2_np = np.zeros((2, 2, 128), np.float32)
    bc2_np[0, 0, :] = 1.0
    bc2_np[1, 1, :] = 1.0
    d_bc1 = it(bf(bc2_np), "bc2")
    d_eps = it(np.full((128, 1), EPS, np.float32), "epsc")

    def scan_raw(out, d0, d1):
        eng = nc.vector
        return eng.add_instruction(mybir.InstTensorScalarPtr(
            name=nc.get_next_instruction_name(),
            is_tensor_tensor_scan=True,
            is_scalar_tensor_tensor=True,
            op0=MM, op1=AD,
            ins=[eng.lower_ap(d0), eng.lower_ap_or_imm(0.0), eng.lower_ap(d1)],
            outs=[eng.lower_ap(out)],
        ))

    def view(ap, dims, off=0):
        return bass.AP(tensor=ap.tensor, offset=ap.offset + off, ap=dims)

    with tile.TileContext(nc) as tc:
        with tc.tile_pool(name="big", bufs=1) as big, \
             tc.tile_pool(name="dnp", bufs=2) as dnp, \
             tc.tile_pool(name="scn", bufs=3) as scn, \
             tc.tile_pool(name="wres", bufs=1) as wres, \
             tc.tile_pool(name="sml", bufs=3) as sml, \
             tc.tile_pool(name="y2b", bufs=8) as y2b, \
             tc.tile_pool(name="wstr", bufs=2) as wstr, \
             tc.tile_pool(name="wsk", bufs=2) as wsk, \
             tc.tile_pool(name="xcpp", bufs=2) as xcpp, \
             tc.tile_pool(name="dsc", bufs=2, space="DRAM") as dsc, \
             tc.tile_pool(name="psmm", bufs=3, space="PSUM") as psmm, \
             tc.tile_pool(name="psst", bufs=1, space="PSUM") as psst, \
             tc.tile_pool(name="psbc", bufs=2, space="PSUM") as psbc:

            # ---- resident constants
            lt = wres.tile([128, 2, 64], F32, name="lt")
            nc.sync.dma_start(out=lt, in_=d_lam[:, :].rearrange("(t p) j -> p t j", p=128))
            cv = wres.tile([128, ncol], F32, name="cv")
            nc.sync.dma_start(out=cv, in_=d_cvec[:, :])
            redb = wres.tile([128, 2, 2], BF16, name="redb")
            nc.sync.dma_start(out=redb, in_=d_redb[:, :, :])
            bc1 = wres.tile([2, 2, 128], BF16, name="bc1")
            nc.sync.dma_start(out=bc1, in_=d_bc1[:, :, :])
            epsc = wres.tile([128, 1], F32, name="epsc")
            nc.sync.dma_start(out=epsc, in_=d_eps[:, :])
            wot = wres.tile([128, 2, 256], BF16, name="wot")
            nc.sync.dma_start(out=wot, in_=d_wo[:, :].rearrange("(kt p) m -> p kt m", p=128))
            kvrw = []
            for nm, dws in (("wk", d_wk), ("wv", d_wv), ("wr", d_wr)):
                wxt = wres.tile([128, 2, 256], BF16, name=f"{nm}x")
                wdt = wres.tile([128, 2, 256], BF16, name=f"{nm}d")
                nc.sync.dma_start(out=wxt, in_=dws[0][:, :].rearrange("(kt p) m -> p kt m", p=128))
                nc.sync.dma_start(out=wdt, in_=dws[1][:, :].rearrange("(kt p) m -> p kt m", p=128))
                kvrw.append((wxt, wdt))
            c2wt = wres.tile([128, 4, 1024], BF16, name="c2wt")
            nc.sync.dma_start(out=c2wt, in_=d_c2[:, :].rearrange("(kt p) m -> p kt m", p=128))

            def col(name, i=0):
                return cv[:, colidx[name] + i:colidx[name] + i + 1]

            def ln_stats_chunk(Xr, ps, sl, name):
                nc.tensor.matmul(out=ps, lhsT=redb[:, 0, :], rhs=Xr(0)[:, sl],
                                 start=True, stop=False)
                nc.tensor.matmul(out=ps, lhsT=redb[:, 0, :], rhs=Xr(1)[:, sl],
                                 start=False, stop=False)
                for ct in range(2):
                    sq = sml.tile([128, CH], BF16, name=f"sq_{name}", tag="sqc", bufs=2)
                    nc.scalar.activation(out=sq, in_=Xr(ct)[:, sl], func=Act.Square)
                    nc.tensor.matmul(out=ps, lhsT=redb[:, 1, :], rhs=sq,
                                     start=False, stop=(ct == 1))

            def ln_stats_bounce(Xr, dstat, ch, name):
                sl = slice(ch * CH, (ch + 1) * CH)
                ps = psst.tile([2, CH], F32, name=f"lnps_{name}", tag="st")
                ln_stats_chunk(Xr, ps, sl, name)
                stc = sml.tile([2, CH], BF16, name=f"stc_{name}", tag="stc", bufs=2)
                nc.scalar.copy(out=stc, in_=ps)
                nc.sync.dma_start(out=dstat[:, sl], in_=stc)

            def ln_math(dstat, drow, name):
                # [2, S] stats -> [32, 128] tiles (contiguous 512B-per-partition
                # DMA; the naive [[1,128],...] view scatters into 4-byte packets
                # costing ~35us per LN)
                mu = sml.tile([32, 128], F32, name=f"mu_{name}", tag="sm", bufs=2)
                sq = sml.tile([32, 128], BF16, name=f"sq_{name}", tag="sm2", bufs=2)
                nc.gpsimd.dma_start(out=mu, in_=view(dstat[:, :], [[128, 32], [1, 128]]))
                nc.sync.dma_start(out=sq, in_=view(dstat[:, :], [[128, 32], [1, 128]], off=S))
                nc.vector.tensor_scalar_mul(out=mu, in0=mu, scalar1=1.0 / C)
                t2 = sml.tile([32, 128], F32, name=f"t2_{name}", tag="t2", bufs=2)
                nc.vector.tensor_tensor(out=t2, in0=mu, in1=mu, op=MM)
                nc.vector.scalar_tensor_tensor(out=t2, in0=sq, scalar=1.0 / C,
                                               in1=t2, op0=MM, op1=SU)
                nc.scalar.activation(out=t2, in_=t2, func=Act.Sqrt, bias=epsc[0:32, :])
                nc.vector.reciprocal(out=t2, in_=t2)                      # rstd
                smb = sml.tile([32, 2, 128], BF16, name=f"smb_{name}", tag="smb", bufs=2)
                nc.vector.tensor_copy(out=smb[:, 0, :], in_=t2)
                nc.vector.tensor_tensor(out=smb[:, 1, :], in0=mu, in1=t2, op=MM)
                nc.sync.dma_start(out=view(drow[:, :], [[128, 32], [1, 128]]),
                                  in_=smb[:, 0, :])
                nc.sync.dma_start(out=view(drow[:, :], [[128, 32], [1, 128]], off=S),
                                  in_=smb[:, 1, :])

            def ln_bcast(drow, ch, name):
                sl = slice(ch * CH, (ch + 1) * CH)
                bcrc = sml.tile([2, CH], BF16, name=f"bcr_{name}", tag="bcrc", bufs=2)
                nc.scalar.dma_start(out=bcrc, in_=drow[:, sl])
                pr = psbc.tile([128, CH], F32, name=f"pr_{name}", tag="pr")
                pm = psbc.tile([128, CH], F32, name=f"pm_{name}", tag="pm")
                nc.tensor.matmul(out=pr, lhsT=bc1[:, 0, :], rhs=bcrc)
                nc.tensor.matmul(out=pm, lhsT=bc1[:, 1, :], rhs=bcrc)
                return pr, pm

            # ============ phase A: load x (casting DMAs), LN1, skip ======
            xb = big.tile([128, 2, S], BF16, name="xb", tag="Xb")
            HS = S // 2
            for hf in range(2):
                hsl = slice(hf * HS, (hf + 1) * HS)
                nc.gpsimd.dma_start(out=xb[:, 0, hsl], in_=xin[0:128, hsl])
                nc.gpsimd.dma_start(out=xb[:, 1, hsl], in_=xin[128:256, hsl])

            # mask: fp32 row -> bf16 (in [32,128] layout) -> DRAM -> broadcast
            m1b = sml.tile([32, 128], BF16, name="m1b", tag="m1b", bufs=1)
            nc.gpsimd.dma_start(out=m1b, in_=view(mrow[:, :], [[128, 32], [1, 128]]))
            dmask = dsc.tile([32, 128], BF16, name="dmask", tag="dmask", bufs=1)
            nc.sync.dma_start(out=dmask, in_=m1b)
            mfb = big.tile([128, S], BF16, name="mfb", tag="Mf")
            nc.sync.dma_start(out=mfb, in_=view(dmask[:, :], [[0, 128], [1, S]]))

            # xcp tiles for skip group written early; borders zeroed
            xcp_g1 = []
            for i in range(2):
                t = xcpp.tile([128, 66, 66], BF16, name=f"xcp{2 + i}", tag="xcp")
                nc.gpsimd.memset(t[:, 0:1, :], 0.0)
                nc.gpsimd.memset(t[:, 65:66, :], 0.0)
                nc.gpsimd.memset(t[:, 1:65, 0:1], 0.0)
                nc.gpsimd.memset(t[:, 1:65, 65:66], 0.0)
                xcp_g1.append(t)

            sptb = wstr.tile([128, 4, 256], BF16, name="sptb", tag="wst9")
            nc.sync.dma_start(out=sptb, in_=d_sp[:, :].rearrange("(kt p) m -> p kt m", p=128))

            xn = big.tile([128, 2, S], BF16, name="xn", tag="Cxn")
            xn4 = xn.rearrange("p t (h w) -> p t h w", h=H)

            # LN1 stats per chunk with skip-conv half-chunks interleaved
            dstat1 = dsc.tile([2, S], BF16, name="dstat_ln1", tag="dstat")
            drow1 = dsc.tile([2, S], BF16, name="drow_ln1", tag="drow")
            HCH = CH // 2
            for ch in range(NCH):
                ln_stats_bounce(lambda ct: xb[:, ct, :], dstat1, ch, "ln1")
                for hh in range(2):
                    hsl = slice(ch * CH + hh * HCH, ch * CH + (hh + 1) * HCH)
                    h0 = ch * 8 + hh * 4
                    skb = wsk.tile([128, 4, HCH], BF16, name="skb", tag="skb", bufs=2)
                    nc.gpsimd.dma_start(
                        out=skb,
                        in_=skin[:, hsl].rearrange("(kt p) n -> p kt n", p=128))
                    for mt in range(2):
                        psk = psmm.tile([128, HCH], F32, name="sp_ps", tag="mm")
                        for kt in range(4):
                            nc.tensor.matmul(out=psk,
                                             lhsT=sptb[:, kt, 128 * mt:128 * (mt + 1)],
                                             rhs=skb[:, kt, :],
                                             start=(kt == 0), stop=(kt == 3))
                        nc.scalar.activation(
                            out=xcp_g1[mt][:, 1 + h0:5 + h0, 1:65],
                            in_=psk.rearrange("p (a b) -> p a b", a=4),
                            func=Act.Identity, bias=col('spb', mt))
            ln_math(dstat1, drow1, "ln1")

            # ---- per-chunk q_shift diff (+mask) for rows [8ch, 8ch+8);
            # lives in a small ring tile, consumed by k/v/r of the same chunk
            def md_chunk(ch):
                r0 = ch * 8
                rs = slice(r0, r0 + 8)
                mdc = sml.tile([128, 2, 8, W], BF16, name="mdc", tag="mdc", bufs=2)
                # ct=0: w-shifts (chunk-local rows)
                nc.vector.tensor_tensor(out=mdc[0:64, 0, :, 1:],
                                        in0=xn4[0:64, 0, rs, 0:63],
                                        in1=xn4[0:64, 0, rs, 1:], op=SU)
                nc.gpsimd.tensor_scalar_mul(out=mdc[0:64, 0, :, 0:1],
                                            in0=xn4[0:64, 0, rs, 0:1], scalar1=-1.0)
                nc.vector.tensor_tensor(out=mdc[64:128, 0, :, 0:63],
                                        in0=xn4[64:128, 0, rs, 1:],
                                        in1=xn4[64:128, 0, rs, 0:63], op=SU)
                nc.gpsimd.tensor_scalar_mul(out=mdc[64:128, 0, :, 63:64],
                                            in0=xn4[64:128, 0, rs, 63:64], scalar1=-1.0)
                # ct=1: h-shifts (reads rows r0-1 .. r0+8)
                if ch == 0:
                    nc.gpsimd.tensor_scalar_mul(out=mdc[0:64, 1, 0:1, :],
                                                in0=xn4[0:64, 1, 0:1, :], scalar1=-1.0)
                    nc.vector.tensor_tensor(out=mdc[0:64, 1, 1:8, :],
                                            in0=xn4[0:64, 1, 0:7, :],
                                            in1=xn4[0:64, 1, 1:8, :], op=SU)
                else:
                    nc.vector.tensor_tensor(out=mdc[0:64, 1, :, :],
                                            in0=xn4[0:64, 1, r0 - 1:r0 + 7, :],
                                            in1=xn4[0:64, 1, rs, :], op=SU)
                if ch == NCH - 1:
                    nc.vector.tensor_tensor(out=mdc[64:128, 1, 0:7, :],
                                            in0=xn4[64:128, 1, 57:64, :],
                                            in1=xn4[64:128, 1, 56:63, :], op=SU)
                    nc.gpsimd.tensor_scalar_mul(out=mdc[64:128, 1, 7:8, :],
                                                in0=xn4[64:128, 1, 63:64, :], scalar1=-1.0)
                else:
                    nc.vector.tensor_tensor(out=mdc[64:128, 1, :, :],
                                            in0=xn4[64:128, 1, r0 + 1:r0 + 9, :],
                                            in1=xn4[64:128, 1, rs, :], op=SU)
                sl = slice(ch * CH, (ch + 1) * CH)
                mdr = mdc.rearrange("p t h w -> p t (h w)")
                for ct in range(2):
                    nc.vector.tensor_tensor(out=mdr[:, ct, :], in0=mdr[:, ct, :],
                                            in1=mfb[:, sl], op=MM)
                return mdr

            et = big.tile([128, 2, S], BF16, name="et", tag="A")
            vv = big.tile([128, 2, S], BF16, name="vv", tag="D")
            ev = big.tile([128, 2, S], BF16, name="ev", tag="B")
            sr = big.tile([128, 2, S], BF16, name="sr", tag="Fsr")

            def kv_chunk(widx, ch, mdr, evac):
                wxt, wdt = kvrw[widx]
                sl = slice(ch * CH, (ch + 1) * CH)
                for mt in range(2):
                    ps = psmm.tile([128, CH], F32, name="kv_ps", tag="mm")
                    for kt in range(2):
                        nc.tensor.matmul(out=ps, lhsT=wxt[:, kt, 128 * mt:128 * (mt + 1)],
                                         rhs=xn[:, kt, sl], start=(kt == 0), stop=False)
                    for kt in range(2):
                        nc.tensor.matmul(out=ps, lhsT=wdt[:, kt, 128 * mt:128 * (mt + 1)],
                                         rhs=mdr[:, kt, :], start=False, stop=(kt == 1))
                    evac(mt, sl, ps)

            def kve_chunk(ch):
                mdr = md_chunk(ch)
                kv_chunk(0, ch, mdr, lambda mt, sl, ps: nc.scalar.activation(
                    out=et[:, mt, sl], in_=ps, func=Act.Exp))
                kv_chunk(1, ch, mdr, lambda mt, sl, ps: nc.scalar.copy(
                    out=vv[:, mt, sl], in_=ps))
                # r gate: tanh(x/2) (tanh shares the exp ACT table; no reload)
                kv_chunk(2, ch, mdr, lambda mt, sl, ps: nc.scalar.activation(
                    out=sr[:, mt, sl], in_=ps, func=Act.Tanh, scale=0.5))
                sl = slice(ch * CH, (ch + 1) * CH)
                nc.vector.tensor_tensor(out=ev[:, :, sl], in0=et[:, :, sl],
                                        in1=vv[:, :, sl], op=MM)

            # LN1 tail: broadcast/apply per chunk, fused with md/k/v/ev at lag 1
            for ch in range(NCH):
                pr, pm = ln_bcast(drow1, ch, "ln1")
                sl = slice(ch * CH, (ch + 1) * CH)
                for ct in range(2):
                    nc.vector.tensor_tensor(out=xn[:, ct, sl], in0=xb[:, ct, sl],
                                            in1=pr, op=MM)
                    nc.vector.tensor_tensor(out=xn[:, ct, sl], in0=xn[:, ct, sl],
                                            in1=pm, op=SU)
                if not ln1_triv:
                    for ct in range(2):
                        nc.vector.tensor_scalar(
                            out=xn[:, ct, sl], in0=xn[:, ct, sl],
                            scalar1=col('ln1w', ct), scalar2=col('ln1b', ct),
                            op0=MM, op1=AD)
                if ch >= 1:
                    kve_chunk(ch - 1)
            kve_chunk(NCH - 1)
            if probe:
                pxn = mkprobe("p_xn", [C, S])
                for ct in range(2):
                    nc.gpsimd.dma_start(out=pxn[128 * ct:128 * (ct + 1), :],
                                        in_=xn[:, ct, :])

            # ============ phase C: WKV scans (DVE) vs r/c1-skip (PE) =========
            ev4 = ev.rearrange("p t (h w) -> p t h w", h=H)
            et4 = et.rearrange("p t (h w) -> p t h w", h=H)
            outv = big.tile([128, 2, W, H], BF16, name="outv", tag="D")  # w-major
            lt_ap = lt[:, :, :]

            def lamview(ct, nseq):
                return view(lt_ap, [lt_ap.ap[0], [0, nseq], [1, 64]], off=ct * 64)

            # vertical: scan along h per (w, ct); outputs w-major, zero-padded
            # leading h column so den/num read the h-1 shift without edge ops.
            def vscan_group(half):
                wr_ = slice(half * 32, (half + 1) * 32)
                for ct in range(2):
                    avh = scn.tile([128, 32, 65], BF16, name="avh", tag="scnt")
                    bvh = scn.tile([128, 32, 65], BF16, name="bvh", tag="scnt")
                    nc.gpsimd.memset(avh[:, :, 0:1], 0.0)
                    nc.gpsimd.memset(bvh[:, :, 0:1], 0.0)
                    dv_ev = view(ev[:, :, :], [ev.ap[0], [1, 32], [64, 64]],
                                 off=ct * S + half * 32)
                    dv_et = view(et[:, :, :], [et.ap[0], [1, 32], [64, 64]],
                                 off=ct * S + half * 32)
                    scan_raw(view(avh[:, :, :], [avh.ap[0], [65, 32], [1, 64]], off=1),
                             lamview(ct, 32), dv_ev)
                    scan_raw(view(bvh[:, :, :], [bvh.ap[0], [65, 32], [1, 64]], off=1),
                             lamview(ct, 32), dv_et)
                    den = dnp.tile([128, 32, 64], F32, name="denv", tag="den")
                    nc.vector.scalar_tensor_tensor(
                        out=den, in0=dv_et,
                        scalar=col('eu', ct), in1=bvh[:, :, 0:64], op0=MM, op1=AD)
                    nc.vector.reciprocal_approx_fast(out=den, in_=den)
                    rdb = scn.tile([128, 32, 64], BF16, name="rdbv", tag="scnt")
                    nc.scalar.copy(out=rdb, in_=den)
                    nc.vector.scalar_tensor_tensor(
                        out=outv[:, ct, wr_, :], in0=dv_ev,
                        scalar=col('eu', ct), in1=avh[:, :, 0:64], op0=MM, op1=AD)
                    nc.gpsimd.tensor_tensor(out=outv[:, ct, wr_, :],
                                            in0=outv[:, ct, wr_, :], in1=rdb, op=MM)

            # horizontal: scan along w per (h, ct); num/out in place on ev
            def hscan_group(half):
                hr = slice(half * 32, (half + 1) * 32)
                for ct in range(2):
                    ahz = scn.tile([128, 32, 66], BF16, name="ahz", tag="scnt")
                    bhz = scn.tile([128, 32, 66], BF16, name="bhz", tag="scnt")
                    nc.gpsimd.memset(ahz[:, :, 0:1], 0.0)
                    nc.gpsimd.memset(bhz[:, :, 0:1], 0.0)
                    scan_raw(view(ahz[:, :, :], [ahz.ap[0], [66, 32], [1, 64]], off=1),
                             lamview(ct, 32), ev4[:, ct, hr, :])
                    scan_raw(view(bhz[:, :, :], [bhz.ap[0], [66, 32], [1, 64]], off=1),
                             lamview(ct, 32), et4[:, ct, hr, :])
                    den = dnp.tile([128, 32, 64], F32, name="denh", tag="den")
                    nc.vector.scalar_tensor_tensor(
                        out=den, in0=et4[:, ct, hr, :],
                        scalar=col('eu', ct), in1=bhz[:, :, 0:64], op0=MM, op1=AD)
                    nc.vector.reciprocal_approx_fast(out=den, in_=den)
                    rdb = scn.tile([128, 32, 64], BF16, name="rdbh", tag="scnt")
                    nc.scalar.copy(out=rdb, in_=den)
                    nc.vector.scalar_tensor_tensor(
                        out=ev4[:, ct, hr, :], in0=ev4[:, ct, hr, :],
                        scalar=col('eu', ct), in1=ahz[:, :, 0:64], op0=MM, op1=AD)
                    nc.gpsimd.tensor_tensor(out=ev4[:, ct, hr, :],
                                            in0=ev4[:, ct, hr, :], in1=rdb, op=MM)

            def c1_chunk(ch, wts, ytile, xtiles, g):
                h0 = ch * 8
                for mt in range(2):
                    ps = psmm.tile([128, CH], F32, name="c1_ps", tag="mm")
                    i = 0
                    for ti in range(9):
                        dy, dx = ti // 3 - 1, ti % 3 - 1
                        for kt in range(2):
                            nc.tensor.matmul(
                                out=ps.rearrange("p (a b) -> p a b", a=8),
                                lhsT=wts[mt][:, ti, kt, :],
                                rhs=xtiles[kt][:, 1 + h0 + dy:9 + h0 + dy,
                                               1 + dx:65 + dx],
                                start=(i == 0), stop=(i == 17))
                            i += 1
                    nc.scalar.activation(
                        out=ytile[:, mt, ch * CH:(ch + 1) * CH], in_=ps,
                        func=Act.Gelu, bias=col('c1b', 2 * g + mt))

            vscan_group(0)
            vscan_group(1)

            y1b = big.tile([128, 2, S], BF16, name="y1b", tag="Xb")
            c1w_g1 = []
            for mt in range(2):
                t = wstr.tile([128, 9, 2, 128], BF16, name=f"c1g1m{mt}", tag="wst9")
                nc.sync.dma_start(out=t, in_=d_c1[1, mt, :, :, :, :])
                c1w_g1.append(t)
            for ch in range(NCH // 2):
                c1_chunk(ch, c1w_g1, y1b, xcp_g1, 1)

            hscan_group(0)
            hscan_group(1)

            for ch in range(NCH // 2, NCH):
                c1_chunk(ch, c1w_g1, y1b, xcp_g1, 1)

            # wkv = out_h + out_v^T per chunk, fused with key-LN stats
            # (0.5 factor dropped: LN-invariant)
            dstat_kn = dsc.tile([2, S], BF16, name="dstat_kn", tag="dstat")
            drow_kn = dsc.tile([2, S], BF16, name="drow_kn", tag="drow")
            for ch in range(NCH):
                h0 = ch * 8
                for ct in range(2):
                    ovT = view(outv[:, :, :, :], [outv.ap[0], [1, 8], [64, 64]],
                               off=ct * S + h0)
                    nc.vector.tensor_tensor(out=ev4[:, ct, h0:h0 + 8, :],
                                            in0=ev4[:, ct, h0:h0 + 8, :],
                                            in1=ovT, op=AD)
                ln_stats_bounce(lambda ct: ev[:, ct, :], dstat_kn, ch, "kn")
            ln_math(dstat_kn, drow_kn, "kn")
            if probe:
                pwkv = mkprobe("p_wkv", [C, S])
                for ct in range(2):
                    nc.gpsimd.dma_start(out=pwkv[128 * ct:128 * (ct + 1), :],
                                        in_=ev[:, ct, :])

            # ============ phase D: kn apply + srw + Wo + c1 main (lag 1) =====
            xcp_g0 = []
            for i in range(2):
                t = xcpp.tile([128, 66, 66], BF16, name=f"xcp{i}", tag="xcp")
                nc.gpsimd.memset(t[:, 0:1, :], 0.0)
                nc.gpsimd.memset(t[:, 65:66, :], 0.0)
                nc.gpsimd.memset(t[:, 1:65, 0:1], 0.0)
                nc.gpsimd.memset(t[:, 1:65, 65:66], 0.0)
                xcp_g0.append(t)
            c1w_g0 = []
            for mt in range(2):
                t = wstr.tile([128, 9, 2, 128], BF16, name=f"c1g0m{mt}", tag="wst9")
                nc.sync.dma_start(out=t, in_=d_c1[0, mt, :, :, :, :])
                c1w_g0.append(t)

            def wo_chunk(ch):
                sl = slice(ch * CH, (ch + 1) * CH)
                h0 = ch * 8
                # srw = (tanh(r/2)+1) * wkv  (sigmoid affine, 0.5 in Wo)
                nc.vector.scalar_tensor_tensor(out=sr[:, :, sl], in0=sr[:, :, sl],
                                               scalar=1.0, in1=ev[:, :, sl],
                                               op0=AD, op1=MM)
                for mt in range(2):
                    ps = psmm.tile([128, CH], F32, name="wo_ps", tag="mm")
                    for kt in range(2):
                        nc.tensor.matmul(out=ps, lhsT=wot[:, kt, 128 * mt:128 * (mt + 1)],
                                         rhs=sr[:, kt, sl], start=(kt == 0), stop=(kt == 1))
                    nc.vector.tensor_tensor(
                        out=xcp_g0[mt][:, 1 + h0:9 + h0, 1:65],
                        in0=xn4[:, mt, h0:h0 + 8, :],
                        in1=ps.rearrange("p (a b) -> p a b", a=8), op=AD)

            y1a = big.tile([128, 2, S], BF16, name="y1a", tag="A")
            for ch in range(NCH):
                pr, pm = ln_bcast(drow_kn, ch, "kn")
                sl = slice(ch * CH, (ch + 1) * CH)
                for ct in range(2):
                    nc.vector.tensor_tensor(out=ev[:, ct, sl], in0=ev[:, ct, sl],
                                            in1=pr, op=MM)
                    nc.vector.tensor_tensor(out=ev[:, ct, sl], in0=ev[:, ct, sl],
                                            in1=pm, op=SU)
                if not kn_triv:
                    for ct in range(2):
                        nc.vector.tensor_scalar(
                            out=ev[:, ct, sl], in0=ev[:, ct, sl],
                            scalar1=col('knw', ct), scalar2=col('knb', ct),
                            op0=MM, op1=AD)
                wo_chunk(ch)
                if ch >= 1:
                    c1_chunk(ch - 1, c1w_g0, y1a, xcp_g0, 0)
            if probe:
                pxc = mkprobe("p_xcat", [CS, S])
                for i, t in enumerate(xcp_g0 + xcp_g1):
                    nc.gpsimd.dma_start(
                        out=pxc[128 * i:128 * (i + 1), :].rearrange("p (a b) -> p a b", a=64),
                        in_=t[:, 1:65, 1:65])
            c1_chunk(NCH - 1, c1w_g0, y1a, xcp_g0, 0)
            y1t = [y1a, y1b]

            if probe:
                py1 = mkprobe("p_y1", [CS, S])
                for i in range(4):
                    nc.gpsimd.dma_start(out=py1[128 * i:128 * (i + 1), :],
                                        in_=y1t[i // 2][:, i % 2, :])

            # ============ phase F: c2/c3 with LN2 stats fused ============
            y3 = [big.tile([128, S], BF16, name="y3_0", tag="Cxn"),
                  big.tile([128, S], BF16, name="y3_1", tag="Mf")]
            c3wt = wstr.tile([128, 8, 256], BF16, name="c3wt", tag="wst9")
            nc.sync.dma_start(out=c3wt, in_=d_c3[:, :].rearrange("(kt p) m -> p kt m", p=128))
            dstat2 = dsc.tile([2, S], BF16, name="dstat_ln2", tag="dstat")
            drow2 = dsc.tile([2, S], BF16, name="drow_ln2", tag="drow")
            for ch in range(NCH):
                sl = slice(ch * CH, (ch + 1) * CH)
                ytiles = []
                for mt in range(8):
                    ps = psmm.tile([128, CH], F32, name="c2_ps", tag="mm")
                    for kt in range(4):
                        nc.tensor.matmul(out=ps, lhsT=c2wt[:, kt, 128 * mt:128 * (mt + 1)],
                                         rhs=y1t[kt // 2][:, kt % 2, sl],
                                         start=(kt == 0), stop=(kt == 3))
                    yt = y2b.tile([128, CH], BF16, name="y2t", tag="y2t")
                    nc.scalar.activation(out=yt, in_=ps, func=Act.Gelu, bias=col('c2b', mt))
                    ytiles.append(yt)
                for mt in range(2):
                    ps = psmm.tile([128, CH], F32, name="c3_ps", tag="mm")
                    for kt in range(8):
                        nc.tensor.matmul(out=ps, lhsT=c3wt[:, kt, 128 * mt:128 * (mt + 1)],
                                         rhs=ytiles[kt], start=(kt == 0), stop=(kt == 7))
                    nc.scalar.activation(out=y3[mt][:, sl], in_=ps, func=Act.Gelu,
                                         bias=col('c3b', mt))
                    if not bn3_triv:
                        nc.vector.tensor_scalar(out=y3[mt][:, sl], in0=y3[mt][:, sl],
                                                scalar1=col('g3p', mt),
                                                scalar2=col('b3p', mt), op0=MM, op1=AD)
                ln_stats_bounce(lambda ct: y3[ct][:, :], dstat2, ch, "ln2")
            ln_math(dstat2, drow2, "ln2")

            if probe:
                py3 = mkprobe("p_y3", [C, S])
                for i in range(2):
                    nc.gpsimd.dma_start(out=py3[128 * i:128 * (i + 1), :],
                                        in_=y3[i][:, :])

            # ============ phase G: LN2 apply + up-proj + shuffle-out =========
            uptb = wstr.tile([128, 2, 512], BF16, name="uptb", tag="wst9")
            nc.sync.dma_start(out=uptb, in_=d_up[:, :].rearrange("(kt p) m -> p kt m", p=128))
            for ch in range(NCH):
                pr, pm = ln_bcast(drow2, ch, "ln2")
                sl = slice(ch * CH, (ch + 1) * CH)
                for ct in range(2):
                    nc.vector.tensor_tensor(out=y3[ct][:, sl], in0=y3[ct][:, sl],
                                            in1=pr, op=MM)
                    nc.vector.tensor_tensor(out=y3[ct][:, sl], in0=y3[ct][:, sl],
                                            in1=pm, op=SU)
                if not ln2_triv:
                    for ct in range(2):
                        nc.vector.tensor_scalar(
                            out=y3[ct][:, sl], in0=y3[ct][:, sl],
                            scalar1=col('ln2w', ct), scalar2=col('ln2b', ct),
                            op0=MM, op1=AD)
                h0 = ch * 8
                for r in range(2):
                    ub = wsk.tile([128, 8, 64, 2], F32, name="ub", tag="skb")
                    for q in range(2):
                        rq = 2 * r + q
                        ps = psmm.tile([128, CH], F32, name="up_ps", tag="mm")
                        for kt in range(2):
                            nc.tensor.matmul(out=ps,
                                             lhsT=uptb[:, kt, 128 * rq:128 * (rq + 1)],
                                             rhs=y3[kt][:, sl],
                                             start=(kt == 0), stop=(kt == 1))
                        nc.scalar.activation(out=ub[:, :, :, q],
                                             in_=ps.rearrange("p (a b) -> p a b", a=8),
                                             func=Act.Identity, bias=col('upb', rq))
                    dst = view(yout[:, :, :], [[128 * 128, 128], [256, 8], [1, 128]],
                               off=(2 * h0 + r) * 128)
                    nc.sync.dma_start(out=dst, in_=ub.rearrange("p a b q -> p a (b q)"))

    nc.compile()
    return nc, const_inputs


def _get_nc(weights, probe=False):
    import hashlib
    hsh = hashlib.sha1()
    for k in sorted(weights):
        hsh.update(k.encode())
        hsh.update(np.ascontiguousarray(weights[k]).tobytes())
    key = (hsh.hexdigest(), probe)
    if key not in _CACHE:
        _CACHE[key] = _build(weights, probe=probe)
    return _CACHE[key]


def kernel(**inputs):
    from concourse.bass_utils import run_bass_kernel_spmd

    x = np.asarray(inputs['x'], np.float32)
    skip = np.asarray(inputs['skip'], np.float32)
    mask = np.asarray(inputs['saliency_mask'], np.float32)
    weights = {k: np.asarray(v, np.float32) for k, v in inputs.items()
               if k not in ('x', 'skip', 'saliency_mask')}

    probe = bool(os.environ.get('BASSK_PROBE'))
    nc, const_inputs = _get_nc(weights, probe=probe)

    in_maps = []
    for b in range(B):
        m = dict(
            xin=np.ascontiguousarray(x[b].reshape(C, S)),
            skin=np.ascontiguousarray(skip[b].reshape(CS, S)),
            mrow=np.ascontiguousarray(mask[b].reshape(1, S)),
        )
        m.update(const_inputs)
        in_maps.append(m)
    res = run_bass_kernel_spmd(nc, in_maps, core_ids=list(range(B)),
                               trace=bool(os.environ.get('BASSK_TRACE')))
    kernel.last_results = res
    out = np.stack([res.results[b]['yout'] for b in range(B)], axis=0)
    return out


# revision 44
# speedup vs baseline: 1.5469x; 1.0026x over previous
"""Trainium2 Bass kernel for nn_DecoderBlock (shape-guided RWKV decoder block).

Data-parallel over batch: B=8 samples -> 8 NeuronCores, one NEFF.

v3: fully pipelined per-chunk structure for PE occupancy (HAM clock-gate)
and engine balance:
- all matmuls bf16 (FWL enabled, no fp32-HIGH power throttle);
- LN1 tail loop fuses broadcast/apply with per-chunk q_shift-diff (md) and
  the k/v projections + ev product, so the PE never waits on a serial md;
- WKV scans write near-contiguous zero-padded tiles; r-projection and the
  skip-group half of the 3x3 conv run on PE during the scan phase;
- key-LN tail loop fuses broadcast/apply/srw/Wo with the main-group 3x3
  conv at 1-chunk lag; c2/c3 loop fuses LN2 stats; LN2 tail fuses up-proj
  and output DMA;
- Pool (gpsimd) takes strided edge ops, scan-output multiplies, skip
  converts, and LN bounce DMA issue; DVE keeps scans/den/num/applies.
"""
import sys
import os

for _p in ('/opt/trn_rl_repo', '/root/.axon_site/_ro/trn_rl_repo'):
    if _p not in sys.path and os.path.isdir(_p):
        sys.path.append(_p)

import numpy as np

B, C, CS, COUT, H, W = 8, 256, 512, 128, 64, 64
S = H * W          # 4096
NCH = 8            # spatial chunks
CH = S // NCH      # 512
EPS = 1e-5

_CACHE = {}


def _build(weights, probe=False):
    const_inputs = {}
    import concourse.bass as bass
    from concourse import bacc
    import concourse.tile as tile
    import concourse.mybir as mybir
    import ml_dtypes

    F32 = mybir.dt.float32
    BF16 = mybir.dt.bfloat16
    Alu = mybir.AluOpType
    Act = mybir.ActivationFunctionType
    MM, AD, SU = Alu.mult, Alu.add, Alu.subtract

    w = weights
    f64 = lambda x: np.asarray(x, np.float64)
    bf = lambda a: np.asarray(a, dtype=ml_dtypes.bfloat16)

    # ---------------- host-side folding
    bnscale = 1.0 / np.sqrt(1.0 + EPS)
    g1p = f64(w['bn1_g']) * bnscale
    b1p = f64(w['bn1_b'])
    g2p = f64(w['bn2_g']) * bnscale
    b2p = f64(w['bn2_b'])
    g3p = (f64(w['bn3_g']) * bnscale).astype(np.float32)
    b3p = f64(w['bn3_b']).astype(np.float32)

    c2_eff = f64(w['c2_w']) * g1p[None, :]
    c2b_eff = (f64(w['c2_b']) + f64(w['c2_w']) @ b1p).astype(np.float32)
    c3_eff = f64(w['c3_w']) * g2p[None, :]
    c3b_eff = (f64(w['c3_b']) + f64(w['c3_w']) @ b2p).astype(np.float32)

    wk_x = f64(w['Wk']).T
    wk_d = (f64(w['Wk']) * (1.0 - f64(w['mix_k']))[None, :]).T
    wv_x = f64(w['Wv']).T
    wv_d = (f64(w['Wv']) * (1.0 - f64(w['mix_v']))[None, :]).T
    wr_x = f64(w['Wr']).T
    wr_d = (f64(w['Wr']) * (1.0 - f64(w['mix_r']))[None, :]).T
    # r gate evacuated as tanh(x/2); sigmoid(x) = 0.5*(tanh(x/2)+1), the
    # (t+1) is folded into srw and the 0.5 into Wo here.
    wo_t = f64(w['Wo']).T * 0.5
    sp_t = f64(w['sp_w']).T

    lam = np.exp(-np.exp(f64(w['decay']))).astype(np.float32)
    lam64 = np.tile(lam[:, None], (1, 64))
    lam64[:, 0] = 0.0
    lam64 = lam64.astype(np.float32)
    eu = np.exp(f64(w['first'])).astype(np.float32)

    pidx = np.arange(512)
    old = (pidx % 128) * 4 + (pidx // 128)
    up_t = f64(w['up_w'])[old].T                                # [256, 512]
    upb_p = f64(w['up_b'])[old].astype(np.float32)

    c1w = f64(w['c1_w'])
    c1_l = np.zeros((9, 2, 256, 256), np.float32)
    for ti in range(9):
        dy, dx = ti // 3, ti % 3
        for g in range(2):
            c1_l[ti, g] = c1w[g * 256:(g + 1) * 256, :, dy, dx].T

    # per-channel vectors as columns of one [128, ncol] const
    cols, order = {}, []

    def addcol(name, vec):
        v = np.asarray(vec, np.float32).reshape(-1, 128)
        cols[name] = v
        order.append(name)

    addcol('eu', eu)
    addcol('ln1w', w['ln1_w'])
    addcol('ln1b', w['ln1_b'])
    addcol('knw', w['kn_w'])
    addcol('knb', w['kn_b'])
    addcol('ln2w', w['ln2_w'])
    addcol('ln2b', w['ln2_b'])
    addcol('g3p', g3p)
    addcol('b3p', b3p)
    addcol('spb', w['sp_b'])
    addcol('c3b', c3b_eff)
    addcol('c1b', w['c1_b'])
    addcol('c2b', c2b_eff)
    addcol('upb', upb_p)
    colidx, ncol = {}, 0
    for n in order:
        colidx[n] = ncol
        ncol += cols[n].shape[0]
    cvec_np = np.zeros((128, ncol), np.float32)
    for n in order:
        for i in range(cols[n].shape[0]):
            cvec_np[:, colidx[n] + i] = cols[n][i]

    ln1_triv = np.all(w['ln1_w'] == 1.0) and np.all(w['ln1_b'] == 0.0)
    kn_triv = np.all(w['kn_w'] == 1.0) and np.all(w['kn_b'] == 0.0)
    ln2_triv = np.all(w['ln2_w'] == 1.0) and np.all(w['ln2_b'] == 0.0)
    bn3_triv = np.all(g3p == g3p[0]) and np.all(b3p == 0.0)
    # uniform bn3 scale commutes with LN2 -> drop it entirely when trivial

    # ---------------- bass module
    nc = bacc.Bacc("TRN2", target_bir_lowering=False, debug=False, name="decblk")

    xin = nc.dram_tensor("xin", [C, S], F32, kind="ExternalInput")
    skin = nc.dram_tensor("skin", [CS, S], F32, kind="ExternalInput")
    mrow = nc.dram_tensor("mrow", [1, S], F32, kind="ExternalInput")
    yout = nc.dram_tensor("yout", [COUT, 2 * H, 2 * W], F32, kind="ExternalOutput")
    probes = {}

    def mkprobe(name, shape):
        if probe:
            probes[name] = nc.dram_tensor(name, shape, F32, kind="ExternalOutput")
        return probes.get(name)

    def it(arr, name):
        arr = np.ascontiguousarray(arr)
        import ml_dtypes as _md
        dt_ = {np.dtype(np.float32): F32, np.dtype(_md.bfloat16): BF16}[arr.dtype]
        const_inputs[name] = arr
        return nc.dram_tensor(name, list(arr.shape), dt_, kind="ExternalInput")

    d_lam = it(lam64, "lam64")
    d_cvec = it(cvec_np, "cvec")
    d_wk = [it(bf(wk_x), "wkx"), it(bf(wk_d), "wkd")]
    d_wv = [it(bf(wv_x), "wvx"), it(bf(wv_d), "wvd")]
    d_wr = [it(bf(wr_x), "wrx"), it(bf(wr_d), "wrd")]
    d_wo = it(bf(wo_t), "wo")
    d_sp = it(bf(sp_t), "sp")
    d_up = it(bf(up_t), "up")
    c1_r = c1_l.reshape(9, 2, 2, 128, 2, 128).transpose(1, 4, 3, 0, 2, 5)
    d_c1 = it(bf(c1_r), "c1")   # [g, mt, p, t, kt, m]
    d_c2 = it(bf(c2_eff.T), "c2")
    d_c3 = it(bf(c3_eff.T), "c3")
    red_np = np.zeros((128, 2, 2), np.float32)
    red_np[:, 0, 0] = 1.0
    red_np[:, 1, 1] = 1.0
    d_redb = it(bf(red_np), "redb")
    bc2_np = np.zeros((2, 2, 128), np.float32)
    bc2_np[0, 0, :] = 1.0
    bc2_np[1, 1, :] = 1.0
    d_bc1 = it(bf(bc2_np), "bc2")
    d_eps = it(np.full((128, 1), EPS, np.float32), "epsc")

    def scan_raw(out, d0, d1):
        eng = nc.vector
        return eng.add_instruction(mybir.InstTensorScalarPtr(
            name=nc.get_next_instruction_name(),
            is_tensor_tensor_scan=True,
            is_scalar_tensor_tensor=True,
            op0=MM, op1=AD,
            ins=[eng.lower_ap(d0), eng.lower_ap_or_imm(0.0), eng.lower_ap(d1)],
            outs=[eng.lower_ap(out)],
        ))

    def view(ap, dims, off=0):
        return bass.AP(tensor=ap.tensor, offset=ap.offset + off, ap=dims)

    with tile.TileContext(nc) as tc:
        with tc.tile_pool(name="big", bufs=1) as big, \
             tc.tile_pool(name="dnp", bufs=2) as dnp, \
             tc.tile_pool(name="scn", bufs=3) as scn, \
             tc.tile_pool(name="wres", bufs=1) as wres, \
             tc.tile_pool(name="sml", bufs=3) as sml, \
             tc.tile_pool(name="y2b", bufs=8) as y2b, \
             tc.tile_pool(name="wstr", bufs=2) as wstr, \
             tc.tile_pool(name="wsk", bufs=2) as wsk, \
             tc.tile_pool(name="xcpp", bufs=2) as xcpp, \
             tc.tile_pool(name="dsc", bufs=2, space="DRAM") as dsc, \
             tc.tile_pool(name="psmm", bufs=3, space="PSUM") as psmm, \
             tc.tile_pool(name="psst", bufs=1, space="PSUM") as psst, \
             tc.tile_pool(name="psbc", bufs=2, space="PSUM") as psbc:

            # ---- resident constants
            lt = wres.tile([128, 2, 64], F32, name="lt")
            nc.sync.dma_start(out=lt, in_=d_lam[:, :].rearrange("(t p) j -> p t j", p=128))
            cv = wres.tile([128, ncol], F32, name="cv")
            nc.sync.dma_start(out=cv, in_=d_cvec[:, :])
            redb = wres.tile([128, 2, 2], BF16, name="redb")
            nc.sync.dma_start(out=redb, in_=d_redb[:, :, :])
            bc1 = wres.tile([2, 2, 128], BF16, name="bc1")
            nc.sync.dma_start(out=bc1, in_=d_bc1[:, :, :])
            epsc = wres.tile([128, 1], F32, name="epsc")
            nc.sync.dma_start(out=epsc, in_=d_eps[:, :])
            wot = wres.tile([128, 2, 256], BF16, name="wot")
            nc.sync.dma_start(out=wot, in_=d_wo[:, :].rearrange("(kt p) m -> p kt m", p=128))
            kvrw = []
            for nm, dws in (("wk", d_wk), ("wv", d_wv), ("wr", d_wr)):
                wxt = wres.tile([128, 2, 256], BF16, name=f"{nm}x")
                wdt = wres.tile([128, 2, 256], BF16, name=f"{nm}d")
                nc.sync.dma_start(out=wxt, in_=dws[0][:, :].rearrange("(kt p) m -> p kt m", p=128))
                nc.sync.dma_start(out=wdt, in_=dws[1][:, :].rearrange("(kt p) m -> p kt m", p=128))
                kvrw.append((wxt, wdt))
            c2wt = wres.tile([128, 4, 1024], BF16, name="c2wt")
            nc.sync.dma_start(out=c2wt, in_=d_c2[:, :].rearrange("(kt p) m -> p kt m", p=128))

            def col(name, i=0):
                return cv[:, colidx[name] + i:colidx[name] + i + 1]

            def ln_stats_chunk(Xr, ps, sl, name):
                nc.tensor.matmul(out=ps, lhsT=redb[:, 0, :], rhs=Xr(0)[:, sl],
                                 start=True, stop=False)
                nc.tensor.matmul(out=ps, lhsT=redb[:, 0, :], rhs=Xr(1)[:, sl],
                                 start=False, stop=False)
                for ct in range(2):
                    sq = sml.tile([128, CH], BF16, name=f"sq_{name}", tag="sqc", bufs=2)
                    nc.scalar.activation(out=sq, in_=Xr(ct)[:, sl], func=Act.Square)
                    nc.tensor.matmul(out=ps, lhsT=redb[:, 1, :], rhs=sq,
                                     start=False, stop=(ct == 1))

            def ln_stats_bounce(Xr, dstat, ch, name):
                sl = slice(ch * CH, (ch + 1) * CH)
                ps = psst.tile([2, CH], F32, name=f"lnps_{name}", tag="st")
                ln_stats_chunk(Xr, ps, sl, name)
                stc = sml.tile([2, CH], BF16, name=f"stc_{name}", tag="stc", bufs=2)
                nc.scalar.copy(out=stc, in_=ps)
                nc.sync.dma_start(out=dstat[:, sl], in_=stc)

            def ln_math(dstat, drow, name):
                # [2, S] stats -> [32, 128] tiles (contiguous 512B-per-partition
                # DMA; the naive [[1,128],...] view scatters into 4-byte packets
                # costing ~35us per LN)
                mu = sml.tile([32, 128], F32, name=f"mu_{name}", tag="sm", bufs=2)
                sq = sml.tile([32, 128], BF16, name=f"sq_{name}", tag="sm2", bufs=2)
                nc.gpsimd.dma_start(out=mu, in_=view(dstat[:, :], [[128, 32], [1, 128]]))
                nc.sync.dma_start(out=sq, in_=view(dstat[:, :], [[128, 32], [1, 128]], off=S))
                nc.vector.tensor_scalar_mul(out=mu, in0=mu, scalar1=1.0 / C)
                t2 = sml.tile([32, 128], F32, name=f"t2_{name}", tag="t2", bufs=2)
                nc.vector.tensor_tensor(out=t2, in0=mu, in1=mu, op=MM)
                nc.vector.scalar_tensor_tensor(out=t2, in0=sq, scalar=1.0 / C,
                                               in1=t2, op0=MM, op1=SU)
                nc.scalar.activation(out=t2, in_=t2, func=Act.Sqrt, bias=epsc[0:32, :])
                nc.vector.reciprocal(out=t2, in_=t2)                      # rstd
                smb = sml.tile([32, 2, 128], BF16, name=f"smb_{name}", tag="smb", bufs=2)
                nc.vector.tensor_copy(out=smb[:, 0, :], in_=t2)
                nc.vector.tensor_tensor(out=smb[:, 1, :], in0=mu, in1=t2, op=MM)
                nc.sync.dma_start(out=view(drow[:, :], [[128, 32], [1, 128]]),
                                  in_=smb[:, 0, :])
                nc.sync.dma_start(out=view(drow[:, :], [[128, 32], [1, 128]], off=S),
                                  in_=smb[:, 1, :])

            def ln_bcast(drow, ch, name):
                sl = slice(ch * CH, (ch + 1) * CH)
                bcrc = sml.tile([2, CH], BF16, name=f"bcr_{name}", tag="bcrc", bufs=2)
                nc.scalar.dma_start(out=bcrc, in_=drow[:, sl])
                pr = psbc.tile([128, CH], F32, name=f"pr_{name}", tag="pr")
                pm = psbc.tile([128, CH], F32, name=f"pm_{name}", tag="pm")
                nc.tensor.matmul(out=pr, lhsT=bc1[:, 0, :], rhs=bcrc)
                nc.tensor.matmul(out=pm, lhsT=bc1[:, 1, :], rhs=bcrc)
                return pr, pm

            # ============ phase A: load x (casting DMAs), LN1, skip ======
            xb = big.tile([128, 2, S], BF16, name="xb", tag="Xb")
            HS = S // 2
            for hf in range(2):
                hsl = slice(hf * HS, (hf + 1) * HS)
                nc.gpsimd.dma_start(out=xb[:, 0, hsl], in_=xin[0:128, hsl])
                nc.gpsimd.dma_start(out=xb[:, 1, hsl], in_=xin[128:256, hsl])

            # mask: fp32 row -> bf16 (in [32,128] layout) -> DRAM -> broadcast
            m1b = sml.tile([32, 128], BF16, name="m1b", tag="m1b", bufs=1)
            nc.gpsimd.dma_start(out=m1b, in_=view(mrow[:, :], [[128, 32], [1, 128]]))
            dmask = dsc.tile([32, 128], BF16, name="dmask", tag="dmask", bufs=1)
            nc.sync.dma_start(out=dmask, in_=m1b)
            mfb = big.tile([128, S], BF16, name="mfb", tag="Mf")
            nc.sync.dma_start(out=mfb, in_=view(dmask[:, :], [[0, 128], [1, S]]))

            # xcp tiles for skip group written early; borders zeroed
            xcp_g1 = []
            for i in range(2):
                t = xcpp.tile([128, 66, 66], BF16, name=f"xcp{2 + i}", tag="xcp")
                nc.gpsimd.memset(t[:, 0:1, :], 0.0)
                nc.gpsimd.memset(t[:, 65:66, :], 0.0)
                nc.gpsimd.memset(t[:, 1:65, 0:1], 0.0)
                nc.gpsimd.memset(t[:, 1:65, 65:66], 0.0)
                xcp_g1.append(t)

            sptb = wstr.tile([128, 4, 256], BF16, name="sptb", tag="wst9")
            nc.sync.dma_start(out=sptb, in_=d_sp[:, :].rearrange("(kt p) m -> p kt m", p=128))

            xn = big.tile([128, 2, S], BF16, name="xn", tag="Cxn")
            xn4 = xn.rearrange("p t (h w) -> p t h w", h=H)

            # LN1 stats per chunk with skip-conv half-chunks interleaved
            dstat1 = dsc.tile([2, S], BF16, name="dstat_ln1", tag="dstat")
            drow1 = dsc.tile([2, S], BF16, name="drow_ln1", tag="drow")
            HCH = CH // 2

            def skip_half(ch, hh):
                hsl = slice(ch * CH + hh * HCH, ch * CH + (hh + 1) * HCH)
                h0 = ch * 8 + hh * 4
                skb = wsk.tile([128, 4, HCH], BF16, name="skb", tag="skb", bufs=2)
                nc.gpsimd.dma_start(
                    out=skb,
                    in_=skin[:, hsl].rearrange("(kt p) n -> p kt n", p=128))
                for mt in range(2):
                    psk = psmm.tile([128, HCH], F32, name="sp_ps", tag="mm")
                    for kt in range(4):
                        nc.tensor.matmul(out=psk,
                                         lhsT=sptb[:, kt, 128 * mt:128 * (mt + 1)],
                                         rhs=skb[:, kt, :],
                                         start=(kt == 0), stop=(kt == 3))
                    nc.scalar.activation(
                        out=xcp_g1[mt][:, 1 + h0:5 + h0, 1:65],
                        in_=psk.rearrange("p (a b) -> p a b", a=4),
                        func=Act.Identity, bias=col('spb', mt))

            for ch in range(NCH):
                ln_stats_bounce(lambda ct: xb[:, ct, :], dstat1, ch, "ln1")
                if ch < 6:
                    skip_half(ch, 0)
                    skip_half(ch, 1)
            ln_math(dstat1, drow1, "ln1")

            # ---- per-chunk q_shift diff (+mask) for rows [8ch, 8ch+8);
            # lives in a small ring tile, consumed by k/v/r of the same chunk
            def md_chunk(ch):
                r0 = ch * 8
                rs = slice(r0, r0 + 8)
                mdc = sml.tile([128, 2, 8, W], BF16, name="mdc", tag="mdc", bufs=2)
                # ct=0: w-shifts (chunk-local rows)
                nc.vector.tensor_tensor(out=mdc[0:64, 0, :, 1:],
                                        in0=xn4[0:64, 0, rs, 0:63],
                                        in1=xn4[0:64, 0, rs, 1:], op=SU)
                nc.gpsimd.tensor_scalar_mul(out=mdc[0:64, 0, :, 0:1],
                                            in0=xn4[0:64, 0, rs, 0:1], scalar1=-1.0)
                nc.vector.tensor_tensor(out=mdc[64:128, 0, :, 0:63],
                                        in0=xn4[64:128, 0, rs, 1:],
                                        in1=xn4[64:128, 0, rs, 0:63], op=SU)
                nc.gpsimd.tensor_scalar_mul(out=mdc[64:128, 0, :, 63:64],
                                            in0=xn4[64:128, 0, rs, 63:64], scalar1=-1.0)
                # ct=1: h-shifts (reads rows r0-1 .. r0+8)
                if ch == 0:
                    nc.gpsimd.tensor_scalar_mul(out=mdc[0:64, 1, 0:1, :],
                                                in0=xn4[0:64, 1, 0:1, :], scalar1=-1.0)
                    nc.vector.tensor_tensor(out=mdc[0:64, 1, 1:8, :],
                                            in0=xn4[0:64, 1, 0:7, :],
                                            in1=xn4[0:64, 1, 1:8, :], op=SU)
                else:
                    nc.vector.tensor_tensor(out=mdc[0:64, 1, :, :],
                                            in0=xn4[0:64, 1, r0 - 1:r0 + 7, :],
                                            in1=xn4[0:64, 1, rs, :], op=SU)
                if ch == NCH - 1:
                    nc.vector.tensor_tensor(out=mdc[64:128, 1, 0:7, :],
                                            in0=xn4[64:128, 1, 57:64, :],
                                            in1=xn4[64:128, 1, 56:63, :], op=SU)
                    nc.gpsimd.tensor_scalar_mul(out=mdc[64:128, 1, 7:8, :],
                                                in0=xn4[64:128, 1, 63:64, :], scalar1=-1.0)
                else:
                    nc.vector.tensor_tensor(out=mdc[64:128, 1, :, :],
                                            in0=xn4[64:128, 1, r0 + 1:r0 + 9, :],
                                            in1=xn4[64:128, 1, rs, :], op=SU)
                sl = slice(ch * CH, (ch + 1) * CH)
                mdr = mdc.rearrange("p t h w -> p t (h w)")
                for ct in range(2):
                    nc.vector.tensor_tensor(out=mdr[:, ct, :], in0=mdr[:, ct, :],
                                            in1=mfb[:, sl], op=MM)
                return mdr

            et = big.tile([128, 2, S], BF16, name="et", tag="A")
            vv = big.tile([128, 2, S], BF16, name="vv", tag="D")
            ev = big.tile([128, 2, S], BF16, name="ev", tag="B")
            sr = big.tile([128, 2, S], BF16, name="sr", tag="Fsr")

            def kv_chunk(widx, ch, mdr, evac):
                wxt, wdt = kvrw[widx]
                sl = slice(ch * CH, (ch + 1) * CH)
                for mt in range(2):
                    ps = psmm.tile([128, CH], F32, name="kv_ps", tag="mm")
                    for kt in range(2):
                        nc.tensor.matmul(out=ps, lhsT=wxt[:, kt, 128 * mt:128 * (mt + 1)],
                                         rhs=xn[:, kt, sl], start=(kt == 0), stop=False)
                    for kt in range(2):
                        nc.tensor.matmul(out=ps, lhsT=wdt[:, kt, 128 * mt:128 * (mt + 1)],
                                         rhs=mdr[:, kt, :], start=False, stop=(kt == 1))
                    evac(mt, sl, ps)

            def kve_chunk(ch):
                mdr = md_chunk(ch)
                kv_chunk(0, ch, mdr, lambda mt, sl, ps: nc.scalar.activation(
                    out=et[:, mt, sl], in_=ps, func=Act.Exp))
                kv_chunk(1, ch, mdr, lambda mt, sl, ps: nc.scalar.copy(
                    out=vv[:, mt, sl], in_=ps))
                # r gate: tanh(x/2) (tanh shares the exp ACT table; no reload)
                kv_chunk(2, ch, mdr, lambda mt, sl, ps: nc.scalar.activation(
                    out=sr[:, mt, sl], in_=ps, func=Act.Tanh, scale=0.5))
                sl = slice(ch * CH, (ch + 1) * CH)
                nc.vector.tensor_tensor(out=ev[:, :, sl], in0=et[:, :, sl],
                                        in1=vv[:, :, sl], op=MM)

            # LN1 tail: broadcast/apply per chunk, fused with md/k/v/ev at lag 1;
            # remaining skip-conv chunks fill the PE during the pipeline ramp
            for ch in range(NCH):
                if ch < 4:
                    skip_half(6 + ch // 2, ch % 2)
                pr, pm = ln_bcast(drow1, ch, "ln1")
                sl = slice(ch * CH, (ch + 1) * CH)
                for ct in range(2):
                    nc.vector.tensor_tensor(out=xn[:, ct, sl], in0=xb[:, ct, sl],
                                            in1=pr, op=MM)
                    nc.vector.tensor_tensor(out=xn[:, ct, sl], in0=xn[:, ct, sl],
                                            in1=pm, op=SU)
                if not ln1_triv:
                    for ct in range(2):
                        nc.vector.tensor_scalar(
                            out=xn[:, ct, sl], in0=xn[:, ct, sl],
                            scalar1=col('ln1w', ct), scalar2=col('ln1b', ct),
                            op0=MM, op1=AD)
                if ch >= 1:
                    kve_chunk(ch - 1)
            kve_chunk(NCH - 1)
            if probe:
                pxn = mkprobe("p_xn", [C, S])
                for ct in range(2):
                    nc.gpsimd.dma_start(out=pxn[128 * ct:128 * (ct + 1), :],
                                        in_=xn[:, ct, :])

            # ============ phase C: WKV scans (DVE) vs r/c1-skip (PE) =========
            ev4 = ev.rearrange("p t (h w) -> p t h w", h=H)
            et4 = et.rearrange("p t (h w) -> p t h w", h=H)
            outv = big.tile([128, 2, W, H], BF16, name="outv", tag="D")  # w-major
            lt_ap = lt[:, :, :]

            def lamview(ct, nseq):
                return view(lt_ap, [lt_ap.ap[0], [0, nseq], [1, 64]], off=ct * 64)

            # vertical: scan along h per (w, ct); outputs w-major, zero-padded
            # leading h column so den/num read the h-1 shift without edge ops.
            def vscan_group(half):
                wr_ = slice(half * 32, (half + 1) * 32)
                for ct in range(2):
                    avh = scn.tile([128, 32, 65], BF16, name="avh", tag="scnt")
                    bvh = scn.tile([128, 32, 65], BF16, name="bvh", tag="scnt")
                    nc.gpsimd.memset(avh[:, :, 0:1], 0.0)
                    nc.gpsimd.memset(bvh[:, :, 0:1], 0.0)
                    dv_ev = view(ev[:, :, :], [ev.ap[0], [1, 32], [64, 64]],
                                 off=ct * S + half * 32)
                    dv_et = view(et[:, :, :], [et.ap[0], [1, 32], [64, 64]],
                                 off=ct * S + half * 32)
                    scan_raw(view(avh[:, :, :], [avh.ap[0], [65, 32], [1, 64]], off=1),
                             lamview(ct, 32), dv_ev)
                    scan_raw(view(bvh[:, :, :], [bvh.ap[0], [65, 32], [1, 64]], off=1),
                             lamview(ct, 32), dv_et)
                    den = dnp.tile([128, 32, 64], F32, name="denv", tag="den")
                    nc.vector.scalar_tensor_tensor(
                        out=den, in0=dv_et,
                        scalar=col('eu', ct), in1=bvh[:, :, 0:64], op0=MM, op1=AD)
                    nc.vector.reciprocal_approx_fast(out=den, in_=den)
                    rdb = scn.tile([128, 32, 64], BF16, name="rdbv", tag="scnt")
                    nc.scalar.copy(out=rdb, in_=den)
                    nc.vector.scalar_tensor_tensor(
                        out=outv[:, ct, wr_, :], in0=dv_ev,
                        scalar=col('eu', ct), in1=avh[:, :, 0:64], op0=MM, op1=AD)
                    nc.gpsimd.tensor_tensor(out=outv[:, ct, wr_, :],
                                            in0=outv[:, ct, wr_, :], in1=rdb, op=MM)

            # horizontal: scan along w per (h, ct); num/out in place on ev
            def hscan_group(half):
                hr = slice(half * 32, (half + 1) * 32)
                for ct in range(2):
                    ahz = scn.tile([128, 32, 66], BF16, name="ahz", tag="scnt")
                    bhz = scn.tile([128, 32, 66], BF16, name="bhz", tag="scnt")
                    nc.gpsimd.memset(ahz[:, :, 0:1], 0.0)
                    nc.gpsimd.memset(bhz[:, :, 0:1], 0.0)
                    scan_raw(view(ahz[:, :, :], [ahz.ap[0], [66, 32], [1, 64]], off=1),
                             lamview(ct, 32), ev4[:, ct, hr, :])
                    scan_raw(view(bhz[:, :, :], [bhz.ap[0], [66, 32], [1, 64]], off=1),
                             lamview(ct, 32), et4[:, ct, hr, :])
                    den = dnp.tile([128, 32, 64], F32, name="denh", tag="den")
                    nc.vector.scalar_tensor_tensor(
                        out=den, in0=et4[:, ct, hr, :],
                        scalar=col('eu', ct), in1=bhz[:, :, 0:64], op0=MM, op1=AD)
                    nc.vector.reciprocal_approx_fast(out=den, in_=den)
                    rdb = scn.tile([128, 32, 64], BF16, name="rdbh", tag="scnt")
                    nc.scalar.copy(out=rdb, in_=den)
                    nc.vector.scalar_tensor_tensor(
                        out=ev4[:, ct, hr, :], in0=ev4[:, ct, hr, :],
                        scalar=col('eu', ct), in1=ahz[:, :, 0:64], op0=MM, op1=AD)
                    nc.gpsimd.tensor_tensor(out=ev4[:, ct, hr, :],
                                            in0=ev4[:, ct, hr, :], in1=rdb, op=MM)

            def c1_chunk(ch, wts, ytile, xtiles, g):
                h0 = ch * 8
                for mt in range(2):
                    ps = psmm.tile([128, CH], F32, name="c1_ps", tag="mm")
                    i = 0
                    for ti in range(9):
                        dy, dx = ti // 3 - 1, ti % 3 - 1
                        for kt in range(2):
                            nc.tensor.matmul(
                                out=ps.rearrange("p (a b) -> p a b", a=8),
                                lhsT=wts[mt][:, ti, kt, :],
                                rhs=xtiles[kt][:, 1 + h0 + dy:9 + h0 + dy,
                                               1 + dx:65 + dx],
                                start=(i == 0), stop=(i == 17))
                            i += 1
                    nc.scalar.activation(
                        out=ytile[:, mt, ch * CH:(ch + 1) * CH], in_=ps,
                        func=Act.Gelu, bias=col('c1b', 2 * g + mt))

            vscan_group(0)
            vscan_group(1)

            y1b = big.tile([128, 2, S], BF16, name="y1b", tag="Xb")
            c1w_g1 = []
            for mt in range(2):
                t = wstr.tile([128, 9, 2, 128], BF16, name=f"c1g1m{mt}", tag="wst9")
                nc.sync.dma_start(out=t, in_=d_c1[1, mt, :, :, :, :])
                c1w_g1.append(t)
            for ch in range(NCH // 2):
                c1_chunk(ch, c1w_g1, y1b, xcp_g1, 1)

            hscan_group(0)
            hscan_group(1)

            for ch in range(NCH // 2, NCH):
                c1_chunk(ch, c1w_g1, y1b, xcp_g1, 1)

            # wkv = out_h + out_v^T per chunk, fused with key-LN stats
            # (0.5 factor dropped: LN-invariant)
            dstat_kn = dsc.tile([2, S], BF16, name="dstat_kn", tag="dstat")
            drow_kn = dsc.tile([2, S], BF16, name="drow_kn", tag="drow")
            for ch in range(NCH):
                h0 = ch * 8
                for ct in range(2):
                    ovT = view(outv[:, :, :, :], [outv.ap[0], [1, 8], [64, 64]],
                               off=ct * S + h0)
                    nc.vector.tensor_tensor(out=ev4[:, ct, h0:h0 + 8, :],
                                            in0=ev4[:, ct, h0:h0 + 8, :],
                                            in1=ovT, op=AD)
                ln_stats_bounce(lambda ct: ev[:, ct, :], dstat_kn, ch, "kn")
            ln_math(dstat_kn, drow_kn, "kn")
            if probe:
                pwkv = mkprobe("p_wkv", [C, S])
                for ct in range(2):
                    nc.gpsimd.dma_start(out=pwkv[128 * ct:128 * (ct + 1), :],
                                        in_=ev[:, ct, :])

            # ============ phase D: kn apply + srw + Wo + c1 main (lag 1) =====
            xcp_g0 = []
            for i in range(2):
                t = xcpp.tile([128, 66, 66], BF16, name=f"xcp{i}", tag="xcp")
                nc.gpsimd.memset(t[:, 0:1, :], 0.0)
                nc.gpsimd.memset(t[:, 65:66, :], 0.0)
                nc.gpsimd.memset(t[:, 1:65, 0:1], 0.0)
                nc.gpsimd.memset(t[:, 1:65, 65:66], 0.0)
                xcp_g0.append(t)
            c1w_g0 = []
            for mt in range(2):
                t = wstr.tile([128, 9, 2, 128], BF16, name=f"c1g0m{mt}", tag="wst9")
                nc.sync.dma_start(out=t, in_=d_c1[0, mt, :, :, :, :])
                c1w_g0.append(t)

            def wo_chunk(ch):
                sl = slice(ch * CH, (ch + 1) * CH)
                h0 = ch * 8
                # srw = (tanh(r/2)+1) * wkv  (sigmoid affine, 0.5 in Wo)
                nc.vector.scalar_tensor_tensor(out=sr[:, :, sl], in0=sr[:, :, sl],
                                               scalar=1.0, in1=ev[:, :, sl],
                                               op0=AD, op1=MM)
                for mt in range(2):
                    ps = psmm.tile([128, CH], F32, name="wo_ps", tag="mm")
                    for kt in range(2):
                        nc.tensor.matmul(out=ps, lhsT=wot[:, kt, 128 * mt:128 * (mt + 1)],
                                         rhs=sr[:, kt, sl], start=(kt == 0), stop=(kt == 1))
                    nc.vector.tensor_tensor(
                        out=xcp_g0[mt][:, 1 + h0:9 + h0, 1:65],
                        in0=xn4[:, mt, h0:h0 + 8, :],
                        in1=ps.rearrange("p (a b) -> p a b", a=8), op=AD)

            y1a = big.tile([128, 2, S], BF16, name="y1a", tag="A")
            for ch in range(NCH):
                pr, pm = ln_bcast(drow_kn, ch, "kn")
                sl = slice(ch * CH, (ch + 1) * CH)
                for ct in range(2):
                    nc.vector.tensor_tensor(out=ev[:, ct, sl], in0=ev[:, ct, sl],
                                            in1=pr, op=MM)
                    nc.vector.tensor_tensor(out=ev[:, ct, sl], in0=ev[:, ct, sl],
                                            in1=pm, op=SU)
                if not kn_triv:
                    for ct in range(2):
                        nc.vector.tensor_scalar(
                            out=ev[:, ct, sl], in0=ev[:, ct, sl],
                            scalar1=col('knw', ct), scalar2=col('knb', ct),
                            op0=MM, op1=AD)
                wo_chunk(ch)
                if ch >= 1:
                    c1_chunk(ch - 1, c1w_g0, y1a, xcp_g0, 0)
            if probe:
                pxc = mkprobe("p_xcat", [CS, S])
                for i, t in enumerate(xcp_g0 + xcp_g1):
                    nc.gpsimd.dma_start(
                        out=pxc[128 * i:128 * (i + 1), :].rearrange("p (a b) -> p a b", a=64),
                        in_=t[:, 1:65, 1:65])
            c1_chunk(NCH - 1, c1w_g0, y1a, xcp_g0, 0)
            y1t = [y1a, y1b]

            if probe:
                py1 = mkprobe("p_y1", [CS, S])
                for i in range(4):
                    nc.gpsimd.dma_start(out=py1[128 * i:128 * (i + 1), :],
                                        in_=y1t[i // 2][:, i % 2, :])

            # ============ phase F: c2/c3 with LN2 stats fused ============
            y3 = [big.tile([128, S], BF16, name="y3_0", tag="Cxn"),
                  big.tile([128, S], BF16, name="y3_1", tag="Mf")]
            c3wt = wstr.tile([128, 8, 256], BF16, name="c3wt", tag="wst9")
            nc.sync.dma_start(out=c3wt, in_=d_c3[:, :].rearrange("(kt p) m -> p kt m", p=128))
            dstat2 = dsc.tile([2, S], BF16, name="dstat_ln2", tag="dstat")
            drow2 = dsc.tile([2, S], BF16, name="drow_ln2", tag="drow")
            for ch in range(NCH):
                sl = slice(ch * CH, (ch + 1) * CH)
                ytiles = []
                for mt in range(8):
                    ps = psmm.tile([128, CH], F32, name="c2_ps", tag="mm")
                    for kt in range(4):
                        nc.tensor.matmul(out=ps, lhsT=c2wt[:, kt, 128 * mt:128 * (mt + 1)],
                                         rhs=y1t[kt // 2][:, kt % 2, sl],
                                         start=(kt == 0), stop=(kt == 3))
                    yt = y2b.tile([128, CH], BF16, name="y2t", tag="y2t")
                    nc.scalar.activation(out=yt, in_=ps, func=Act.Gelu, bias=col('c2b', mt))
                    ytiles.append(yt)
                for mt in range(2):
                    ps = psmm.tile([128, CH], F32, name="c3_ps", tag="mm")
                    for kt in range(8):
                        nc.tensor.matmul(out=ps, lhsT=c3wt[:, kt, 128 * mt:128 * (mt + 1)],
                                         rhs=ytiles[kt], start=(kt == 0), stop=(kt == 7))
                    nc.scalar.activation(out=y3[mt][:, sl], in_=ps, func=Act.Gelu,
                                         bias=col('c3b', mt))
                    if not bn3_triv:
                        nc.vector.tensor_scalar(out=y3[mt][:, sl], in0=y3[mt][:, sl],
                                                scalar1=col('g3p', mt),
                                                scalar2=col('b3p', mt), op0=MM, op1=AD)
                ln_stats_bounce(lambda ct: y3[ct][:, :], dstat2, ch, "ln2")
            ln_math(dstat2, drow2, "ln2")

            if probe:
                py3 = mkprobe("p_y3", [C, S])
                for i in range(2):
                    nc.gpsimd.dma_start(out=py3[128 * i:128 * (i + 1), :],
                                        in_=y3[i][:, :])

            # ============ phase G: LN2 apply + up-proj + shuffle-out =========
            uptb = wstr.tile([128, 2, 512], BF16, name="uptb", tag="wst9")
            nc.sync.dma_start(out=uptb, in_=d_up[:, :].rearrange("(kt p) m -> p kt m", p=128))
            for ch in range(NCH):
                pr, pm = ln_bcast(drow2, ch, "ln2")
                sl = slice(ch * CH, (ch + 1) * CH)
                for ct in range(2):
                    nc.vector.tensor_tensor(out=y3[ct][:, sl], in0=y3[ct][:, sl],
                                            in1=pr, op=MM)
                    nc.vector.tensor_tensor(out=y3[ct][:, sl], in0=y3[ct][:, sl],
                                            in1=pm, op=SU)
                if not ln2_triv:
                    for ct in range(2):
                        nc.vector.tensor_scalar(
                            out=y3[ct][:, sl], in0=y3[ct][:, sl],
                            scalar1=col('ln2w', ct), scalar2=col('ln2b', ct),
                            op0=MM, op1=AD)
                h0 = ch * 8
                for r in range(2):
                    ub = wsk.tile([128, 8, 64, 2], F32, name="ub", tag="skb")
                    for q in range(2):
                        rq = 2 * r + q
                        ps = psmm.tile([128, CH], F32, name="up_ps", tag="mm")
                        for kt in range(2):
                            nc.tensor.matmul(out=ps,
                                             lhsT=uptb[:, kt, 128 * rq:128 * (rq + 1)],
                                             rhs=y3[kt][:, sl],
                                             start=(kt == 0), stop=(kt == 1))
                        nc.scalar.activation(out=ub[:, :, :, q],
                                             in_=ps.rearrange("p (a b) -> p a b", a=8),
                                             func=Act.Identity, bias=col('upb', rq))
                    dst = view(yout[:, :, :], [[128 * 128, 128], [256, 8], [1, 128]],
                               off=(2 * h0 + r) * 128)
                    nc.sync.dma_start(out=dst, in_=ub.rearrange("p a b q -> p a (b q)"))

    nc.compile()
    return nc, const_inputs


def _get_nc(weights, probe=False):
    import hashlib
    hsh = hashlib.sha1()
    for k in sorted(weights):
        hsh.update(k.encode())
        hsh.update(np.ascontiguousarray(weights[k]).tobytes())
    key = (hsh.hexdigest(), probe)
    if key not in _CACHE:
        _CACHE[key] = _build(weights, probe=probe)
    return _CACHE[key]


def kernel(**inputs):
    from concourse.bass_utils import run_bass_kernel_spmd

    x = np.asarray(inputs['x'], np.float32)
    skip = np.asarray(inputs['skip'], np.float32)
    mask = np.asarray(inputs['saliency_mask'], np.float32)
    weights = {k: np.asarray(v, np.float32) for k, v in inputs.items()
               if k not in ('x', 'skip', 'saliency_mask')}

    probe = bool(os.environ.get('BASSK_PROBE'))
    nc, const_inputs = _get_nc(weights, probe=probe)

    in_maps = []
    for b in range(B):
        m = dict(
            xin=np.ascontiguousarray(x[b].reshape(C, S)),
            skin=np.ascontiguousarray(skip[b].reshape(CS, S)),
            mrow=np.ascontiguousarray(mask[b].reshape(1, S)),
        )
        m.update(const_inputs)
        in_maps.append(m)
    res = run_bass_kernel_spmd(nc, in_maps, core_ids=list(range(B)),
                               trace=bool(os.environ.get('BASSK_TRACE')))
    kernel.last_results = res
    out = np.stack([res.results[b]['yout'] for b in range(B)], axis=0)
    return out


# revision 47
# speedup vs baseline: 1.5840x; 1.0240x over previous
"""Trainium2 Bass kernel for nn_DecoderBlock (shape-guided RWKV decoder block).

Data-parallel over batch: B=8 samples -> 8 NeuronCores, one NEFF.

Fully pipelined per-chunk structure for PE occupancy (HAM clock-gate)
and engine balance (810us -> 603us vs the session-start baseline):
- all matmuls bf16 (FWL enabled, no fp32-HIGH mode);
- inputs loaded via casting DMAs (gpsimd queue casts fp32->bf16 in flight);
- LN stat transpose-bounce uses [32,128]-shaped contiguous DMA views (the
  naive scatter view generated 16k 4-byte DMA packets, ~35us per LN);
- LN1 tail loop fuses broadcast/apply with per-chunk q_shift-diff (md) and
  the k/v/r projections + ev product; r gate evacuated as tanh(x/2) with
  sigmoid's affine folded into srw/Wo (keeps one ACT table, no reloads);
- WKV scans write near-contiguous zero-padded tiles; the skip-group half
  of the 3x3 conv runs on PE during the DVE scan phase; wkv combine and
  key-LN stats fused per chunk into the scan tail;
- key-LN tail loop fuses broadcast/apply/srw/Wo with the main-group 3x3
  conv at 1-chunk lag; c2/c3 loop fuses LN2 stats; LN2 tail fuses up-proj
  and pixel-shuffle output DMA;
- Pool (gpsimd) takes strided edge ops and scan-output multiplies; DVE
  keeps scans/den/num/applies; ACT evacuates all PSUM with fused bias+act.
"""
import sys
import os

for _p in ('/opt/trn_rl_repo', '/root/.axon_site/_ro/trn_rl_repo'):
    if _p not in sys.path and os.path.isdir(_p):
        sys.path.append(_p)

import numpy as np

B, C, CS, COUT, H, W = 8, 256, 512, 128, 64, 64
S = H * W          # 4096
NCH = 8            # spatial chunks
CH = S // NCH      # 512
EPS = 1e-5

_CACHE = {}


def _build(weights, probe=False):
    const_inputs = {}
    import concourse.bass as bass
    from concourse import bacc
    import concourse.tile as tile
    import concourse.mybir as mybir
    import ml_dtypes

    F32 = mybir.dt.float32
    BF16 = mybir.dt.bfloat16
    Alu = mybir.AluOpType
    Act = mybir.ActivationFunctionType
    MM, AD, SU = Alu.mult, Alu.add, Alu.subtract

    w = weights
    f64 = lambda x: np.asarray(x, np.float64)
    bf = lambda a: np.asarray(a, dtype=ml_dtypes.bfloat16)

    # ---------------- host-side folding
    bnscale = 1.0 / np.sqrt(1.0 + EPS)
    g1p = f64(w['bn1_g']) * bnscale
    b1p = f64(w['bn1_b'])
    g2p = f64(w['bn2_g']) * bnscale
    b2p = f64(w['bn2_b'])
    g3p = (f64(w['bn3_g']) * bnscale).astype(np.float32)
    b3p = f64(w['bn3_b']).astype(np.float32)

    c2_eff = f64(w['c2_w']) * g1p[None, :]
    c2b_eff = (f64(w['c2_b']) + f64(w['c2_w']) @ b1p).astype(np.float32)
    c3_eff = f64(w['c3_w']) * g2p[None, :]
    c3b_eff = (f64(w['c3_b']) + f64(w['c3_w']) @ b2p).astype(np.float32)

    wk_x = f64(w['Wk']).T
    wk_d = (f64(w['Wk']) * (1.0 - f64(w['mix_k']))[None, :]).T
    wv_x = f64(w['Wv']).T
    wv_d = (f64(w['Wv']) * (1.0 - f64(w['mix_v']))[None, :]).T
    wr_x = f64(w['Wr']).T
    wr_d = (f64(w['Wr']) * (1.0 - f64(w['mix_r']))[None, :]).T
    # r gate evacuated as tanh(x/2); sigmoid(x) = 0.5*(tanh(x/2)+1), the
    # (t+1) is folded into srw and the 0.5 into Wo here.
    wo_t = f64(w['Wo']).T * 0.5
    sp_t = f64(w['sp_w']).T

    lam = np.exp(-np.exp(f64(w['decay']))).astype(np.float32)
    lam64 = np.tile(lam[:, None], (1, 64))
    lam64[:, 0] = 0.0
    lam64 = lam64.astype(np.float32)
    eu = np.exp(f64(w['first'])).astype(np.float32)

    pidx = np.arange(512)
    old = (pidx % 128) * 4 + (pidx // 128)
    up_t = f64(w['up_w'])[old].T                                # [256, 512]
    upb_p = f64(w['up_b'])[old].astype(np.float32)

    c1w = f64(w['c1_w'])
    c1_l = np.zeros((9, 2, 256, 256), np.float32)
    for ti in range(9):
        dy, dx = ti // 3, ti % 3
        for g in range(2):
            c1_l[ti, g] = c1w[g * 256:(g + 1) * 256, :, dy, dx].T

    # per-channel vectors as columns of one [128, ncol] const
    cols, order = {}, []

    def addcol(name, vec):
        v = np.asarray(vec, np.float32).reshape(-1, 128)
        cols[name] = v
        order.append(name)

    addcol('eu', eu)
    addcol('ln1w', w['ln1_w'])
    addcol('ln1b', w['ln1_b'])
    addcol('knw', w['kn_w'])
    addcol('knb', w['kn_b'])
    addcol('ln2w', w['ln2_w'])
    addcol('ln2b', w['ln2_b'])
    addcol('g3p', g3p)
    addcol('b3p', b3p)
    addcol('spb', w['sp_b'])
    addcol('c3b', c3b_eff)
    addcol('c1b', w['c1_b'])
    addcol('c2b', c2b_eff)
    addcol('upb', upb_p)
    colidx, ncol = {}, 0
    for n in order:
        colidx[n] = ncol
        ncol += cols[n].shape[0]
    cvec_np = np.zeros((128, ncol), np.float32)
    for n in order:
        for i in range(cols[n].shape[0]):
            cvec_np[:, colidx[n] + i] = cols[n][i]

    ln1_triv = np.all(w['ln1_w'] == 1.0) and np.all(w['ln1_b'] == 0.0)
    kn_triv = np.all(w['kn_w'] == 1.0) and np.all(w['kn_b'] == 0.0)
    ln2_triv = np.all(w['ln2_w'] == 1.0) and np.all(w['ln2_b'] == 0.0)
    bn3_triv = np.all(g3p == g3p[0]) and np.all(b3p == 0.0)
    # uniform bn3 scale commutes with LN2 -> drop it entirely when trivial

    # ---------------- bass module
    nc = bacc.Bacc("TRN2", target_bir_lowering=False, debug=False, name="decblk")

    xin = nc.dram_tensor("xin", [C, S], F32, kind="ExternalInput")
    skin = nc.dram_tensor("skin", [CS, S], F32, kind="ExternalInput")
    mrow = nc.dram_tensor("mrow", [1, S], F32, kind="ExternalInput")
    yout = nc.dram_tensor("yout", [COUT, 2 * H, 2 * W], F32, kind="ExternalOutput")
    probes = {}

    def mkprobe(name, shape):
        if probe:
            probes[name] = nc.dram_tensor(name, shape, F32, kind="ExternalOutput")
        return probes.get(name)

    def it(arr, name):
        arr = np.ascontiguousarray(arr)
        import ml_dtypes as _md
        dt_ = {np.dtype(np.float32): F32, np.dtype(_md.bfloat16): BF16}[arr.dtype]
        const_inputs[name] = arr
        return nc.dram_tensor(name, list(arr.shape), dt_, kind="ExternalInput")

    d_lam = it(lam64, "lam64")
    d_cvec = it(cvec_np, "cvec")
    d_wk = [it(bf(wk_x), "wkx"), it(bf(wk_d), "wkd")]
    d_wv = [it(bf(wv_x), "wvx"), it(bf(wv_d), "wvd")]
    d_wr = [it(bf(wr_x), "wrx"), it(bf(wr_d), "wrd")]
    d_wo = it(bf(wo_t), "wo")
    d_sp = it(bf(sp_t), "sp")
    d_up = it(bf(up_t), "up")
    c1_r = c1_l.reshape(9, 2, 2, 128, 2, 128).transpose(1, 4, 3, 0, 2, 5)
    d_c1 = it(bf(c1_r), "c1")   # [g, mt, p, t, kt, m]
    d_c2 = it(bf(c2_eff.T), "c2")
    d_c3 = it(bf(c3_eff.T), "c3")
    red_np = np.zeros((128, 2, 2), np.float32)
    red_np[:, 0, 0] = 1.0
    red_np[:, 1, 1] = 1.0
    d_redb = it(bf(red_np), "redb")
    bc2_np = np.zeros((2, 2, 128), np.float32)
    bc2_np[0, 0, :] = 1.0
    bc2_np[1, 1, :] = 1.0
    d_bc1 = it(bf(bc2_np), "bc2")
    d_eps = it(np.full((128, 1), EPS, np.float32), "epsc")

    def scan_raw(out, d0, d1):
        eng = nc.vector
        return eng.add_instruction(mybir.InstTensorScalarPtr(
            name=nc.get_next_instruction_name(),
            is_tensor_tensor_scan=True,
            is_scalar_tensor_tensor=True,
            op0=MM, op1=AD,
            ins=[eng.lower_ap(d0), eng.lower_ap_or_imm(0.0), eng.lower_ap(d1)],
            outs=[eng.lower_ap(out)],
        ))

    def view(ap, dims, off=0):
        return bass.AP(tensor=ap.tensor, offset=ap.offset + off, ap=dims)

    with tile.TileContext(nc) as tc:
        with tc.tile_pool(name="big", bufs=1) as big, \
             tc.tile_pool(name="dnp", bufs=2) as dnp, \
             tc.tile_pool(name="scn", bufs=3) as scn, \
             tc.tile_pool(name="wres", bufs=1) as wres, \
             tc.tile_pool(name="sml", bufs=3) as sml, \
             tc.tile_pool(name="y2b", bufs=8) as y2b, \
             tc.tile_pool(name="wstr", bufs=2) as wstr, \
             tc.tile_pool(name="wsk", bufs=2) as wsk, \
             tc.tile_pool(name="xcpp", bufs=2) as xcpp, \
             tc.tile_pool(name="dsc", bufs=2, space="DRAM") as dsc, \
             tc.tile_pool(name="psmm", bufs=3, space="PSUM") as psmm, \
             tc.tile_pool(name="psst", bufs=1, space="PSUM") as psst, \
             tc.tile_pool(name="psbc", bufs=2, space="PSUM") as psbc:

            # ---- resident constants
            lt = wres.tile([128, 2, 64], F32, name="lt")
            nc.sync.dma_start(out=lt, in_=d_lam[:, :].rearrange("(t p) j -> p t j", p=128))
            cv = wres.tile([128, ncol], F32, name="cv")
            nc.sync.dma_start(out=cv, in_=d_cvec[:, :])
            redb = wres.tile([128, 2, 2], BF16, name="redb")
            nc.sync.dma_start(out=redb, in_=d_redb[:, :, :])
            bc1 = wres.tile([2, 2, 128], BF16, name="bc1")
            nc.sync.dma_start(out=bc1, in_=d_bc1[:, :, :])
            epsc = wres.tile([128, 1], F32, name="epsc")
            nc.sync.dma_start(out=epsc, in_=d_eps[:, :])
            wot = wres.tile([128, 2, 256], BF16, name="wot")
            nc.sync.dma_start(out=wot, in_=d_wo[:, :].rearrange("(kt p) m -> p kt m", p=128))
            kvrw = []
            for nm, dws in (("wk", d_wk), ("wv", d_wv), ("wr", d_wr)):
                wxt = wres.tile([128, 2, 256], BF16, name=f"{nm}x")
                wdt = wres.tile([128, 2, 256], BF16, name=f"{nm}d")
                nc.sync.dma_start(out=wxt, in_=dws[0][:, :].rearrange("(kt p) m -> p kt m", p=128))
                nc.sync.dma_start(out=wdt, in_=dws[1][:, :].rearrange("(kt p) m -> p kt m", p=128))
                kvrw.append((wxt, wdt))
            c2wt = wres.tile([128, 4, 1024], BF16, name="c2wt")
            nc.sync.dma_start(out=c2wt, in_=d_c2[:, :].rearrange("(kt p) m -> p kt m", p=128))

            def col(name, i=0):
                return cv[:, colidx[name] + i:colidx[name] + i + 1]

            def ln_stats_chunk(Xr, ps, sl, name):
                nc.tensor.matmul(out=ps, lhsT=redb[:, 0, :], rhs=Xr(0)[:, sl],
                                 start=True, stop=False)
                nc.tensor.matmul(out=ps, lhsT=redb[:, 0, :], rhs=Xr(1)[:, sl],
                                 start=False, stop=False)
                for ct in range(2):
                    sq = sml.tile([128, CH], BF16, name=f"sq_{name}", tag="sqc", bufs=2)
                    nc.scalar.activation(out=sq, in_=Xr(ct)[:, sl], func=Act.Square)
                    nc.tensor.matmul(out=ps, lhsT=redb[:, 1, :], rhs=sq,
                                     start=False, stop=(ct == 1))

            def ln_stats_bounce(Xr, dstat, ch, name):
                sl = slice(ch * CH, (ch + 1) * CH)
                ps = psst.tile([2, CH], F32, name=f"lnps_{name}", tag="st")
                ln_stats_chunk(Xr, ps, sl, name)
                stc = sml.tile([2, CH], BF16, name=f"stc_{name}", tag="stc", bufs=2)
                nc.scalar.copy(out=stc, in_=ps)
                nc.sync.dma_start(out=dstat[:, sl], in_=stc)

            def ln_math(dstat, drow, name):
                # [2, S] stats -> [32, 128] tiles (contiguous 512B-per-partition
                # DMA; the naive [[1,128],...] view scatters into 4-byte packets
                # costing ~35us per LN)
                mu = sml.tile([32, 128], F32, name=f"mu_{name}", tag="sm", bufs=2)
                sq = sml.tile([32, 128], BF16, name=f"sq_{name}", tag="sm2", bufs=2)
                nc.gpsimd.dma_start(out=mu, in_=view(dstat[:, :], [[128, 32], [1, 128]]))
                nc.sync.dma_start(out=sq, in_=view(dstat[:, :], [[128, 32], [1, 128]], off=S))
                nc.vector.tensor_scalar_mul(out=mu, in0=mu, scalar1=1.0 / C)
                t2 = sml.tile([32, 128], F32, name=f"t2_{name}", tag="t2", bufs=2)
                nc.vector.tensor_tensor(out=t2, in0=mu, in1=mu, op=MM)
                nc.vector.scalar_tensor_tensor(out=t2, in0=sq, scalar=1.0 / C,
                                               in1=t2, op0=MM, op1=SU)
                nc.scalar.activation(out=t2, in_=t2, func=Act.Sqrt, bias=epsc[0:32, :])
                nc.vector.reciprocal(out=t2, in_=t2)                      # rstd
                smb = sml.tile([32, 2, 128], BF16, name=f"smb_{name}", tag="smb", bufs=2)
                nc.vector.tensor_copy(out=smb[:, 0, :], in_=t2)
                nc.vector.tensor_tensor(out=smb[:, 1, :], in0=mu, in1=t2, op=MM)
                nc.sync.dma_start(out=view(drow[:, :], [[128, 32], [1, 128]]),
                                  in_=smb[:, 0, :])
                nc.sync.dma_start(out=view(drow[:, :], [[128, 32], [1, 128]], off=S),
                                  in_=smb[:, 1, :])

            def ln_bcast(drow, ch, name):
                sl = slice(ch * CH, (ch + 1) * CH)
                bcrc = sml.tile([2, CH], BF16, name=f"bcr_{name}", tag="bcrc", bufs=2)
                nc.scalar.dma_start(out=bcrc, in_=drow[:, sl])
                pr = psbc.tile([128, CH], F32, name=f"pr_{name}", tag="pr")
                pm = psbc.tile([128, CH], F32, name=f"pm_{name}", tag="pm")
                nc.tensor.matmul(out=pr, lhsT=bc1[:, 0, :], rhs=bcrc)
                nc.tensor.matmul(out=pm, lhsT=bc1[:, 1, :], rhs=bcrc)
                return pr, pm

            # ============ phase A: load x (casting DMAs), LN1, skip ======
            xb = big.tile([128, 2, S], BF16, name="xb", tag="Xb")
            HS = S // 2
            for hf in range(2):
                hsl = slice(hf * HS, (hf + 1) * HS)
                nc.gpsimd.dma_start(out=xb[:, 0, hsl], in_=xin[0:128, hsl])
                nc.gpsimd.dma_start(out=xb[:, 1, hsl], in_=xin[128:256, hsl])

            # mask: fp32 row -> bf16 (in [32,128] layout) -> DRAM -> broadcast
            m1b = sml.tile([32, 128], BF16, name="m1b", tag="m1b", bufs=1)
            nc.gpsimd.dma_start(out=m1b, in_=view(mrow[:, :], [[128, 32], [1, 128]]))
            dmask = dsc.tile([32, 128], BF16, name="dmask", tag="dmask", bufs=1)
            nc.sync.dma_start(out=dmask, in_=m1b)
            mfb = big.tile([128, S], BF16, name="mfb", tag="Mf")
            nc.sync.dma_start(out=mfb, in_=view(dmask[:, :], [[0, 128], [1, S]]))

            # xcp tiles for skip group written early; borders zeroed
            xcp_g1 = []
            for i in range(2):
                t = xcpp.tile([128, 66, 66], BF16, name=f"xcp{2 + i}", tag="xcp")
                nc.gpsimd.memset(t[:, 0:1, :], 0.0)
                nc.gpsimd.memset(t[:, 65:66, :], 0.0)
                nc.gpsimd.memset(t[:, 1:65, 0:1], 0.0)
                nc.gpsimd.memset(t[:, 1:65, 65:66], 0.0)
                xcp_g1.append(t)

            sptb = wstr.tile([128, 4, 256], BF16, name="sptb", tag="wst9")
            nc.sync.dma_start(out=sptb, in_=d_sp[:, :].rearrange("(kt p) m -> p kt m", p=128))

            xn = big.tile([128, 2, S], BF16, name="xn", tag="Cxn")
            xn4 = xn.rearrange("p t (h w) -> p t h w", h=H)

            # LN1 stats per chunk with skip-conv half-chunks interleaved
            dstat1 = dsc.tile([2, S], BF16, name="dstat_ln1", tag="dstat")
            drow1 = dsc.tile([2, S], BF16, name="drow_ln1", tag="drow")
            HCH = CH // 2

            def skip_half(ch, hh):
                hsl = slice(ch * CH + hh * HCH, ch * CH + (hh + 1) * HCH)
                h0 = ch * 8 + hh * 4
                skb = wsk.tile([128, 4, HCH], BF16, name="skb", tag="skb", bufs=2)
                nc.gpsimd.dma_start(
                    out=skb,
                    in_=skin[:, hsl].rearrange("(kt p) n -> p kt n", p=128))
                for mt in range(2):
                    psk = psmm.tile([128, HCH], F32, name="sp_ps", tag="mm")
                    for kt in range(4):
                        nc.tensor.matmul(out=psk,
                                         lhsT=sptb[:, kt, 128 * mt:128 * (mt + 1)],
                                         rhs=skb[:, kt, :],
                                         start=(kt == 0), stop=(kt == 3))
                    nc.scalar.activation(
                        out=xcp_g1[mt][:, 1 + h0:5 + h0, 1:65],
                        in_=psk.rearrange("p (a b) -> p a b", a=4),
                        func=Act.Identity, bias=col('spb', mt))

            for ch in range(NCH):
                ln_stats_bounce(lambda ct: xb[:, ct, :], dstat1, ch, "ln1")
                if ch < 6:
                    skip_half(ch, 0)
                    skip_half(ch, 1)
            ln_math(dstat1, drow1, "ln1")

            # ---- per-chunk q_shift diff (+mask) for rows [8ch, 8ch+8);
            # lives in a small ring tile, consumed by k/v/r of the same chunk
            def md_chunk(ch):
                r0 = ch * 8
                rs = slice(r0, r0 + 8)
                mdc = sml.tile([128, 2, 8, W], BF16, name="mdc", tag="mdc", bufs=2)
                # ct=0: w-shifts (chunk-local rows)
                nc.vector.tensor_tensor(out=mdc[0:64, 0, :, 1:],
                                        in0=xn4[0:64, 0, rs, 0:63],
                                        in1=xn4[0:64, 0, rs, 1:], op=SU)
                nc.gpsimd.tensor_scalar_mul(out=mdc[0:64, 0, :, 0:1],
                                            in0=xn4[0:64, 0, rs, 0:1], scalar1=-1.0)
                nc.vector.tensor_tensor(out=mdc[64:128, 0, :, 0:63],
                                        in0=xn4[64:128, 0, rs, 1:],
                                        in1=xn4[64:128, 0, rs, 0:63], op=SU)
                nc.gpsimd.tensor_scalar_mul(out=mdc[64:128, 0, :, 63:64],
                                            in0=xn4[64:128, 0, rs, 63:64], scalar1=-1.0)
                # ct=1: h-shifts (reads rows r0-1 .. r0+8)
                if ch == 0:
                    nc.gpsimd.tensor_scalar_mul(out=mdc[0:64, 1, 0:1, :],
                                                in0=xn4[0:64, 1, 0:1, :], scalar1=-1.0)
                    nc.vector.tensor_tensor(out=mdc[0:64, 1, 1:8, :],
                                            in0=xn4[0:64, 1, 0:7, :],
                                            in1=xn4[0:64, 1, 1:8, :], op=SU)
                else:
                    nc.vector.tensor_tensor(out=mdc[0:64, 1, :, :],
                                            in0=xn4[0:64, 1, r0 - 1:r0 + 7, :],
                                            in1=xn4[0:64, 1, rs, :], op=SU)
                if ch == NCH - 1:
                    nc.vector.tensor_tensor(out=mdc[64:128, 1, 0:7, :],
                                            in0=xn4[64:128, 1, 57:64, :],
                                            in1=xn4[64:128, 1, 56:63, :], op=SU)
                    nc.gpsimd.tensor_scalar_mul(out=mdc[64:128, 1, 7:8, :],
                                                in0=xn4[64:128, 1, 63:64, :], scalar1=-1.0)
                else:
                    nc.vector.tensor_tensor(out=mdc[64:128, 1, :, :],
                                            in0=xn4[64:128, 1, r0 + 1:r0 + 9, :],
                                            in1=xn4[64:128, 1, rs, :], op=SU)
                sl = slice(ch * CH, (ch + 1) * CH)
                mdr = mdc.rearrange("p t h w -> p t (h w)")
                for ct in range(2):
                    nc.vector.tensor_tensor(out=mdr[:, ct, :], in0=mdr[:, ct, :],
                                            in1=mfb[:, sl], op=MM)
                return mdr

            et = big.tile([128, 2, S], BF16, name="et", tag="A")
            vv = big.tile([128, 2, S], BF16, name="vv", tag="D")
            ev = big.tile([128, 2, S], BF16, name="ev", tag="B")
            sr = big.tile([128, 2, S], BF16, name="sr", tag="Fsr")

            def kv_chunk(widx, ch, mdr, evac):
                wxt, wdt = kvrw[widx]
                sl = slice(ch * CH, (ch + 1) * CH)
                for mt in range(2):
                    ps = psmm.tile([128, CH], F32, name="kv_ps", tag="mm")
                    for kt in range(2):
                        nc.tensor.matmul(out=ps, lhsT=wxt[:, kt, 128 * mt:128 * (mt + 1)],
                                         rhs=xn[:, kt, sl], start=(kt == 0), stop=False)
                    for kt in range(2):
                        nc.tensor.matmul(out=ps, lhsT=wdt[:, kt, 128 * mt:128 * (mt + 1)],
                                         rhs=mdr[:, kt, :], start=False, stop=(kt == 1))
                    evac(mt, sl, ps)

            def kve_chunk(ch):
                mdr = md_chunk(ch)
                kv_chunk(0, ch, mdr, lambda mt, sl, ps: nc.scalar.activation(
                    out=et[:, mt, sl], in_=ps, func=Act.Exp))
                kv_chunk(1, ch, mdr, lambda mt, sl, ps: nc.scalar.copy(
                    out=vv[:, mt, sl], in_=ps))
                # r gate: tanh(x/2) (tanh shares the exp ACT table; no reload)
                kv_chunk(2, ch, mdr, lambda mt, sl, ps: nc.scalar.activation(
                    out=sr[:, mt, sl], in_=ps, func=Act.Tanh, scale=0.5))
                sl = slice(ch * CH, (ch + 1) * CH)
                nc.vector.tensor_tensor(out=ev[:, :, sl], in0=et[:, :, sl],
                                        in1=vv[:, :, sl], op=MM)

            # LN1 tail: broadcast/apply per chunk, fused with md/k/v/ev at lag 1;
            # remaining skip-conv chunks fill the PE during the pipeline ramp
            for ch in range(NCH):
                if ch < 4:
                    skip_half(6 + ch // 2, ch % 2)
                pr, pm = ln_bcast(drow1, ch, "ln1")
                sl = slice(ch * CH, (ch + 1) * CH)
                for ct in range(2):
                    nc.vector.tensor_tensor(out=xn[:, ct, sl], in0=xb[:, ct, sl],
                                            in1=pr, op=MM)
                    nc.vector.tensor_tensor(out=xn[:, ct, sl], in0=xn[:, ct, sl],
                                            in1=pm, op=SU)
                if not ln1_triv:
                    for ct in range(2):
                        nc.vector.tensor_scalar(
                            out=xn[:, ct, sl], in0=xn[:, ct, sl],
                            scalar1=col('ln1w', ct), scalar2=col('ln1b', ct),
                            op0=MM, op1=AD)
                if ch >= 1:
                    kve_chunk(ch - 1)
            kve_chunk(NCH - 1)
            if probe:
                pxn = mkprobe("p_xn", [C, S])
                for ct in range(2):
                    nc.gpsimd.dma_start(out=pxn[128 * ct:128 * (ct + 1), :],
                                        in_=xn[:, ct, :])

            # ============ phase C: WKV scans (DVE) vs r/c1-skip (PE) =========
            ev4 = ev.rearrange("p t (h w) -> p t h w", h=H)
            et4 = et.rearrange("p t (h w) -> p t h w", h=H)
            outv = big.tile([128, 2, W, H], BF16, name="outv", tag="D")  # w-major
            lt_ap = lt[:, :, :]

            def lamview(ct, nseq):
                return view(lt_ap, [lt_ap.ap[0], [0, nseq], [1, 64]], off=ct * 64)

            # vertical: scan along h per (w, ct); outputs w-major, zero-padded
            # leading h column so den/num read the h-1 shift without edge ops.
            def vscan_group(half):
                wr_ = slice(half * 32, (half + 1) * 32)
                for ct in range(2):
                    avh = scn.tile([128, 32, 65], BF16, name="avh", tag="scnt")
                    bvh = scn.tile([128, 32, 65], BF16, name="bvh", tag="scnt")
                    nc.gpsimd.memset(avh[:, :, 0:1], 0.0)
                    nc.gpsimd.memset(bvh[:, :, 0:1], 0.0)
                    dv_ev = view(ev[:, :, :], [ev.ap[0], [1, 32], [64, 64]],
                                 off=ct * S + half * 32)
                    dv_et = view(et[:, :, :], [et.ap[0], [1, 32], [64, 64]],
                                 off=ct * S + half * 32)
                    scan_raw(view(avh[:, :, :], [avh.ap[0], [65, 32], [1, 64]], off=1),
                             lamview(ct, 32), dv_ev)
                    scan_raw(view(bvh[:, :, :], [bvh.ap[0], [65, 32], [1, 64]], off=1),
                             lamview(ct, 32), dv_et)
                    den = dnp.tile([128, 32, 64], F32, name="denv", tag="den")
                    nc.vector.scalar_tensor_tensor(
                        out=den, in0=dv_et,
                        scalar=col('eu', ct), in1=bvh[:, :, 0:64], op0=MM, op1=AD)
                    nc.vector.reciprocal_approx_fast(out=den, in_=den)
                    nc.vector.scalar_tensor_tensor(
                        out=outv[:, ct, wr_, :], in0=dv_ev,
                        scalar=col('eu', ct), in1=avh[:, :, 0:64], op0=MM, op1=AD)
                    nc.vector.tensor_tensor(out=outv[:, ct, wr_, :],
                                            in0=outv[:, ct, wr_, :], in1=den, op=MM)

            # horizontal: scan along w per (h, ct); num/out in place on ev
            def hscan_group(half):
                hr = slice(half * 32, (half + 1) * 32)
                for ct in range(2):
                    ahz = scn.tile([128, 32, 66], BF16, name="ahz", tag="scnt")
                    bhz = scn.tile([128, 32, 66], BF16, name="bhz", tag="scnt")
                    nc.gpsimd.memset(ahz[:, :, 0:1], 0.0)
                    nc.gpsimd.memset(bhz[:, :, 0:1], 0.0)
                    scan_raw(view(ahz[:, :, :], [ahz.ap[0], [66, 32], [1, 64]], off=1),
                             lamview(ct, 32), ev4[:, ct, hr, :])
                    scan_raw(view(bhz[:, :, :], [bhz.ap[0], [66, 32], [1, 64]], off=1),
                             lamview(ct, 32), et4[:, ct, hr, :])
                    den = dnp.tile([128, 32, 64], F32, name="denh", tag="den")
                    nc.vector.scalar_tensor_tensor(
                        out=den, in0=et4[:, ct, hr, :],
                        scalar=col('eu', ct), in1=bhz[:, :, 0:64], op0=MM, op1=AD)
                    nc.vector.reciprocal_approx_fast(out=den, in_=den)
                    nc.vector.scalar_tensor_tensor(
                        out=ev4[:, ct, hr, :], in0=ev4[:, ct, hr, :],
                        scalar=col('eu', ct), in1=ahz[:, :, 0:64], op0=MM, op1=AD)
                    nc.vector.tensor_tensor(out=ev4[:, ct, hr, :],
                                            in0=ev4[:, ct, hr, :], in1=den, op=MM)

            def c1_chunk(ch, wts, ytile, xtiles, g):
                h0 = ch * 8
                for mt in range(2):
                    ps = psmm.tile([128, CH], F32, name="c1_ps", tag="mm")
                    i = 0
                    for ti in range(9):
                        dy, dx = ti // 3 - 1, ti % 3 - 1
                        for kt in range(2):
                            nc.tensor.matmul(
                                out=ps.rearrange("p (a b) -> p a b", a=8),
                                lhsT=wts[mt][:, ti, kt, :],
                                rhs=xtiles[kt][:, 1 + h0 + dy:9 + h0 + dy,
                                               1 + dx:65 + dx],
                                start=(i == 0), stop=(i == 17))
                            i += 1
                    nc.scalar.activation(
                        out=ytile[:, mt, ch * CH:(ch + 1) * CH], in_=ps,
                        func=Act.Gelu, bias=col('c1b', 2 * g + mt))

            vscan_group(0)
            vscan_group(1)

            y1b = big.tile([128, 2, S], BF16, name="y1b", tag="Xb")
            c1w_g1 = []
            for mt in range(2):
                t = wstr.tile([128, 9, 2, 128], BF16, name=f"c1g1m{mt}", tag="wst9")
                nc.sync.dma_start(out=t, in_=d_c1[1, mt, :, :, :, :])
                c1w_g1.append(t)
            for ch in range(NCH // 2):
                c1_chunk(ch, c1w_g1, y1b, xcp_g1, 1)

            hscan_group(0)
            hscan_group(1)

            for ch in range(NCH // 2, NCH):
                c1_chunk(ch, c1w_g1, y1b, xcp_g1, 1)

            # wkv = out_h + out_v^T per chunk, fused with key-LN stats
            # (0.5 factor dropped: LN-invariant)
            dstat_kn = dsc.tile([2, S], BF16, name="dstat_kn", tag="dstat")
            drow_kn = dsc.tile([2, S], BF16, name="drow_kn", tag="drow")
            for ch in range(NCH):
                h0 = ch * 8
                for ct in range(2):
                    ovT = view(outv[:, :, :, :], [outv.ap[0], [1, 8], [64, 64]],
                               off=ct * S + h0)
                    nc.vector.tensor_tensor(out=ev4[:, ct, h0:h0 + 8, :],
                                            in0=ev4[:, ct, h0:h0 + 8, :],
                                            in1=ovT, op=AD)
                ln_stats_bounce(lambda ct: ev[:, ct, :], dstat_kn, ch, "kn")
            ln_math(dstat_kn, drow_kn, "kn")
            if probe:
                pwkv = mkprobe("p_wkv", [C, S])
                for ct in range(2):
                    nc.gpsimd.dma_start(out=pwkv[128 * ct:128 * (ct + 1), :],
                                        in_=ev[:, ct, :])

            # ============ phase D: kn apply + srw + Wo + c1 main (lag 1) =====
            xcp_g0 = []
            for i in range(2):
                t = xcpp.tile([128, 66, 66], BF16, name=f"xcp{i}", tag="xcp")
                nc.gpsimd.memset(t[:, 0:1, :], 0.0)
                nc.gpsimd.memset(t[:, 65:66, :], 0.0)
                nc.gpsimd.memset(t[:, 1:65, 0:1], 0.0)
                nc.gpsimd.memset(t[:, 1:65, 65:66], 0.0)
                xcp_g0.append(t)
            c1w_g0 = []
            for mt in range(2):
                t = wstr.tile([128, 9, 2, 128], BF16, name=f"c1g0m{mt}", tag="wst9")
                nc.sync.dma_start(out=t, in_=d_c1[0, mt, :, :, :, :])
                c1w_g0.append(t)

            def wo_chunk(ch):
                sl = slice(ch * CH, (ch + 1) * CH)
                h0 = ch * 8
                # srw = (tanh(r/2)+1) * wkv  (sigmoid affine, 0.5 in Wo)
                nc.vector.scalar_tensor_tensor(out=sr[:, :, sl], in0=sr[:, :, sl],
                                               scalar=1.0, in1=ev[:, :, sl],
                                               op0=AD, op1=MM)
                for mt in range(2):
                    ps = psmm.tile([128, CH], F32, name="wo_ps", tag="mm")
                    for kt in range(2):
                        nc.tensor.matmul(out=ps, lhsT=wot[:, kt, 128 * mt:128 * (mt + 1)],
                                         rhs=sr[:, kt, sl], start=(kt == 0), stop=(kt == 1))
                    nc.vector.tensor_tensor(
                        out=xcp_g0[mt][:, 1 + h0:9 + h0, 1:65],
                        in0=xn4[:, mt, h0:h0 + 8, :],
                        in1=ps.rearrange("p (a b) -> p a b", a=8), op=AD)

            y1a = big.tile([128, 2, S], BF16, name="y1a", tag="A")
            for ch in range(NCH):
                pr, pm = ln_bcast(drow_kn, ch, "kn")
                sl = slice(ch * CH, (ch + 1) * CH)
                for ct in range(2):
                    nc.vector.tensor_tensor(out=ev[:, ct, sl], in0=ev[:, ct, sl],
                                            in1=pr, op=MM)
                    nc.vector.tensor_tensor(out=ev[:, ct, sl], in0=ev[:, ct, sl],
                                            in1=pm, op=SU)
                if not kn_triv:
                    for ct in range(2):
                        nc.vector.tensor_scalar(
                            out=ev[:, ct, sl], in0=ev[:, ct, sl],
                            scalar1=col('knw', ct), scalar2=col('knb', ct),
                            op0=MM, op1=AD)
                wo_chunk(ch)
                if ch >= 1:
                    c1_chunk(ch - 1, c1w_g0, y1a, xcp_g0, 0)
            if probe:
                pxc = mkprobe("p_xcat", [CS, S])
                for i, t in enumerate(xcp_g0 + xcp_g1):
                    nc.gpsimd.dma_start(
                        out=pxc[128 * i:128 * (i + 1), :].rearrange("p (a b) -> p a b", a=64),
                        in_=t[:, 1:65, 1:65])
            c1_chunk(NCH - 1, c1w_g0, y1a, xcp_g0, 0)
            y1t = [y1a, y1b]

            if probe:
                py1 = mkprobe("p_y1", [CS, S])
                for i in range(4):
                    nc.gpsimd.dma_start(out=py1[128 * i:128 * (i + 1), :],
                                        in_=y1t[i // 2][:, i % 2, :])

            # ============ phase F: c2/c3 with LN2 stats fused ============
            y3 = [big.tile([128, S], BF16, name="y3_0", tag="Cxn"),
                  big.tile([128, S], BF16, name="y3_1", tag="Mf")]
            c3wt = wstr.tile([128, 8, 256], BF16, name="c3wt", tag="wst9")
            nc.sync.dma_start(out=c3wt, in_=d_c3[:, :].rearrange("(kt p) m -> p kt m", p=128))
            dstat2 = dsc.tile([2, S], BF16, name="dstat_ln2", tag="dstat")
            drow2 = dsc.tile([2, S], BF16, name="drow_ln2", tag="drow")
            for ch in range(NCH):
                sl = slice(ch * CH, (ch + 1) * CH)
                ytiles = []
                for mt in range(8):
                    ps = psmm.tile([128, CH], F32, name="c2_ps", tag="mm")
                    for kt in range(4):
                        nc.tensor.matmul(out=ps, lhsT=c2wt[:, kt, 128 * mt:128 * (mt + 1)],
                                         rhs=y1t[kt // 2][:, kt % 2, sl],
                                         start=(kt == 0), stop=(kt == 3))
                    yt = y2b.tile([128, CH], BF16, name="y2t", tag="y2t")
                    nc.scalar.activation(out=yt, in_=ps, func=Act.Gelu, bias=col('c2b', mt))
                    ytiles.append(yt)
                for mt in range(2):
                    ps = psmm.tile([128, CH], F32, name="c3_ps", tag="mm")
                    for kt in range(8):
                        nc.tensor.matmul(out=ps, lhsT=c3wt[:, kt, 128 * mt:128 * (mt + 1)],
                                         rhs=ytiles[kt], start=(kt == 0), stop=(kt == 7))
                    nc.scalar.activation(out=y3[mt][:, sl], in_=ps, func=Act.Gelu,
                                         bias=col('c3b', mt))
                    if not bn3_triv:
                        nc.vector.tensor_scalar(out=y3[mt][:, sl], in0=y3[mt][:, sl],
                                                scalar1=col('g3p', mt),
                                                scalar2=col('b3p', mt), op0=MM, op1=AD)
                ln_stats_bounce(lambda ct: y3[ct][:, :], dstat2, ch, "ln2")
            ln_math(dstat2, drow2, "ln2")

            if probe:
                py3 = mkprobe("p_y3", [C, S])
                for i in range(2):
                    nc.gpsimd.dma_start(out=py3[128 * i:128 * (i + 1), :],
                                        in_=y3[i][:, :])

            # ============ phase G: LN2 apply + up-proj + shuffle-out =========
            uptb = wstr.tile([128, 2, 512], BF16, name="uptb", tag="wst9")
            nc.sync.dma_start(out=uptb, in_=d_up[:, :].rearrange("(kt p) m -> p kt m", p=128))
            for ch in range(NCH):
                pr, pm = ln_bcast(drow2, ch, "ln2")
                sl = slice(ch * CH, (ch + 1) * CH)
                for ct in range(2):
                    nc.vector.tensor_tensor(out=y3[ct][:, sl], in0=y3[ct][:, sl],
                                            in1=pr, op=MM)
                    nc.vector.tensor_tensor(out=y3[ct][:, sl], in0=y3[ct][:, sl],
                                            in1=pm, op=SU)
                if not ln2_triv:
                    for ct in range(2):
                        nc.vector.tensor_scalar(
                            out=y3[ct][:, sl], in0=y3[ct][:, sl],
                            scalar1=col('ln2w', ct), scalar2=col('ln2b', ct),
                            op0=MM, op1=AD)
                h0 = ch * 8
                for r in range(2):
                    ub = wsk.tile([128, 8, 64, 2], F32, name="ub", tag="skb")
                    for q in range(2):
                        rq = 2 * r + q
                        ps = psmm.tile([128, CH], F32, name="up_ps", tag="mm")
                        for kt in range(2):
                            nc.tensor.matmul(out=ps,
                                             lhsT=uptb[:, kt, 128 * rq:128 * (rq + 1)],
                                             rhs=y3[kt][:, sl],
                                             start=(kt == 0), stop=(kt == 1))
                        nc.scalar.activation(out=ub[:, :, :, q],
                                             in_=ps.rearrange("p (a b) -> p a b", a=8),
                                             func=Act.Identity, bias=col('upb', rq))
                    dst = view(yout[:, :, :], [[128 * 128, 128], [256, 8], [1, 128]],
                               off=(2 * h0 + r) * 128)
                    nc.sync.dma_start(out=dst, in_=ub.rearrange("p a b q -> p a (b q)"))

    nc.compile()
    return nc, const_inputs


def _get_nc(weights, probe=False):
    import hashlib
    hsh = hashlib.sha1()
    for k in sorted(weights):
        hsh.update(k.encode())
        hsh.update(np.ascontiguousarray(weights[k]).tobytes())
    key = (hsh.hexdigest(), probe)
    if key not in _CACHE:
        _CACHE[key] = _build(weights, probe=probe)
    return _CACHE[key]


def kernel(**inputs):
    from concourse.bass_utils import run_bass_kernel_spmd

    x = np.asarray(inputs['x'], np.float32)
    skip = np.asarray(inputs['skip'], np.float32)
    mask = np.asarray(inputs['saliency_mask'], np.float32)
    weights = {k: np.asarray(v, np.float32) for k, v in inputs.items()
               if k not in ('x', 'skip', 'saliency_mask')}

    probe = bool(os.environ.get('BASSK_PROBE'))
    nc, const_inputs = _get_nc(weights, probe=probe)

    in_maps = []
    for b in range(B):
        m = dict(
            xin=np.ascontiguousarray(x[b].reshape(C, S)),
            skin=np.ascontiguousarray(skip[b].reshape(CS, S)),
            mrow=np.ascontiguousarray(mask[b].reshape(1, S)),
        )
        m.update(const_inputs)
        in_maps.append(m)
    res = run_bass_kernel_spmd(nc, in_maps, core_ids=list(range(B)),
                               trace=bool(os.environ.get('BASSK_TRACE')))
    kernel.last_results = res
    out = np.stack([res.results[b]['yout'] for b in range(B)], axis=0)
    return out
